# revision 1
# baseline (speedup 1.0000x reference)
"""Trainium2 Bass kernel for nn_ConcatCharLSTM_LSTM_CRF.

Strategy (8 NeuronCores, SPMD, no collectives -- host does data movement
between three device launches):
  L1: char BiLSTM. Sequence time-chunked into 128 chunks/direction with a
      warmup window (LSTM forget-gate contraction makes chunk-boundary state
      errors decay below decision thresholds). 4 cores fwd + 4 cores bwd,
      32 lanes (chunks) per core batched into one instruction stream.
  L2: word BiLSTM, same scheme (128 chunks/dir, 32 lanes/core) + on-device
      embedding gather + input projections + hid2tag partial feats.
  L3: Viterbi forward scan (16 time-chunks as partition sublanes with warmup)
      + exact chunked backtrace via one-hot map composition, on 1 core.
"""

import os
import sys
import numpy as np
import time as _time

sys.path.insert(0, "/opt/trn_rl_repo")
os.environ.setdefault("JAX_PLATFORMS", "axon,cpu")

from concourse import bass, mybir
from concourse import bacc
import concourse.tile as tile
from concourse.bass_utils import run_bass_kernel_spmd
from concourse.masks import make_identity

F32 = mybir.dt.float32
I32 = mybir.dt.int32
AF = mybir.ActivationFunctionType
OP = mybir.AluOpType
AX = mybir.AxisListType

# problem constants
T, C, V, WD, CS, CD = 2048, 8192, 50000, 1024, 8000, 256
CH, WH = 128, 512            # per-direction hidden sizes
NEG = -10000.0

# chunking parameters
LC, LEN1, W1 = 32, 64, 64    # char: lanes/core, chunk len, warmup
S1 = LEN1 + W1               # char steps per core = 128
NR1 = LC * S1                # char rows per core = 4096
LW, LEN2, W2 = 32, 16, 64    # word
S2 = LEN2 + W2               # 80
WIN = 512 + W2               # word per-core column window = 576
NV, LV, WV = 16, 128, 32     # viterbi chunks, chunk len, warmup
SV = LV + WV                 # 160

# gate reorder: torch (i,f,g,o) -> (i,f,o,g) so sigmoid cols are contiguous
PERM = (0, 1, 3, 2)


def _reorder(w, H):
    """reorder gate blocks of leading dim 4H from (i,f,g,o) to (i,f,o,g)."""
    blocks = [w[i * H:(i + 1) * H] for i in range(4)]
    return np.concatenate([blocks[p] for p in PERM], axis=0)


def _ap(ap, dims, extra_off=0):
    """Build an AP with custom free dims [[step,count],...] keeping partition dim."""
    return bass.AP(ap.tensor, ap.offset + extra_off, [list(ap.ap[0])] + [list(d) for d in dims])


def _dap(ap, dims, extra_off=0):
    """Build an AP replacing ALL dims (for DRAM tensors)."""
    return bass.AP(ap.tensor, ap.offset + extra_off, [list(d) for d in dims])


def _new_nc(num_devices):
    return bacc.Bacc("TRN2", target_bir_lowering=False, debug=False,
                     num_devices=num_devices)


# ---------------------------------------------------------------- L1: char
def build_l1():
    nc = _new_nc(8)
    tbl = nc.dram_tensor("tbl", [CS, CD], F32, kind="ExternalInput")
    idx = nc.dram_tensor("idx", [NR1, 1], I32, kind="ExternalInput")
    wihT = nc.dram_tensor("wihT", [CD, 4 * CH], F32, kind="ExternalInput")
    whhT = nc.dram_tensor("whhT", [CH, 4 * CH], F32, kind="ExternalInput")
    biasT = nc.dram_tensor("biasT", [128, 4], F32, kind="ExternalInput")
    maskH = nc.dram_tensor("maskH", [128, LC], F32, kind="ExternalInput")
    fillH = nc.dram_tensor("fillH", [128, LC], F32, kind="ExternalInput")
    fillC = nc.dram_tensor("fillC", [128, LC], F32, kind="ExternalInput")
    hout = nc.dram_tensor("hout", [128, LEN1 * LC], F32, kind="ExternalOutput")

    with tile.TileContext(nc) as tc:
        with tc.tile_pool(name="p", bufs=1) as pp, \
             tc.tile_pool(name="ps", bufs=2, space="PSUM") as psp, \
             tc.tile_pool(name="tmp", bufs=2) as tp:
            ident = pp.tile([128, 128], F32)
            make_identity(nc, ident[:])
            idxs = pp.tile([128, NR1 // 128], I32)
            nc.sync.dma_start(idxs[:].rearrange("p (j o) -> p j o", j=NR1 // 128),
                              idx[:].rearrange("(j p) o -> p j o", p=128))
            Xc = pp.tile([128, (NR1 // 128) * CD], F32)
            for j in range(NR1 // 128):
                nc.gpsimd.indirect_dma_start(
                    out=Xc[:, j * CD:(j + 1) * CD], out_offset=None,
                    in_=tbl[:], in_offset=bass.IndirectOffsetOnAxis(ap=idxs[:, j:j + 1], axis=0))
            # transpose X -> XT [128, 2*NR1]  (dim-chunk major)
            XT = pp.tile([128, 2 * NR1], F32)
            for j in range(NR1 // 128):
                for d in range(2):
                    pst = psp.tile([128, 128], F32, tag="tps", space="PSUM")
                    nc.tensor.transpose(out=pst[:], in_=Xc[:, j * CD + d * 128: j * CD + d * 128 + 128],
                                        identity=ident[:])
                    nc.vector.tensor_copy(out=XT[:, d * NR1 + j * 128: d * NR1 + (j + 1) * 128], in_=pst[:])
            # bulk xproj: xpT [128, 4*NR1] (gate-chunk major)
            wih_s = pp.tile([128, 2 * 4 * CH], F32)
            nc.sync.dma_start(wih_s[:].rearrange("p (k g) -> p k g", k=2),
                              wihT[:].rearrange("(k p) g -> p k g", p=128))
            bias_s = pp.tile([128, 4], F32)
            nc.sync.dma_start(bias_s[:], biasT[:])
            xpT = pp.tile([128, 4 * NR1], F32)
            for g in range(4):
                for cb in range(NR1 // 512):
                    psx = psp.tile([128, 512], F32, tag="psx", space="PSUM")
                    for k in range(2):
                        nc.tensor.matmul(out=psx[:], lhsT=wih_s[:, k * 512 + g * 128: k * 512 + (g + 1) * 128],
                                         rhs=XT[:, k * NR1 + cb * 512: k * NR1 + (cb + 1) * 512],
                                         start=(k == 0), stop=(k == 1))
                    nc.vector.tensor_tensor(out=xpT[:, g * NR1 + cb * 512: g * NR1 + (cb + 1) * 512],
                                            in0=psx[:], in1=bias_s[:, g:g + 1].to_broadcast([128, 512]),
                                            op=OP.add)
            # scan
            whh_s = pp.tile([128, 4 * CH], F32)
            nc.sync.dma_start(whh_s[:], whhT[:])
            mH = pp.tile([128, LC], F32)
            fH = pp.tile([128, LC], F32)
            fC = pp.tile([128, LC], F32)
            nc.sync.dma_start(mH[:], maskH[:])
            nc.sync.dma_start(fH[:], fillH[:])
            nc.sync.dma_start(fC[:], fillC[:])
            hh = pp.tile([128, (S1 + 1) * LC], F32)
            cst = pp.tile([128, LC], F32)
            nc.vector.memset(hh[:, 0:LC], 0.0)
            nc.vector.memset(cst[:], 0.0)
            for t in range(S1):
                gps = psp.tile([128, 4 * LC], F32, tag="g", space="PSUM")
                for g in range(4):
                    nc.tensor.matmul(out=gps[:, g * LC:(g + 1) * LC],
                                     lhsT=whh_s[:, g * 128:(g + 1) * 128],
                                     rhs=hh[:, t * LC:(t + 1) * LC],
                                     start=(g == 0), stop=(g == 3))
                G = tp.tile([128, 4 * LC], F32, tag="G")
                nc.vector.tensor_tensor(
                    out=_ap(G[:], [[LC, 4], [1, LC]]),
                    in0=_ap(gps[:], [[LC, 4], [1, LC]]),
                    in1=_ap(xpT[:], [[NR1, 4], [S1, LC]], extra_off=t),
                    op=OP.add)
                Ssig = tp.tile([128, 3 * LC], F32, tag="S")
                nc.scalar.activation(out=Ssig[:], in_=G[:, 0:3 * LC], func=AF.Sigmoid)
                Tg = tp.tile([128, LC], F32, tag="Tg")
                nc.scalar.activation(out=Tg[:], in_=G[:, 3 * LC:4 * LC], func=AF.Tanh)
                t1 = tp.tile([128, LC], F32, tag="t1")
                nc.vector.tensor_tensor(out=t1[:], in0=Ssig[:, 0:LC], in1=Tg[:], op=OP.mult)
                nc.vector.tensor_tensor(out=cst[:], in0=Ssig[:, LC:2 * LC], in1=cst[:], op=OP.mult)
                nc.vector.tensor_tensor(out=cst[:], in0=cst[:], in1=t1[:], op=OP.add)
                Tc = tp.tile([128, LC], F32, tag="Tc")
                nc.scalar.activation(out=Tc[:], in_=cst[:], func=AF.Tanh)
                nc.vector.tensor_tensor(out=hh[:, (t + 1) * LC:(t + 2) * LC],
                                        in0=Ssig[:, 2 * LC:3 * LC], in1=Tc[:], op=OP.mult)
                if t == W1 - 1:
                    blk = hh[:, (t + 1) * LC:(t + 2) * LC]
                    nc.vector.tensor_tensor(out=blk, in0=blk, in1=mH[:], op=OP.mult)
                    nc.vector.tensor_tensor(out=blk, in0=blk, in1=fH[:], op=OP.add)
                    nc.vector.tensor_tensor(out=cst[:], in0=cst[:], in1=mH[:], op=OP.mult)
                    nc.vector.tensor_tensor(out=cst[:], in0=cst[:], in1=fC[:], op=OP.add)
            nc.sync.dma_start(hout[:], hh[:, (W1 + 1) * LC:(S1 + 1) * LC])
    nc.compile()
    return nc


# ---------------------------------------------------------------- L2: word
def build_l2():
    nc = _new_nc(8)
    NWG = 5 * 128  # padded gather rows (640 >= WIN)
    tbl = nc.dram_tensor("tbl", [V, WD], F32, kind="ExternalInput")
    widx = nc.dram_tensor("widx", [NWG, 1], I32, kind="ExternalInput")
    cfT = nc.dram_tensor("cfT", [512, WIN], F32, kind="ExternalInput")
    wihTwe = nc.dram_tensor("wihTwe", [WD, 4 * WH], F32, kind="ExternalInput")
    wihTcf = nc.dram_tensor("wihTcf", [512, 4 * WH], F32, kind="ExternalInput")
    whhT = nc.dram_tensor("whhT", [WH, 4 * WH], F32, kind="ExternalInput")
    biasT = nc.dram_tensor("biasT", [128, 16], F32, kind="ExternalInput")
    maskH = nc.dram_tensor("maskH", [128, 4 * LW], F32, kind="ExternalInput")
    fillH = nc.dram_tensor("fillH", [128, 4 * LW], F32, kind="ExternalInput")
    fillC = nc.dram_tensor("fillC", [128, 4 * LW], F32, kind="ExternalInput")
    h2tT = nc.dram_tensor("h2tT", [WH, 6], F32, kind="ExternalInput")
    bias6 = nc.dram_tensor("bias6", [128, 6], F32, kind="ExternalInput")
    fpart = nc.dram_tensor("fpart", [512, 6], F32, kind="ExternalOutput")

    with tile.TileContext(nc) as tc:
        with tc.tile_pool(name="p", bufs=1) as pp, \
             tc.tile_pool(name="ps", bufs=2, space="PSUM") as psp, \
             tc.tile_pool(name="tmp", bufs=2) as tp:
            bias_s = pp.tile([128, 16], F32)
            nc.sync.dma_start(bias_s[:], biasT[:])
            xpT = pp.tile([128, 16 * WIN], F32)
            # phase a: word-embedding part of xproj
            with tc.tile_pool(name="wih", bufs=1) as wp:
                ident = wp.tile([128, 128], F32)
                make_identity(nc, ident[:])
                idxs = wp.tile([128, 5], I32)
                nc.sync.dma_start(idxs[:].rearrange("p (j o) -> p j o", j=5),
                                  widx[:].rearrange("(j p) o -> p j o", p=128))
                embT = wp.tile([128, 8 * 640], F32)
                for j in range(5):
                    Xw = wp.tile([128, WD], F32, tag="Xw")
                    nc.gpsimd.indirect_dma_start(
                        out=Xw[:], out_offset=None,
                        in_=tbl[:], in_offset=bass.IndirectOffsetOnAxis(ap=idxs[:, j:j + 1], axis=0))
                    for d in range(8):
                        pst = psp.tile([128, 128], F32, tag="tps", space="PSUM")
                        nc.tensor.transpose(out=pst[:], in_=Xw[:, d * 128:(d + 1) * 128],
                                            identity=ident[:])
                        nc.vector.tensor_copy(out=embT[:, d * 640 + j * 128: d * 640 + (j + 1) * 128], in_=pst[:])
                cf_s = wp.tile([128, 4 * WIN], F32)
                nc.sync.dma_start(cf_s[:].rearrange("p (k w) -> p k w", k=4),
                                  cfT[:].rearrange("(k p) w -> p k w", p=128))
                for half in range(2):
                    wih_s = wp.tile([128, 4 * 4 * WH], F32, tag="wih")
                    src = wihTwe[half * 512:(half + 1) * 512, :]
                    nc.sync.dma_start(wih_s[:].rearrange("p (k g) -> p k g", k=4),
                                      src.rearrange("(k p) g -> p k g", p=128))
                    for g in range(16):
                        for cb in range(2):
                            c0 = cb * 288
                            cw = 288 if cb == 0 else WIN - 288
                            psx = psp.tile([128, 288], F32, tag="psx", space="PSUM")
                            for k in range(4):
                                nc.tensor.matmul(out=psx[:, :cw],
                                                 lhsT=wih_s[:, k * 2048 + g * 128: k * 2048 + (g + 1) * 128],
                                                 rhs=embT[:, (half * 4 + k) * 640 + c0: (half * 4 + k) * 640 + c0 + cw],
                                                 start=(k == 0), stop=(k == 3))
                            dst = xpT[:, g * WIN + c0: g * WIN + c0 + cw]
                            if half == 0:
                                nc.vector.tensor_tensor(out=dst, in0=psx[:, :cw],
                                                        in1=bias_s[:, g:g + 1].to_broadcast([128, cw]),
                                                        op=OP.add)
                            else:
                                nc.vector.tensor_tensor(out=dst, in0=dst, in1=psx[:, :cw], op=OP.add)
                # phase b: char-feat part accumulated on top
                wih2 = wp.tile([128, 4 * 4 * WH], F32, tag="wih")
                nc.sync.dma_start(wih2[:].rearrange("p (k g) -> p k g", k=4),
                                  wihTcf[:].rearrange("(k p) g -> p k g", p=128))
                for g in range(16):
                    for cb in range(2):
                        c0 = cb * 288
                        cw = 288 if cb == 0 else WIN - 288
                        psx = psp.tile([128, 288], F32, tag="psx", space="PSUM")
                        for k in range(4):
                            nc.tensor.matmul(out=psx[:, :cw],
                                             lhsT=wih2[:, k * 2048 + g * 128: k * 2048 + (g + 1) * 128],
                                             rhs=cf_s[:, k * WIN + c0: k * WIN + c0 + cw],
                                             start=(k == 0), stop=(k == 3))
                        dst = xpT[:, g * WIN + c0: g * WIN + c0 + cw]
                        nc.vector.tensor_tensor(out=dst, in0=dst, in1=psx[:, :cw], op=OP.add)
            # scan
            whh_s = pp.tile([128, 4 * 4 * WH], F32)
            nc.sync.dma_start(whh_s[:].rearrange("p (k g) -> p k g", k=4),
                              whhT[:].rearrange("(k p) g -> p k g", p=128))
            mH = pp.tile([128, 4 * LW], F32)
            fH = pp.tile([128, 4 * LW], F32)
            fC = pp.tile([128, 4 * LW], F32)
            nc.sync.dma_start(mH[:], maskH[:])
            nc.sync.dma_start(fH[:], fillH[:])
            nc.sync.dma_start(fC[:], fillC[:])
            hh = pp.tile([128, (S2 + 1) * 4 * LW], F32)
            cst = pp.tile([128, 4 * LW], F32)
            nc.vector.memset(hh[:, 0:4 * LW], 0.0)
            nc.vector.memset(cst[:], 0.0)
            for t in range(S2):
                gps = psp.tile([128, 16 * LW], F32, tag="g", space="PSUM")
                for m in range(16):
                    for k in range(4):
                        nc.tensor.matmul(out=gps[:, m * LW:(m + 1) * LW],
                                         lhsT=whh_s[:, k * 2048 + m * 128: k * 2048 + (m + 1) * 128],
                                         rhs=hh[:, t * 4 * LW + k * LW: t * 4 * LW + (k + 1) * LW],
                                         start=(k == 0), stop=(k == 3))
                G = tp.tile([128, 16 * LW], F32, tag="G")
                nc.vector.tensor_tensor(
                    out=_ap(G[:], [[LW, 16], [1, LW]]),
                    in0=_ap(gps[:], [[LW, 16], [1, LW]]),
                    in1=_ap(xpT[:], [[WIN, 16], [LEN2, LW]], extra_off=t),
                    op=OP.add)
                Ssig = tp.tile([128, 12 * LW], F32, tag="S")
                nc.scalar.activation(out=Ssig[:], in_=G[:, 0:12 * LW], func=AF.Sigmoid)
                Tg = tp.tile([128, 4 * LW], F32, tag="Tg")
                nc.scalar.activation(out=Tg[:], in_=G[:, 12 * LW:16 * LW], func=AF.Tanh)
                t1 = tp.tile([128, 4 * LW], F32, tag="t1")
                nc.vector.tensor_tensor(out=t1[:], in0=Ssig[:, 0:4 * LW], in1=Tg[:], op=OP.mult)
                nc.vector.tensor_tensor(out=cst[:], in0=Ssig[:, 4 * LW:8 * LW], in1=cst[:], op=OP.mult)
                nc.vector.tensor_tensor(out=cst[:], in0=cst[:], in1=t1[:], op=OP.add)
                Tc = tp.tile([128, 4 * LW], F32, tag="Tc")
                nc.scalar.activation(out=Tc[:], in_=cst[:], func=AF.Tanh)
                nc.vector.tensor_tensor(out=hh[:, (t + 1) * 4 * LW:(t + 2) * 4 * LW],
                                        in0=Ssig[:, 8 * LW:12 * LW], in1=Tc[:], op=OP.mult)
                if t == W2 - 1:
                    blk = hh[:, (t + 1) * 4 * LW:(t + 2) * 4 * LW]
                    nc.vector.tensor_tensor(out=blk, in0=blk, in1=mH[:], op=OP.mult)
                    nc.vector.tensor_tensor(out=blk, in0=blk, in1=fH[:], op=OP.add)
                    nc.vector.tensor_tensor(out=cst[:], in0=cst[:], in1=mH[:], op=OP.mult)
                    nc.vector.tensor_tensor(out=cst[:], in0=cst[:], in1=fC[:], op=OP.add)
            # repack post-warmup h (t-major) then feats partial
            hT = pp.tile([128, 4 * 512], F32)
            for k in range(4):
                nc.vector.tensor_copy(
                    out=_ap(hT[:], [[16, 32], [1, 16]], extra_off=k * 512),
                    in_=_ap(hh[:], [[1, 32], [4 * LW, 16]],
                            extra_off=(W2 + 1) * 4 * LW + k * LW))
            h2t_s = pp.tile([128, 4 * 6], F32)
            nc.sync.dma_start(h2t_s[:].rearrange("p (k s) -> p k s", k=4),
                              h2tT[:].rearrange("(k p) s -> p k s", p=128))
            b6_s = pp.tile([128, 6], F32)
            nc.sync.dma_start(b6_s[:], bias6[:])
            fp_s = pp.tile([128, 4 * 6], F32)
            for m in range(4):
                psf = psp.tile([128, 6], F32, tag="psf", space="PSUM")
                for k in range(4):
                    nc.tensor.matmul(out=psf[:],
                                     lhsT=hT[:, k * 512 + m * 128: k * 512 + (m + 1) * 128],
                                     rhs=h2t_s[:, k * 6:(k + 1) * 6],
                                     start=(k == 0), stop=(k == 3))
                nc.vector.tensor_tensor(out=fp_s[:, m * 6:(m + 1) * 6], in0=psf[:], in1=b6_s[:], op=OP.add)
            nc.sync.dma_start(fpart[:].rearrange("(m p) s -> p m s", p=128),
                              fp_s[:].rearrange("p (m s) -> p m s", m=4))
    nc.compile()
    return nc


# ---------------------------------------------------------------- L3: viterbi
def build_l3():
    nc = _new_nc(1)
    fstack = nc.dram_tensor("fstack", [8 * 512, 6], F32, kind="ExternalInput")
    transR = nc.dram_tensor("transR", [16, 36], F32, kind="ExternalInput")
    iotaM = nc.dram_tensor("iotaM", [16, 36], F32, kind="ExternalInput")
    maskV = nc.dram_tensor("maskV", [16, 6], F32, kind="ExternalInput")
    fillV = nc.dram_tensor("fillV", [16, 6], F32, kind="ExternalInput")
    tstop = nc.dram_tensor("tstop", [16, 6], F32, kind="ExternalInput")
    iotaI = nc.dram_tensor("iotaI", [96, 36], F32, kind="ExternalInput")
    iotaJ = nc.dram_tensor("iotaJ", [96, 768], F32, kind="ExternalInput")
    uinit = nc.dram_tensor("uinit", [96, 6], F32, kind="ExternalInput")
    bmask = nc.dram_tensor("bmask", [96, 16], F32, kind="ExternalInput")
    ids_o = nc.dram_tensor("ids_o", [T], I32, kind="ExternalOutput")

    with tile.TileContext(nc) as tc:
        with tc.tile_pool(name="p", bufs=1) as pp, \
             tc.tile_pool(name="ps", bufs=2, space="PSUM") as psp, \
             tc.tile_pool(name="d", bufs=1, space="DRAM") as dp, \
             tc.tile_pool(name="tmp", bufs=2) as tp:
            # sum the 8 partial feats
            Ff = pp.tile([128, 16 * 6], F32)
            Fb = pp.tile([128, 16 * 6], F32)
            for k in range(4):
                nc.sync.dma_start(Ff[32 * k:32 * (k + 1), :],
                                  fstack[:].rearrange("(c p a) s -> c p a s", c=8, p=32)[k])
                nc.sync.dma_start(Fb[32 * k:32 * (k + 1), :],
                                  fstack[:].rearrange("(c p a) s -> c p a s", c=8, p=32)[4 + k])
            F = pp.tile([128, 16 * 6], F32)
            nc.vector.tensor_tensor(out=F[:], in0=Ff[:], in1=Fb[:], op=OP.add)
            featsD = dp.tile([T * 6], F32)
            nc.sync.dma_start(featsD[:].rearrange("(p a) -> p a", p=128), F[:])
            # stage per-sublane feats windows
            fsub = pp.tile([16, SV * 6], F32)
            fD = featsD[:]
            for p in range(16):
                if p == 0:
                    nc.sync.dma_start(fsub[0:1, 0:WV * 6], _dap(fD, [[WV * 6, 1], [1, WV * 6]]))
                    nc.sync.dma_start(fsub[0:1, WV * 6:SV * 6], _dap(fD, [[LV * 6, 1], [1, LV * 6]]))
                else:
                    nc.sync.dma_start(fsub[p:p + 1, :],
                                      _dap(fD, [[SV * 6, 1], [1, SV * 6]], extra_off=(p * LV - WV) * 6))
            trR = pp.tile([16, 36], F32)
            ioM = pp.tile([16, 36], F32)
            mV = pp.tile([16, 6], F32)
            fV = pp.tile([16, 6], F32)
            tS = pp.tile([16, 6], F32)
            for dst, src in ((trR, transR), (ioM, iotaM), (mV, maskV), (fV, fillV), (tS, tstop)):
                nc.sync.dma_start(dst[:], src[:])
            fv = pp.tile([16, 6], F32)
            nc.vector.memset(fv[:], 0.0)
            bpsH = pp.tile([16, LV * 6], F32)
            for t in range(SV):
                if t == WV:
                    nc.vector.tensor_tensor(out=fv[:], in0=fv[:], in1=mV[:], op=OP.mult)
                    nc.vector.tensor_tensor(out=fv[:], in0=fv[:], in1=fV[:], op=OP.add)
                tmp = tp.tile([16, 36], F32, tag="tmp")
                nc.vector.tensor_tensor(out=_ap(tmp[:], [[6, 6], [1, 6]]),
                                        in0=_ap(trR[:], [[6, 6], [1, 6]]),
                                        in1=_ap(fv[:], [[0, 6], [1, 6]]), op=OP.add)
                mx = tp.tile([16, 6], F32, tag="mx")
                nc.vector.tensor_reduce(out=mx[:], in_=_ap(tmp[:], [[6, 6], [1, 6]]),
                                        axis=AX.X, op=OP.max)
                eq = tp.tile([16, 36], F32, tag="eq")
                nc.vector.tensor_tensor(out=_ap(eq[:], [[6, 6], [1, 6]]),
                                        in0=_ap(tmp[:], [[6, 6], [1, 6]]),
                                        in1=_ap(mx[:], [[1, 6], [0, 6]]), op=OP.is_ge)
                nc.vector.tensor_tensor(out=eq[:], in0=eq[:], in1=ioM[:], op=OP.mult)
                if t >= WV:
                    nc.vector.tensor_reduce(out=bpsH[:, (t - WV) * 6:(t - WV + 1) * 6],
                                            in_=_ap(eq[:], [[6, 6], [1, 6]]), axis=AX.X, op=OP.min)
                nc.vector.tensor_tensor(out=fv[:], in0=mx[:], in1=fsub[:, t * 6:(t + 1) * 6], op=OP.add)
            # last-tag onehot
            av = pp.tile([16, 6], F32)
            nc.vector.tensor_tensor(out=av[:], in0=fv[:], in1=tS[:], op=OP.add)
            am = pp.tile([16, 1], F32)
            nc.vector.tensor_reduce(out=am[:], in_=av[:], axis=AX.X, op=OP.max)
            ohf = pp.tile([16, 6], F32)
            nc.vector.tensor_tensor(out=ohf[:], in0=av[:], in1=am[:].to_broadcast([16, 6]), op=OP.is_ge)
            # replicate bps to 96 partitions
            bpsD = dp.tile([16 * LV * 6], F32)
            nc.sync.dma_start(bpsD[:].rearrange("(p a) -> p a", p=16), bpsH[:])
            bpsR = pp.tile([96, LV * 6], F32)
            for e in range(6):
                nc.sync.dma_start(bpsR[16 * e:16 * (e + 1), :],
                                  bpsD[:].rearrange("(p a) -> p a", p=16))
            ioI = pp.tile([96, 36], F32)
            ioJ = pp.tile([96, 768], F32)
            uI = pp.tile([96, 6], F32)
            bM = pp.tile([96, 16], F32)
            for dst, src in ((ioI, iotaI), (ioJ, iotaJ), (uI, uinit), (bM, bmask)):
                nc.sync.dma_start(dst[:], src[:])
            uH = pp.tile([96, (LV + 1) * 6], F32)
            nc.vector.tensor_copy(out=uH[:, LV * 6:(LV + 1) * 6], in_=uI[:])
            for tb in range(LV - 1, -1, -1):
                eqB = tp.tile([96, 36], F32, tag="eqB")
                nc.vector.tensor_tensor(out=_ap(eqB[:], [[6, 6], [1, 6]]),
                                        in0=_ap(bpsR[:], [[0, 6], [1, 6]], extra_off=tb * 6),
                                        in1=_ap(ioI[:], [[6, 6], [1, 6]]), op=OP.is_equal)
                tB = tp.tile([96, 36], F32, tag="tB")
                nc.vector.tensor_tensor(out=_ap(tB[:], [[6, 6], [1, 6]]),
                                        in0=_ap(eqB[:], [[6, 6], [1, 6]]),
                                        in1=_ap(uH[:], [[0, 6], [1, 6]], extra_off=(tb + 1) * 6),
                                        op=OP.mult)
                nc.vector.tensor_reduce(out=uH[:, tb * 6:(tb + 1) * 6],
                                        in_=_ap(tB[:], [[6, 6], [1, 6]]), axis=AX.X, op=OP.max)
            # decode ids for all hypotheses
            idsA = pp.tile([96, LV], F32)
            tJ = pp.tile([96, 768], F32)
            nc.vector.tensor_tensor(out=tJ[:], in0=uH[:, 6:(LV + 1) * 6], in1=ioJ[:], op=OP.mult)
            nc.vector.tensor_reduce(out=idsA[:], in_=_ap(tJ[:], [[6, LV], [1, 6]]), axis=AX.X, op=OP.max)
            # chunk maps flattened onto ONE partition: MT2 [1, 16*36] flat (c,j,e)
            uD = dp.tile([96 * 6], F32)
            nc.sync.dma_start(uD[:].rearrange("(p a) -> p a", p=96), uH[:, 0:6])
            MT2 = pp.tile([1, 16 * 36], F32)
            nc.sync.dma_start(MT2[:], _dap(uD[:], [[576, 1], [6, 16], [1, 6], [96, 6]]))
            # move last-tag onehot (row 15 of ohf) to partition 0
            ohfD = dp.tile([16 * 6], F32)
            nc.sync.dma_start(ohfD[:].rearrange("(p a) -> p a", p=16), ohf[:])
            # stitch on partition 0: ohSeq[:, c*6+e] = onehot(ids at end of chunk c)
            ohSeq = pp.tile([1, 16 * 6], F32)
            nc.sync.dma_start(ohSeq[0:1, 15 * 6:16 * 6],
                              _dap(ohfD[:], [[6, 1], [1, 6]], extra_off=15 * 6))
            for c in range(14, -1, -1):
                tS2 = tp.tile([1, 36], F32, tag="tS2")
                nc.vector.tensor_tensor(out=_ap(tS2[:], [[6, 6], [1, 6]]),
                                        in0=_ap(MT2[:], [[6, 6], [1, 6]], extra_off=(c + 1) * 36),
                                        in1=_ap(ohSeq[:], [[0, 6], [1, 6]], extra_off=(c + 1) * 6),
                                        op=OP.mult)
                nc.vector.tensor_reduce(out=ohSeq[0:1, c * 6:(c + 1) * 6],
                                        in_=_ap(tS2[:], [[6, 6], [1, 6]]), axis=AX.X, op=OP.max)
            ohD = dp.tile([16 * 6], F32)
            nc.sync.dma_start(ohD[:].rearrange("(p a) -> p a", p=1), ohSeq[:])
            selC = pp.tile([96, 1], F32)
            for e in range(6):
                nc.sync.dma_start(selC[16 * e:16 * (e + 1), :],
                                  _dap(ohD[:], [[6, 16], [1, 1]], extra_off=e))
            SEL = pp.tile([96, 16], F32)
            nc.vector.tensor_tensor(out=SEL[:], in0=selC[:].to_broadcast([96, 16]), in1=bM[:], op=OP.mult)
            psi = psp.tile([16, LV], F32, tag="psi", space="PSUM")
            nc.tensor.matmul(out=psi[:], lhsT=SEL[:], rhs=idsA[:], start=True, stop=True)
            idsI = pp.tile([16, LV], I32)
            nc.vector.tensor_copy(out=idsI[:], in_=psi[:])
            nc.sync.dma_start(ids_o[:].rearrange("(p a) -> p a", p=16), idsI[:])
    nc.compile()
    return nc


# ---------------------------------------------------------------- host glue
_cache = {}


def _programs():
    if "l1" not in _cache:
        _cache["l1"] = build_l1()
        _cache["l2"] = build_l2()
        _cache["l3"] = build_l3()
    return _cache["l1"], _cache["l2"], _cache["l3"]


def kernel(**inp):
    inp = {k: np.asarray(v) for k, v in inp.items()}
    nc1, nc2, nc3 = _programs()
    perf = {}

    chars = inp["chars"].astype(np.int32)
    words = inp["words"].astype(np.int32)
    ix = inp["ix_seq"].astype(np.int64)

    # ---------------- L1 inputs
    in_maps1 = []
    for core in range(8):
        d = core // 4
        kk = core % 4
        suf = "f" if d == 0 else "b"
        Wih = _reorder(inp[f"c_Wih_{suf}"], CH)
        Whh = _reorder(inp[f"c_Whh_{suf}"], CH)
        bias = _reorder(inp[f"c_bih_{suf}"] + inp[f"c_bhh_{suf}"], CH)
        src = chars if d == 0 else chars[::-1]
        lanes = np.arange(LC) + LC * kk
        pos = (LEN1 * lanes[:, None] - W1 + np.arange(S1)[None, :]).clip(0, C - 1)
        idx = src[pos.reshape(-1)].astype(np.int32)[:, None]
        maskH = np.ones((128, LC), np.float32)
        fillH = np.zeros((128, LC), np.float32)
        fillC = np.zeros((128, LC), np.float32)
        if kk == 0:
            maskH[:, 0] = 0.0
            fillH[:, 0] = inp["c_h0"][d]
            fillC[:, 0] = inp["c_c0"][d]
        in_maps1.append({
            "tbl": inp["char_embed"].astype(np.float32),
            "idx": idx,
            "wihT": np.ascontiguousarray(Wih.T.astype(np.float32)),
            "whhT": np.ascontiguousarray(Whh.T.astype(np.float32)),
            "biasT": np.ascontiguousarray(bias.reshape(4, 128).T.astype(np.float32)),
            "maskH": maskH, "fillH": fillH, "fillC": fillC,
        })
    t0 = _time.time()
    r1 = run_bass_kernel_spmd(nc1, in_maps1, core_ids=list(range(8)),
                              trace=False, tmpdir=None)
    perf["l1_wall"] = _time.time() - t0
    # reassemble char hids: hout col = j*LC + l -> h at pos LEN1*(LC*kk+l)+j
    chf = np.zeros((C, CH), np.float32)
    chb = np.zeros((C, CH), np.float32)
    for core in range(8):
        h = r1.results[core]["hout"]  # [128, LEN1*LC]
        d, kk = core // 4, core % 4
        hv = h.reshape(CH, LEN1, LC)  # [hid, j, l]
        pos = LEN1 * (LC * kk + np.arange(LC))[None, :] + np.arange(LEN1)[:, None]
        if d == 0:
            chf[pos.reshape(-1)] = hv.reshape(CH, -1).T
        else:
            chb[C - 1 - pos.reshape(-1)] = hv.reshape(CH, -1).T
    starts, ends = ix[:-1], ix[1:] - 1
    char_feats = np.concatenate(
        [chf[starts], chb[starts], chf[ends], chb[ends]], axis=1)  # [T, 512]

    # ---------------- L2 inputs
    in_maps2 = []
    for core in range(8):
        d, kk = core // 4, core % 4
        suf = "f" if d == 0 else "b"
        Wih = _reorder(inp[f"w_Wih_{suf}"], WH)
        Whh = _reorder(inp[f"w_Whh_{suf}"], WH)
        bias = _reorder(inp[f"w_bih_{suf}"] + inp[f"w_bhh_{suf}"], WH)
        cf = char_feats if d == 0 else char_feats[::-1]
        wsrc = words if d == 0 else words[::-1]
        rows = (512 * kk - W2 + np.arange(WIN)).clip(0, T - 1)
        widx = np.zeros((640, 1), np.int32)
        widx[:WIN, 0] = wsrc[rows]
        maskH = np.ones((128, 4 * LW), np.float32)
        fillH = np.zeros((128, 4 * LW), np.float32)
        fillC = np.zeros((128, 4 * LW), np.float32)
        if kk == 0:
            for k in range(4):
                maskH[:, k * LW] = 0.0
                fillH[:, k * LW] = inp["w_h0"][d][k * 128:(k + 1) * 128]
                fillC[:, k * LW] = inp["w_c0"][d][k * 128:(k + 1) * 128]
        h2t = inp["hid2tag_W"][:, :WH] if d == 0 else inp["hid2tag_W"][:, WH:]
        b6 = np.zeros((128, 6), np.float32)
        if d == 0:
            b6[:] = inp["hid2tag_b"][None, :]
        # embeds = [char_feats | word_emb]: Wih cols 0:512 -> cf, 512: -> we
        in_maps2.append({
            "tbl": inp["word_embed"].astype(np.float32),
            "widx": widx,
            "cfT": np.ascontiguousarray(cf[rows].T.astype(np.float32)),
            "wihTwe": np.ascontiguousarray(Wih[:, 512:].T.astype(np.float32)),
            "wihTcf": np.ascontiguousarray(Wih[:, :512].T.astype(np.float32)),
            "whhT": np.ascontiguousarray(Whh.T.astype(np.float32)),
            "biasT": np.ascontiguousarray(bias.reshape(16, 128).T.astype(np.float32)),
            "maskH": maskH, "fillH": fillH, "fillC": fillC,
            "h2tT": np.ascontiguousarray(h2t.T.astype(np.float32)),
            "bias6": b6,
        })
    t0 = _time.time()
    r2 = run_bass_kernel_spmd(nc2, in_maps2, core_ids=list(range(8)),
                              trace=False, tmpdir=None)
    perf["l2_wall"] = _time.time() - t0
    fstack = np.zeros((8 * 512, 6), np.float32)
    for core in range(8):
        fp = r2.results[core]["fpart"]  # [512, 6] for global t block 512*kk
        d, kk = core // 4, core % 4
        if d == 0:
            fstack[512 * core:512 * (core + 1)] = fp
        else:
            # bwd cores computed feats on reversed t ordering
            fstack[512 * core:512 * (core + 1)] = fp[::-1]
    # bwd partials: core (4+kk) block covers reversed rows [512kk:512kk+512]
    # -> global t = T-1 - rev_t, i.e. global block [T-512(kk+1), T-512kk) reversed.
    # Reorder bwd section so that fstack[4*512 + t_local] = bwd partial at global t
    bsec = fstack[4 * 512:].copy()
    fstack[4 * 512:] = 0
    for kk in range(4):
        blk = bsec[512 * kk:512 * (kk + 1)]  # already reversed above -> ascending global t
        g0 = T - 512 * (kk + 1)
        fstack[4 * 512 + g0:4 * 512 + g0 + 512] = blk

    # ---------------- L3 inputs
    trans = inp["transition"].astype(np.float32)
    transR = np.tile(trans.reshape(1, 36), (16, 1)).astype(np.float32)
    ii, jj = np.meshgrid(np.arange(6), np.arange(6), indexing="ij")  # flat j*6+i? see below
    # tmp flat index = j*6 + i ; iotaM value = (i - 6)
    iotaM = np.tile((np.arange(36) % 6 - 6).astype(np.float32)[None, :], (16, 1))
    maskV = np.ones((16, 6), np.float32)
    maskV[0] = 0.0
    fillV = np.zeros((16, 6), np.float32)
    fv0 = np.full(6, NEG, np.float32)
    fv0[4] = 0.0
    fillV[0] = fv0
    tstop = np.tile(trans[:, 5][None, :], (16, 1)).astype(np.float32)
    # backtrace consts: flat index = i*6 + j ; value (i - 6)
    iotaI = np.tile((np.arange(36) // 6 - 6).astype(np.float32)[None, :], (96, 1))
    iotaJ = np.tile((np.arange(768) % 6).astype(np.float32)[None, :], (96, 1))
    uinit = np.zeros((96, 6), np.float32)
    for e in range(6):
        uinit[16 * e:16 * (e + 1), e] = 1.0
    bmask = np.zeros((96, 16), np.float32)
    for e in range(6):
        for c in range(16):
            bmask[16 * e + c, c] = 1.0
    in_map3 = {
        "fstack": fstack, "transR": transR, "iotaM": iotaM, "maskV": maskV,
        "fillV": fillV, "tstop": tstop, "iotaI": iotaI, "iotaJ": iotaJ,
        "uinit": uinit, "bmask": bmask,
    }
    t0 = _time.time()
    r3 = run_bass_kernel_spmd(nc3, [in_map3], core_ids=[0],
                              trace=False, tmpdir=None)
    perf["l3_wall"] = _time.time() - t0
    kernel.last_perf = perf
    return r3.results[0]["ids_o"].astype(np.int32)


kernel.last_perf = {}



# revision 6
# speedup vs baseline: 32.1284x; 32.1284x over previous
"""Trainium2 Bass kernel for nn_ConcatCharLSTM_LSTM_CRF.

Strategy (8 NeuronCores, SPMD, three device launches; host does the
inter-launch data movement). The dominant cost through this runtime path is
host->device transfer bandwidth (~35 MB/s effective), so the design
minimizes shipped bytes:
  - embeddings are gathered on host (pure data movement) and shipped as the
    per-core windows actually consumed, in fp16;
  - LSTM weights ship in fp16; the word-LSTM weight pack is sharded 4-ways
    per direction and reconstructed on-device with an AllGather collective;
  - matmuls against shipped fp16 data run in fp16 (fp32 PSUM accumulate);
    the recurrent path stays fp32.
Layers:
  L1: char BiLSTM. Per core: 32 chunk-lanes x 64 steps + 64 warmup steps
      (LSTM forget-gate contraction decays chunk-boundary state error).
      4 cores fwd + 4 bwd. Compact window: 2112 = 64 + 32*64 input columns,
      scan indexes lane l step t at column 64*l + t.
  L2: word BiLSTM, same scheme (512 cols + 64 warmup per core) + hid2tag
      partial feats.
  L3: Viterbi forward scan (16 time-chunks as partition sublanes with
      warmup) + exact chunked backtrace via one-hot map composition, 1 core.
"""

import os
import sys
import numpy as np
import time as _time

sys.path.insert(0, "/opt/trn_rl_repo")
os.environ.setdefault("JAX_PLATFORMS", "axon,cpu")
os.environ.setdefault("JAX_COMPILATION_CACHE_DIR", "/root/.cache/jax_bass")

from concourse import bass, mybir
from concourse import bacc
import concourse.tile as tile
from concourse.bass_utils import run_bass_kernel_spmd

try:
    import jax
    jax.config.update("jax_compilation_cache_dir", "/root/.cache/jax_bass")
    jax.config.update("jax_persistent_cache_min_entry_size_bytes", -1)
    jax.config.update("jax_persistent_cache_min_compile_time_secs", 0.0)
except Exception:
    pass

F32 = mybir.dt.float32
F16 = mybir.dt.float16
I32 = mybir.dt.int32
AF = mybir.ActivationFunctionType
OP = mybir.AluOpType
AX = mybir.AxisListType

# problem constants
T, C, V, WD, CS, CD = 2048, 8192, 50000, 1024, 8000, 256
CH, WH = 128, 512            # per-direction hidden sizes
NEG = -10000.0

# chunking parameters
LC, LEN1, W1 = 32, 64, 64    # char: lanes/core, chunk len, warmup
S1 = LEN1 + W1               # char steps per core = 128
NC1 = LEN1 * LC + W1         # compact char cols per core = 2112
LW, LEN2, W2 = 32, 16, 64    # word
S2 = LEN2 + W2               # 80
WIN = LEN2 * LW + W2         # word per-core column window = 576
NV, LV, WV = 16, 128, 32     # viterbi chunks, chunk len, warmup
SV = LV + WV                 # 160

# gate reorder: torch (i,f,g,o) -> (i,f,o,g) so sigmoid cols are contiguous
PERM = (0, 1, 3, 2)


def _reorder(w, H):
    """reorder gate blocks of leading dim 4H from (i,f,g,o) to (i,f,o,g)."""
    blocks = [w[i * H:(i + 1) * H] for i in range(4)]
    return np.concatenate([blocks[p] for p in PERM], axis=0)


def _chunkT(a, nk):
    """[R, G] -> [128, nk*G] where col k*G+g holds a[k*128+p, g] (R = nk*128)."""
    R, G = a.shape
    assert R == nk * 128
    return np.ascontiguousarray(
        a.reshape(nk, 128, G).transpose(1, 0, 2).reshape(128, nk * G))


def _ap(ap, dims, extra_off=0):
    """Build an AP with custom free dims [[step,count],...] keeping partition dim."""
    return bass.AP(ap.tensor, ap.offset + extra_off, [list(ap.ap[0])] + [list(d) for d in dims])


def _dap(ap, dims, extra_off=0):
    """Build an AP replacing ALL dims (for DRAM tensors)."""
    return bass.AP(ap.tensor, ap.offset + extra_off, [list(d) for d in dims])


def _new_nc(num_devices):
    return bacc.Bacc("TRN2", target_bir_lowering=False, debug=False,
                     num_devices=num_devices)


# ---------------------------------------------------------------- L1: char
def build_l1():
    nc = _new_nc(8)
    xt16 = nc.dram_tensor("xt16", [128, 2 * NC1], F16, kind="ExternalInput")
    wih16 = nc.dram_tensor("wih16", [128, 2 * 4 * CH], F16, kind="ExternalInput")
    whh16 = nc.dram_tensor("whh16", [128, 4 * CH], F16, kind="ExternalInput")
    cpk = nc.dram_tensor("cpk", [128, 4 + 3 * LC], F32, kind="ExternalInput")
    hout = nc.dram_tensor("hout", [128, LEN1 * LC], F16, kind="ExternalOutput")

    with tile.TileContext(nc) as tc:
        with tc.tile_pool(name="p", bufs=1) as pp, \
             tc.tile_pool(name="ps", bufs=2, space="PSUM") as psp, \
             tc.tile_pool(name="tmp", bufs=2) as tp:
            XT = pp.tile([128, 2 * NC1], F16)
            nc.sync.dma_start(XT[:], xt16[:])
            wih_s = pp.tile([128, 2 * 4 * CH], F16)
            nc.sync.dma_start(wih_s[:], wih16[:])
            cpk_s = pp.tile([128, 4 + 3 * LC], F32)
            nc.sync.dma_start(cpk_s[:], cpk[:])
            mH = cpk_s[:, 4:4 + LC]
            fH = cpk_s[:, 4 + LC:4 + 2 * LC]
            fC = cpk_s[:, 4 + 2 * LC:4 + 3 * LC]
            # bulk xproj: xpT [128, 4*NC1] (gate-chunk major)
            xpT = pp.tile([128, 4 * NC1], F32)
            blocks = [(0, 512), (512, 512), (1024, 512), (1536, 512), (2048, 64)]
            for g in range(4):
                for (c0, cw) in blocks:
                    psx = psp.tile([128, 512], F32, tag="psx", space="PSUM")
                    for k in range(2):
                        nc.tensor.matmul(out=psx[:, :cw],
                                         lhsT=wih_s[:, k * 512 + g * 128: k * 512 + (g + 1) * 128],
                                         rhs=XT[:, k * NC1 + c0: k * NC1 + c0 + cw],
                                         start=(k == 0), stop=(k == 1))
                    nc.vector.tensor_tensor(out=xpT[:, g * NC1 + c0: g * NC1 + c0 + cw],
                                            in0=psx[:, :cw],
                                            in1=cpk_s[:, g:g + 1].to_broadcast([128, cw]),
                                            op=OP.add)
            # recurrent weights fp32
            whh_s16 = pp.tile([128, 4 * CH], F16)
            nc.sync.dma_start(whh_s16[:], whh16[:])
            whh_s = pp.tile([128, 4 * CH], F32)
            nc.vector.tensor_copy(out=whh_s[:], in_=whh_s16[:])
            # scan
            hh = pp.tile([128, (S1 + 1) * LC], F32)
            cst = pp.tile([128, LC], F32)
            nc.vector.memset(hh[:, 0:LC], 0.0)
            nc.vector.memset(cst[:], 0.0)
            for t in range(S1):
                gps = psp.tile([128, 4 * LC], F32, tag="g", space="PSUM")
                for g in range(4):
                    nc.tensor.matmul(out=gps[:, g * LC:(g + 1) * LC],
                                     lhsT=whh_s[:, g * 128:(g + 1) * 128],
                                     rhs=hh[:, t * LC:(t + 1) * LC],
                                     start=(g == 0), stop=(g == 3))
                G = tp.tile([128, 4 * LC], F32, tag="G")
                nc.vector.tensor_tensor(
                    out=_ap(G[:], [[LC, 4], [1, LC]]),
                    in0=_ap(gps[:], [[LC, 4], [1, LC]]),
                    in1=_ap(xpT[:], [[NC1, 4], [LEN1, LC]], extra_off=t),
                    op=OP.add)
                Ssig = tp.tile([128, 3 * LC], F32, tag="S")
                nc.scalar.activation(out=Ssig[:], in_=G[:, 0:3 * LC], func=AF.Sigmoid)
                Tg = tp.tile([128, LC], F32, tag="Tg")
                nc.scalar.activation(out=Tg[:], in_=G[:, 3 * LC:4 * LC], func=AF.Tanh)
                t1 = tp.tile([128, LC], F32, tag="t1")
                nc.vector.tensor_tensor(out=t1[:], in0=Ssig[:, 0:LC], in1=Tg[:], op=OP.mult)
                nc.vector.tensor_tensor(out=cst[:], in0=Ssig[:, LC:2 * LC], in1=cst[:], op=OP.mult)
                nc.vector.tensor_tensor(out=cst[:], in0=cst[:], in1=t1[:], op=OP.add)
                Tc = tp.tile([128, LC], F32, tag="Tc")
                nc.scalar.activation(out=Tc[:], in_=cst[:], func=AF.Tanh)
                nc.vector.tensor_tensor(out=hh[:, (t + 1) * LC:(t + 2) * LC],
                                        in0=Ssig[:, 2 * LC:3 * LC], in1=Tc[:], op=OP.mult)
                if t == W1 - 1:
                    blk = hh[:, (t + 1) * LC:(t + 2) * LC]
                    nc.vector.tensor_tensor(out=blk, in0=blk, in1=mH[:], op=OP.mult)
                    nc.vector.tensor_tensor(out=blk, in0=blk, in1=fH[:], op=OP.add)
                    nc.vector.tensor_tensor(out=cst[:], in0=cst[:], in1=mH[:], op=OP.mult)
                    nc.vector.tensor_tensor(out=cst[:], in0=cst[:], in1=fC[:], op=OP.add)
            hc = pp.tile([128, LEN1 * LC], F16)
            nc.vector.tensor_copy(out=hc[:], in_=hh[:, (W1 + 1) * LC:(S1 + 1) * LC])
            nc.sync.dma_start(hout[:], hc[:])
    nc.compile()
    return nc


# ---------------------------------------------------------------- L2: word
def build_l2():
    nc = _new_nc(8)
    # per-direction weight pack: 16 chunks of [128, 4*WH]:
    #   chunks 0-7  = wihT for word-embedding input dims (WD = 8*128)
    #   chunks 8-11 = wihT for char-feat input dims (512 = 4*128)
    #   chunks 12-15 = whhT (WH = 4*128)
    # each core ships its direction-group-rank's 4 consecutive chunks.
    embT = nc.dram_tensor("embT", [128, 8 * WIN], F16, kind="ExternalInput")
    cfT = nc.dram_tensor("cfT", [128, 4 * WIN], F16, kind="ExternalInput")
    wsh = nc.dram_tensor("wsh", [128, 4 * 4 * WH], F16, kind="ExternalInput")
    cpk = nc.dram_tensor("cpk", [128, 16 + 3 * 4 * LW + 24 + 6], F32, kind="ExternalInput")
    fpart = nc.dram_tensor("fpart", [512, 6], F32, kind="ExternalOutput")

    with tile.TileContext(nc) as tc:
        with tc.tile_pool(name="p", bufs=1) as pp, \
             tc.tile_pool(name="ps", bufs=2, space="PSUM") as psp, \
             tc.tile_pool(name="d", bufs=1, space="DRAM") as dp, \
             tc.tile_pool(name="tmp", bufs=2) as tp:
            # allgather the weight pack within each direction group
            bounce = dp.tile([128, 4 * 4 * WH], F16)
            gath = dp.tile([4 * 128 * 4 * 4 * WH], F16)
            nc.gpsimd.dma_start(bounce[:], wsh[:])
            nc.gpsimd.collective_compute(
                "AllGather", OP.bypass,
                replica_groups=[[0, 1, 2, 3], [4, 5, 6, 7]],
                ins=[bounce[:].opt()], outs=[gath[:].opt()])
            cpk_s = pp.tile([128, 16 + 3 * 4 * LW + 24 + 6], F32)
            nc.sync.dma_start(cpk_s[:], cpk[:])
            mH = cpk_s[:, 16:16 + 128]
            fH = cpk_s[:, 144:144 + 128]
            fC = cpk_s[:, 272:272 + 128]
            xpT = pp.tile([128, 16 * WIN], F32)
            whh_s = pp.tile([128, 4 * 4 * WH], F32)
            with tc.tile_pool(name="wih", bufs=1) as wp:
                embT_s = wp.tile([128, 8 * WIN], F16)
                nc.sync.dma_start(embT_s[:], embT[:])
                cfT_s = wp.tile([128, 4 * WIN], F16)
                nc.sync.dma_start(cfT_s[:], cfT[:])
                wih_s = wp.tile([128, 12 * 4 * WH], F16)
                SH = 4 * 4 * WH  # 8192 cols per rank shard
                for r in range(3):
                    nc.sync.dma_start(
                        wih_s[:, r * SH:(r + 1) * SH],
                        _dap(gath[:], [[SH, 128], [1, SH]], extra_off=r * 128 * SH))
                whh_s16 = wp.tile([128, SH], F16)
                nc.sync.dma_start(
                    whh_s16[:],
                    _dap(gath[:], [[SH, 128], [1, SH]], extra_off=3 * 128 * SH))
                nc.vector.tensor_copy(out=whh_s[:], in_=whh_s16[:])
                for g in range(16):
                    for (c0, cw) in ((0, 288), (288, 288)):
                        psx = psp.tile([128, 288], F32, tag="psx", space="PSUM")
                        for k in range(8):
                            nc.tensor.matmul(out=psx[:, :cw],
                                             lhsT=wih_s[:, k * 2048 + g * 128: k * 2048 + (g + 1) * 128],
                                             rhs=embT_s[:, k * WIN + c0: k * WIN + c0 + cw],
                                             start=(k == 0), stop=False)
                        for k in range(4):
                            nc.tensor.matmul(out=psx[:, :cw],
                                             lhsT=wih_s[:, (8 + k) * 2048 + g * 128: (8 + k) * 2048 + (g + 1) * 128],
                                             rhs=cfT_s[:, k * WIN + c0: k * WIN + c0 + cw],
                                             start=False, stop=(k == 3))
                        nc.vector.tensor_tensor(out=xpT[:, g * WIN + c0: g * WIN + c0 + cw],
                                                in0=psx[:, :cw],
                                                in1=cpk_s[:, g:g + 1].to_broadcast([128, cw]),
                                                op=OP.add)
            # scan
            hh = pp.tile([128, (S2 + 1) * 4 * LW], F32)
            cst = pp.tile([128, 4 * LW], F32)
            nc.vector.memset(hh[:, 0:4 * LW], 0.0)
            nc.vector.memset(cst[:], 0.0)
            for t in range(S2):
                gps = psp.tile([128, 16 * LW], F32, tag="g", space="PSUM")
                for m in range(16):
                    for k in range(4):
                        nc.tensor.matmul(out=gps[:, m * LW:(m + 1) * LW],
                                         lhsT=whh_s[:, k * 2048 + m * 128: k * 2048 + (m + 1) * 128],
                                         rhs=hh[:, t * 4 * LW + k * LW: t * 4 * LW + (k + 1) * LW],
                                         start=(k == 0), stop=(k == 3))
                G = tp.tile([128, 16 * LW], F32, tag="G")
                nc.vector.tensor_tensor(
                    out=_ap(G[:], [[LW, 16], [1, LW]]),
                    in0=_ap(gps[:], [[LW, 16], [1, LW]]),
                    in1=_ap(xpT[:], [[WIN, 16], [LEN2, LW]], extra_off=t),
                    op=OP.add)
                Ssig = tp.tile([128, 12 * LW], F32, tag="S")
                nc.scalar.activation(out=Ssig[:], in_=G[:, 0:12 * LW], func=AF.Sigmoid)
                Tg = tp.tile([128, 4 * LW], F32, tag="Tg")
                nc.scalar.activation(out=Tg[:], in_=G[:, 12 * LW:16 * LW], func=AF.Tanh)
                t1 = tp.tile([128, 4 * LW], F32, tag="t1")
                nc.vector.tensor_tensor(out=t1[:], in0=Ssig[:, 0:4 * LW], in1=Tg[:], op=OP.mult)
                nc.vector.tensor_tensor(out=cst[:], in0=Ssig[:, 4 * LW:8 * LW], in1=cst[:], op=OP.mult)
                nc.vector.tensor_tensor(out=cst[:], in0=cst[:], in1=t1[:], op=OP.add)
                Tc = tp.tile([128, 4 * LW], F32, tag="Tc")
                nc.scalar.activation(out=Tc[:], in_=cst[:], func=AF.Tanh)
                nc.vector.tensor_tensor(out=hh[:, (t + 1) * 4 * LW:(t + 2) * 4 * LW],
                                        in0=Ssig[:, 8 * LW:12 * LW], in1=Tc[:], op=OP.mult)
                if t == W2 - 1:
                    blk = hh[:, (t + 1) * 4 * LW:(t + 2) * 4 * LW]
                    nc.vector.tensor_tensor(out=blk, in0=blk, in1=mH[:], op=OP.mult)
                    nc.vector.tensor_tensor(out=blk, in0=blk, in1=fH[:], op=OP.add)
                    nc.vector.tensor_tensor(out=cst[:], in0=cst[:], in1=mH[:], op=OP.mult)
                    nc.vector.tensor_tensor(out=cst[:], in0=cst[:], in1=fC[:], op=OP.add)
            # repack post-warmup h (t-major) then feats partial
            hT = pp.tile([128, 4 * 512], F32)
            for k in range(4):
                nc.vector.tensor_copy(
                    out=_ap(hT[:], [[16, 32], [1, 16]], extra_off=k * 512),
                    in_=_ap(hh[:], [[1, 32], [4 * LW, 16]],
                            extra_off=(W2 + 1) * 4 * LW + k * LW))
            fp_s = pp.tile([128, 4 * 6], F32)
            for m in range(4):
                psf = psp.tile([128, 6], F32, tag="psf", space="PSUM")
                for k in range(4):
                    nc.tensor.matmul(out=psf[:],
                                     lhsT=hT[:, k * 512 + m * 128: k * 512 + (m + 1) * 128],
                                     rhs=cpk_s[:, 400 + k * 6:400 + (k + 1) * 6],
                                     start=(k == 0), stop=(k == 3))
                nc.vector.tensor_tensor(out=fp_s[:, m * 6:(m + 1) * 6], in0=psf[:],
                                        in1=cpk_s[:, 424:430], op=OP.add)
            nc.sync.dma_start(fpart[:].rearrange("(m p) s -> p m s", p=128),
                              fp_s[:].rearrange("p (m s) -> p m s", m=4))
    nc.compile()
    return nc


# ---------------------------------------------------------------- L3: viterbi
def build_l3():
    nc = _new_nc(1)
    fstack = nc.dram_tensor("fstack", [8 * 512, 6], F32, kind="ExternalInput")
    # c16: transR(36) | iotaM(36) | maskV(6) | fillV(6) | tstop(6)
    c16 = nc.dram_tensor("c16", [16, 90], F32, kind="ExternalInput")
    # c96: iotaI(36) | iotaJ(768) | uinit(6) | bmask(16)
    c96 = nc.dram_tensor("c96", [96, 826], F32, kind="ExternalInput")
    ids_o = nc.dram_tensor("ids_o", [T], I32, kind="ExternalOutput")

    with tile.TileContext(nc) as tc:
        with tc.tile_pool(name="p", bufs=1) as pp, \
             tc.tile_pool(name="ps", bufs=2, space="PSUM") as psp, \
             tc.tile_pool(name="d", bufs=1, space="DRAM") as dp, \
             tc.tile_pool(name="tmp", bufs=2) as tp:
            # sum the 8 partial feats
            Ff = pp.tile([128, 16 * 6], F32)
            Fb = pp.tile([128, 16 * 6], F32)
            for k in range(4):
                nc.sync.dma_start(Ff[32 * k:32 * (k + 1), :],
                                  fstack[:].rearrange("(c p a) s -> c p a s", c=8, p=32)[k])
                nc.sync.dma_start(Fb[32 * k:32 * (k + 1), :],
                                  fstack[:].rearrange("(c p a) s -> c p a s", c=8, p=32)[4 + k])
            F = pp.tile([128, 16 * 6], F32)
            nc.vector.tensor_tensor(out=F[:], in0=Ff[:], in1=Fb[:], op=OP.add)
            featsD = dp.tile([T * 6], F32)
            nc.sync.dma_start(featsD[:].rearrange("(p a) -> p a", p=128), F[:])
            # stage per-sublane feats windows
            fsub = pp.tile([16, SV * 6], F32)
            fD = featsD[:]
            for p in range(16):
                if p == 0:
                    nc.sync.dma_start(fsub[0:1, 0:WV * 6], _dap(fD, [[WV * 6, 1], [1, WV * 6]]))
                    nc.sync.dma_start(fsub[0:1, WV * 6:SV * 6], _dap(fD, [[LV * 6, 1], [1, LV * 6]]))
                else:
                    nc.sync.dma_start(fsub[p:p + 1, :],
                                      _dap(fD, [[SV * 6, 1], [1, SV * 6]], extra_off=(p * LV - WV) * 6))
            c16_s = pp.tile([16, 90], F32)
            nc.sync.dma_start(c16_s[:], c16[:])
            fv = pp.tile([16, 6], F32)
            nc.vector.memset(fv[:], 0.0)
            bpsH = pp.tile([16, LV * 6], F32)
            for t in range(SV):
                if t == WV:
                    nc.vector.tensor_tensor(out=fv[:], in0=fv[:], in1=c16_s[:, 72:78], op=OP.mult)
                    nc.vector.tensor_tensor(out=fv[:], in0=fv[:], in1=c16_s[:, 78:84], op=OP.add)
                tmp = tp.tile([16, 36], F32, tag="tmp")
                nc.vector.tensor_tensor(out=_ap(tmp[:], [[6, 6], [1, 6]]),
                                        in0=_ap(c16_s[:], [[6, 6], [1, 6]]),
                                        in1=_ap(fv[:], [[0, 6], [1, 6]]), op=OP.add)
                mx = tp.tile([16, 6], F32, tag="mx")
                nc.vector.tensor_reduce(out=mx[:], in_=_ap(tmp[:], [[6, 6], [1, 6]]),
                                        axis=AX.X, op=OP.max)
                eq = tp.tile([16, 36], F32, tag="eq")
                nc.vector.tensor_tensor(out=_ap(eq[:], [[6, 6], [1, 6]]),
                                        in0=_ap(tmp[:], [[6, 6], [1, 6]]),
                                        in1=_ap(mx[:], [[1, 6], [0, 6]]), op=OP.is_ge)
                nc.vector.tensor_tensor(out=eq[:], in0=eq[:], in1=c16_s[:, 36:72], op=OP.mult)
                if t >= WV:
                    nc.vector.tensor_reduce(out=bpsH[:, (t - WV) * 6:(t - WV + 1) * 6],
                                            in_=_ap(eq[:], [[6, 6], [1, 6]]), axis=AX.X, op=OP.min)
                nc.vector.tensor_tensor(out=fv[:], in0=mx[:], in1=fsub[:, t * 6:(t + 1) * 6], op=OP.add)
            # last-tag onehot
            av = pp.tile([16, 6], F32)
            nc.vector.tensor_tensor(out=av[:], in0=fv[:], in1=c16_s[:, 84:90], op=OP.add)
            am = pp.tile([16, 1], F32)
            nc.vector.tensor_reduce(out=am[:], in_=av[:], axis=AX.X, op=OP.max)
            ohf = pp.tile([16, 6], F32)
            nc.vector.tensor_tensor(out=ohf[:], in0=av[:], in1=am[:].to_broadcast([16, 6]), op=OP.is_ge)
            # replicate bps to 96 partitions
            bpsD = dp.tile([16 * LV * 6], F32)
            nc.sync.dma_start(bpsD[:].rearrange("(p a) -> p a", p=16), bpsH[:])
            bpsR = pp.tile([96, LV * 6], F32)
            for e in range(6):
                nc.sync.dma_start(bpsR[16 * e:16 * (e + 1), :],
                                  bpsD[:].rearrange("(p a) -> p a", p=16))
            c96_s = pp.tile([96, 826], F32)
            nc.sync.dma_start(c96_s[:], c96[:])
            uH = pp.tile([96, (LV + 1) * 6], F32)
            nc.vector.tensor_copy(out=uH[:, LV * 6:(LV + 1) * 6], in_=c96_s[:, 804:810])
            for tb in range(LV - 1, -1, -1):
                eqB = tp.tile([96, 36], F32, tag="eqB")
                nc.vector.tensor_tensor(out=_ap(eqB[:], [[6, 6], [1, 6]]),
                                        in0=_ap(bpsR[:], [[0, 6], [1, 6]], extra_off=tb * 6),
                                        in1=_ap(c96_s[:], [[6, 6], [1, 6]]), op=OP.is_equal)
                tB = tp.tile([96, 36], F32, tag="tB")
                nc.vector.tensor_tensor(out=_ap(tB[:], [[6, 6], [1, 6]]),
                                        in0=_ap(eqB[:], [[6, 6], [1, 6]]),
                                        in1=_ap(uH[:], [[0, 6], [1, 6]], extra_off=(tb + 1) * 6),
                                        op=OP.mult)
                nc.vector.tensor_reduce(out=uH[:, tb * 6:(tb + 1) * 6],
                                        in_=_ap(tB[:], [[6, 6], [1, 6]]), axis=AX.X, op=OP.max)
            # decode ids for all hypotheses
            idsA = pp.tile([96, LV], F32)
            tJ = pp.tile([96, 768], F32)
            nc.vector.tensor_tensor(out=tJ[:], in0=uH[:, 6:(LV + 1) * 6], in1=c96_s[:, 36:804], op=OP.mult)
            nc.vector.tensor_reduce(out=idsA[:], in_=_ap(tJ[:], [[6, LV], [1, 6]]), axis=AX.X, op=OP.max)
            # chunk maps flattened onto ONE partition: MT2 [1, 16*36] flat (c,j,e)
            uD = dp.tile([96 * 6], F32)
            nc.sync.dma_start(uD[:].rearrange("(p a) -> p a", p=96), uH[:, 0:6])
            MT2 = pp.tile([1, 16 * 36], F32)
            nc.sync.dma_start(MT2[:], _dap(uD[:], [[576, 1], [6, 16], [1, 6], [96, 6]]))
            # move last-tag onehot (row 15 of ohf) to partition 0
            ohfD = dp.tile([16 * 6], F32)
            nc.sync.dma_start(ohfD[:].rearrange("(p a) -> p a", p=16), ohf[:])
            # stitch on partition 0: ohSeq[:, c*6+e] = onehot(ids at end of chunk c)
            ohSeq = pp.tile([1, 16 * 6], F32)
            nc.sync.dma_start(ohSeq[0:1, 15 * 6:16 * 6],
                              _dap(ohfD[:], [[6, 1], [1, 6]], extra_off=15 * 6))
            for c in range(14, -1, -1):
                tS2 = tp.tile([1, 36], F32, tag="tS2")
                nc.vector.tensor_tensor(out=_ap(tS2[:], [[6, 6], [1, 6]]),
                                        in0=_ap(MT2[:], [[6, 6], [1, 6]], extra_off=(c + 1) * 36),
                                        in1=_ap(ohSeq[:], [[0, 6], [1, 6]], extra_off=(c + 1) * 6),
                                        op=OP.mult)
                nc.vector.tensor_reduce(out=ohSeq[0:1, c * 6:(c + 1) * 6],
                                        in_=_ap(tS2[:], [[6, 6], [1, 6]]), axis=AX.X, op=OP.max)
            ohD = dp.tile([16 * 6], F32)
            nc.sync.dma_start(ohD[:].rearrange("(p a) -> p a", p=1), ohSeq[:])
            selC = pp.tile([96, 1], F32)
            for e in range(6):
                nc.sync.dma_start(selC[16 * e:16 * (e + 1), :],
                                  _dap(ohD[:], [[6, 16], [1, 1]], extra_off=e))
            SEL = pp.tile([96, 16], F32)
            nc.vector.tensor_tensor(out=SEL[:], in0=selC[:].to_broadcast([96, 16]), in1=c96_s[:, 810:826], op=OP.mult)
            psi = psp.tile([16, LV], F32, tag="psi", space="PSUM")
            nc.tensor.matmul(out=psi[:], lhsT=SEL[:], rhs=idsA[:], start=True, stop=True)
            idsI = pp.tile([16, LV], I32)
            nc.vector.tensor_copy(out=idsI[:], in_=psi[:])
            nc.sync.dma_start(ids_o[:].rearrange("(p a) -> p a", p=16), idsI[:])
    nc.compile()
    return nc


# ---------------------------------------------------------------- host glue
_cache = {}


def _programs():
    if "l1" not in _cache:
        _cache["l1"] = build_l1()
        _cache["l2"] = build_l2()
        _cache["l3"] = build_l3()
    return _cache["l1"], _cache["l2"], _cache["l3"]


def kernel(**inp):
    inp = {k: np.asarray(v) for k, v in inp.items()}
    nc1, nc2, nc3 = _programs()
    perf = {}

    chars = inp["chars"].astype(np.int32)
    words = inp["words"].astype(np.int32)
    ix = inp["ix_seq"].astype(np.int64)
    cemb = inp["char_embed"].astype(np.float32)
    wemb = inp["word_embed"].astype(np.float32)

    # ---------------- L1 inputs
    l1_dir = []
    for d, suf in ((0, "f"), (1, "b")):
        Wih = _reorder(inp[f"c_Wih_{suf}"], CH)
        Whh = _reorder(inp[f"c_Whh_{suf}"], CH)
        bias = _reorder(inp[f"c_bih_{suf}"] + inp[f"c_bhh_{suf}"], CH)
        l1_dir.append({
            "wih16": _chunkT(Wih.T.astype(np.float32), 2).astype(np.float16),
            "whh16": np.ascontiguousarray(Whh.T).astype(np.float16),
            "bias": np.ascontiguousarray(bias.reshape(4, 128).T.astype(np.float32)),
            "src": chars if d == 0 else chars[::-1],
        })
    in_maps1 = []
    for core in range(8):
        d, kk = core // 4, core % 4
        dd = l1_dir[d]
        pos = (2048 * kk - W1 + np.arange(NC1)).clip(0, C - 1)
        Xc = cemb[dd["src"][pos]]                      # [NC1, 256]
        xt16 = np.ascontiguousarray(
            Xc.T.reshape(2, 128, NC1).transpose(1, 0, 2).reshape(128, 2 * NC1)
        ).astype(np.float16)
        cpk1 = np.zeros((128, 4 + 3 * LC), np.float32)
        cpk1[:, 0:4] = dd["bias"]
        maskH = np.ones((128, LC), np.float32)
        fillH = np.zeros((128, LC), np.float32)
        fillC = np.zeros((128, LC), np.float32)
        if kk == 0:
            maskH[:, 0] = 0.0
            fillH[:, 0] = inp["c_h0"][d]
            fillC[:, 0] = inp["c_c0"][d]
        cpk1[:, 4:4 + LC] = maskH
        cpk1[:, 4 + LC:4 + 2 * LC] = fillH
        cpk1[:, 4 + 2 * LC:4 + 3 * LC] = fillC
        in_maps1.append({
            "xt16": xt16,
            "wih16": dd["wih16"],
            "whh16": dd["whh16"],
            "cpk": cpk1,
        })
    t0 = _time.time()
    r1 = run_bass_kernel_spmd(nc1, in_maps1, core_ids=list(range(8)),
                              trace=False, tmpdir=None)
    perf["l1_wall"] = _time.time() - t0
    # reassemble char hids: hout col = j*LC + l -> h at pos 2048*kk + 64*l + j
    chf = np.zeros((C, CH), np.float32)
    chb = np.zeros((C, CH), np.float32)
    for core in range(8):
        h = r1.results[core]["hout"].astype(np.float32)  # [128, LEN1*LC]
        d, kk = core // 4, core % 4
        hv = h.reshape(CH, LEN1, LC)  # [hid, j, l]
        pos = LEN1 * (LC * kk + np.arange(LC))[None, :] + np.arange(LEN1)[:, None]
        if d == 0:
            chf[pos.reshape(-1)] = hv.reshape(CH, -1).T
        else:
            chb[C - 1 - pos.reshape(-1)] = hv.reshape(CH, -1).T
    starts, ends = ix[:-1], ix[1:] - 1
    char_feats = np.concatenate(
        [chf[starts], chb[starts], chf[ends], chb[ends]], axis=1)  # [T, 512]

    # ---------------- L2 inputs
    l2_dir = []
    for d, suf in ((0, "f"), (1, "b")):
        Wih = _reorder(inp[f"w_Wih_{suf}"], WH)
        Whh = _reorder(inp[f"w_Whh_{suf}"], WH)
        bias = _reorder(inp[f"w_bih_{suf}"] + inp[f"w_bhh_{suf}"], WH)
        # 16-chunk pack: wihTwe (8) | wihTcf (4) | whhT (4), each [128, 2048]
        P = np.concatenate([
            _chunkT(np.ascontiguousarray(Wih[:, 512:].T), 8),
            _chunkT(np.ascontiguousarray(Wih[:, :512].T), 4),
            _chunkT(np.ascontiguousarray(Whh.T), 4),
        ], axis=1).astype(np.float16)            # [128, 16*2048]
        l2_dir.append({
            "P": P,
            "bias": np.ascontiguousarray(bias.reshape(16, 128).T.astype(np.float32)),
            "cf": char_feats if d == 0 else char_feats[::-1],
            "wsrc": words if d == 0 else words[::-1],
            "h2t": inp["hid2tag_W"][:, :WH] if d == 0 else inp["hid2tag_W"][:, WH:],
        })
    in_maps2 = []
    for core in range(8):
        d, kk = core // 4, core % 4
        dd = l2_dir[d]
        rows = (512 * kk - W2 + np.arange(WIN)).clip(0, T - 1)
        We = wemb[dd["wsrc"][rows]]                    # [WIN, 1024]
        embT16 = np.ascontiguousarray(
            We.T.reshape(8, 128, WIN).transpose(1, 0, 2).reshape(128, 8 * WIN)
        ).astype(np.float16)
        cfr = dd["cf"][rows]                           # [WIN, 512]
        cfT16 = np.ascontiguousarray(
            cfr.T.reshape(4, 128, WIN).transpose(1, 0, 2).reshape(128, 4 * WIN)
        ).astype(np.float16)
        cpk2 = np.zeros((128, 16 + 3 * 4 * LW + 24 + 6), np.float32)
        cpk2[:, 0:16] = dd["bias"]
        maskH = np.ones((128, 4 * LW), np.float32)
        fillH = np.zeros((128, 4 * LW), np.float32)
        fillC = np.zeros((128, 4 * LW), np.float32)
        if kk == 0:
            for k in range(4):
                maskH[:, k * LW] = 0.0
                fillH[:, k * LW] = inp["w_h0"][d][k * 128:(k + 1) * 128]
                fillC[:, k * LW] = inp["w_c0"][d][k * 128:(k + 1) * 128]
        cpk2[:, 16:144] = maskH
        cpk2[:, 144:272] = fillH
        cpk2[:, 272:400] = fillC
        cpk2[:, 400:424] = _chunkT(np.ascontiguousarray(dd["h2t"].T), 4)
        if d == 0:
            cpk2[:, 424:430] = inp["hid2tag_b"][None, :]
        in_maps2.append({
            "embT": embT16,
            "cfT": cfT16,
            "wsh": np.ascontiguousarray(dd["P"][:, kk * 8192:(kk + 1) * 8192]),
            "cpk": cpk2,
        })
    t0 = _time.time()
    r2 = run_bass_kernel_spmd(nc2, in_maps2, core_ids=list(range(8)),
                              trace=False, tmpdir=None)
    perf["l2_wall"] = _time.time() - t0
    fstack = np.zeros((8 * 512, 6), np.float32)
    for core in range(8):
        fp = r2.results[core]["fpart"]  # [512, 6] for global t block 512*kk
        d, kk = core // 4, core % 4
        if d == 0:
            fstack[512 * core:512 * (core + 1)] = fp
        else:
            fstack[512 * core:512 * (core + 1)] = fp[::-1]
    # bwd partials: core (4+kk) block covers reversed rows [512kk:512kk+512]
    # -> global t = T-1 - rev_t, i.e. global block [T-512(kk+1), T-512kk) reversed.
    bsec = fstack[4 * 512:].copy()
    fstack[4 * 512:] = 0
    for kk in range(4):
        blk = bsec[512 * kk:512 * (kk + 1)]
        g0 = T - 512 * (kk + 1)
        fstack[4 * 512 + g0:4 * 512 + g0 + 512] = blk

    # ---------------- L3 inputs
    trans = inp["transition"].astype(np.float32)
    c16 = np.zeros((16, 90), np.float32)
    c16[:, 0:36] = trans.reshape(1, 36)
    c16[:, 36:72] = (np.arange(36) % 6 - 6).astype(np.float32)[None, :]
    maskV = np.ones((16, 6), np.float32)
    maskV[0] = 0.0
    fillV = np.zeros((16, 6), np.float32)
    fv0 = np.full(6, NEG, np.float32)
    fv0[4] = 0.0
    fillV[0] = fv0
    c16[:, 72:78] = maskV
    c16[:, 78:84] = fillV
    c16[:, 84:90] = trans[:, 5][None, :]
    c96 = np.zeros((96, 826), np.float32)
    c96[:, 0:36] = (np.arange(36) // 6 - 6).astype(np.float32)[None, :]
    c96[:, 36:804] = (np.arange(768) % 6).astype(np.float32)[None, :]
    uinit = np.zeros((96, 6), np.float32)
    for e in range(6):
        uinit[16 * e:16 * (e + 1), e] = 1.0
    c96[:, 804:810] = uinit
    bmask = np.zeros((96, 16), np.float32)
    for e in range(6):
        for c in range(16):
            bmask[16 * e + c, c] = 1.0
    c96[:, 810:826] = bmask
    in_map3 = {"fstack": fstack, "c16": c16, "c96": c96}
    t0 = _time.time()
    r3 = run_bass_kernel_spmd(nc3, [in_map3], core_ids=[0],
                              trace=False, tmpdir=None)
    perf["l3_wall"] = _time.time() - t0
    kernel.last_perf = perf
    kernel.last_fstack = fstack
    return r3.results[0]["ids_o"].astype(np.int32)


kernel.last_perf = {}


# revision 8
# speedup vs baseline: 43.1205x; 1.3421x over previous
"""Trainium2 Bass kernel for nn_ConcatCharLSTM_LSTM_CRF.

Strategy (8 NeuronCores, SPMD, three device launches; host does the
inter-launch data movement). The dominant cost through this runtime path is
host->device transfer bandwidth (~35 MB/s effective), so the design
minimizes shipped bytes:
  - embeddings are gathered on host (pure data movement) and shipped as the
    per-core windows actually consumed, in fp16;
  - LSTM weights ship in fp16; the word-LSTM weight pack is sharded 4-ways
    per direction and reconstructed on-device with an AllGather collective;
  - matmuls against shipped fp16 data run in fp16 (fp32 PSUM accumulate);
    the recurrent path stays fp32.
Layers:
  L1: char BiLSTM. Per core: 32 chunk-lanes x 64 steps + 64 warmup steps
      (LSTM forget-gate contraction decays chunk-boundary state error).
      4 cores fwd + 4 bwd. Compact window: 2112 = 64 + 32*64 input columns,
      scan indexes lane l step t at column 64*l + t.
  L2: word BiLSTM, same scheme (512 cols + 64 warmup per core) + hid2tag
      partial feats.
  L3: Viterbi forward scan (16 time-chunks as partition sublanes with
      warmup) + exact chunked backtrace via one-hot map composition, 1 core.
"""

import os
import sys
import numpy as np
import time as _time

sys.path.insert(0, "/opt/trn_rl_repo")
os.environ.setdefault("JAX_PLATFORMS", "axon,cpu")
os.environ.setdefault("JAX_COMPILATION_CACHE_DIR", "/root/.cache/jax_bass")

from concourse import bass, mybir
from concourse import bacc
import concourse.tile as tile
from concourse.bass_utils import run_bass_kernel_spmd
from concourse.masks import make_identity

try:
    import jax
    jax.config.update("jax_compilation_cache_dir", "/root/.cache/jax_bass")
    jax.config.update("jax_persistent_cache_min_entry_size_bytes", -1)
    jax.config.update("jax_persistent_cache_min_compile_time_secs", 0.0)
except Exception:
    pass

F32 = mybir.dt.float32
F16 = mybir.dt.float16
I32 = mybir.dt.int32
AF = mybir.ActivationFunctionType
OP = mybir.AluOpType
AX = mybir.AxisListType

# problem constants
T, C, V, WD, CS, CD = 2048, 8192, 50000, 1024, 8000, 256
CH, WH = 128, 512            # per-direction hidden sizes
NEG = -10000.0

# chunking parameters
LC, LEN1, W1 = 32, 64, 64    # char: lanes/core, chunk len, warmup
S1 = LEN1 + W1               # char steps per core = 128
NC1 = LEN1 * LC + W1         # compact char cols per core = 2112
LW, LEN2, W2 = 32, 16, 64    # word
S2 = LEN2 + W2               # 80
WIN = LEN2 * LW + W2         # word per-core column window = 576
NV, LV, WV = 16, 128, 32     # viterbi chunks, chunk len, warmup
SV = LV + WV                 # 160

# gate reorder: torch (i,f,g,o) -> (i,f,o,g) so sigmoid cols are contiguous
PERM = (0, 1, 3, 2)


def _reorder(w, H):
    """reorder gate blocks of leading dim 4H from (i,f,g,o) to (i,f,o,g)."""
    blocks = [w[i * H:(i + 1) * H] for i in range(4)]
    return np.concatenate([blocks[p] for p in PERM], axis=0)


def _chunkT(a, nk):
    """[R, G] -> [128, nk*G] where col k*G+g holds a[k*128+p, g] (R = nk*128)."""
    R, G = a.shape
    assert R == nk * 128
    return np.ascontiguousarray(
        a.reshape(nk, 128, G).transpose(1, 0, 2).reshape(128, nk * G))


def _ap(ap, dims, extra_off=0):
    """Build an AP with custom free dims [[step,count],...] keeping partition dim."""
    return bass.AP(ap.tensor, ap.offset + extra_off, [list(ap.ap[0])] + [list(d) for d in dims])


def _dap(ap, dims, extra_off=0):
    """Build an AP replacing ALL dims (for DRAM tensors)."""
    return bass.AP(ap.tensor, ap.offset + extra_off, [list(d) for d in dims])


def _new_nc(num_devices):
    return bacc.Bacc("TRN2", target_bir_lowering=False, debug=False,
                     num_devices=num_devices)


# ---------------------------------------------------------------- L1: char
def build_l1():
    nc = _new_nc(8)
    xt16 = nc.dram_tensor("xt16", [128, 2 * NC1], F16, kind="ExternalInput")
    wih16 = nc.dram_tensor("wih16", [128, 2 * 4 * CH], F16, kind="ExternalInput")
    whh16 = nc.dram_tensor("whh16", [128, 4 * CH], F16, kind="ExternalInput")
    cpk = nc.dram_tensor("cpk", [128, 4 + 3 * LC], F32, kind="ExternalInput")
    hout = nc.dram_tensor("hout", [128, LEN1 * LC], F16, kind="ExternalOutput")

    with tile.TileContext(nc) as tc:
        with tc.tile_pool(name="p", bufs=1) as pp, \
             tc.tile_pool(name="ps", bufs=2, space="PSUM") as psp, \
             tc.tile_pool(name="tmp", bufs=2) as tp:
            XT = pp.tile([128, 2 * NC1], F16)
            nc.sync.dma_start(XT[:], xt16[:])
            wih_s = pp.tile([128, 2 * 4 * CH], F16)
            nc.sync.dma_start(wih_s[:], wih16[:])
            cpk_s = pp.tile([128, 4 + 3 * LC], F32)
            nc.sync.dma_start(cpk_s[:], cpk[:])
            mH = cpk_s[:, 4:4 + LC]
            fH = cpk_s[:, 4 + LC:4 + 2 * LC]
            fC = cpk_s[:, 4 + 2 * LC:4 + 3 * LC]
            # bulk xproj: xpT [128, 4*NC1] (gate-chunk major)
            xpT = pp.tile([128, 4 * NC1], F32)
            blocks = [(0, 512), (512, 512), (1024, 512), (1536, 512), (2048, 64)]
            for g in range(4):
                for (c0, cw) in blocks:
                    psx = psp.tile([128, 512], F32, tag="psx", space="PSUM")
                    for k in range(2):
                        nc.tensor.matmul(out=psx[:, :cw],
                                         lhsT=wih_s[:, k * 512 + g * 128: k * 512 + (g + 1) * 128],
                                         rhs=XT[:, k * NC1 + c0: k * NC1 + c0 + cw],
                                         start=(k == 0), stop=(k == 1))
                    nc.vector.tensor_tensor(out=xpT[:, g * NC1 + c0: g * NC1 + c0 + cw],
                                            in0=psx[:, :cw],
                                            in1=cpk_s[:, g:g + 1].to_broadcast([128, cw]),
                                            op=OP.add)
            # recurrent weights fp32
            whh_s16 = pp.tile([128, 4 * CH], F16)
            nc.sync.dma_start(whh_s16[:], whh16[:])
            whh_s = pp.tile([128, 4 * CH], F32)
            nc.vector.tensor_copy(out=whh_s[:], in_=whh_s16[:])
            # scan
            hh = pp.tile([128, (S1 + 1) * LC], F32)
            cst = pp.tile([128, LC], F32)
            nc.vector.memset(hh[:, 0:LC], 0.0)
            nc.vector.memset(cst[:], 0.0)
            for t in range(S1):
                gps = psp.tile([128, 4 * LC], F32, tag="g", space="PSUM")
                for g in range(4):
                    nc.tensor.matmul(out=gps[:, g * LC:(g + 1) * LC],
                                     lhsT=whh_s[:, g * 128:(g + 1) * 128],
                                     rhs=hh[:, t * LC:(t + 1) * LC],
                                     start=(g == 0), stop=(g == 3))
                G = tp.tile([128, 4 * LC], F32, tag="G")
                nc.vector.tensor_tensor(
                    out=_ap(G[:], [[LC, 4], [1, LC]]),
                    in0=_ap(gps[:], [[LC, 4], [1, LC]]),
                    in1=_ap(xpT[:], [[NC1, 4], [LEN1, LC]], extra_off=t),
                    op=OP.add)
                Ssig = tp.tile([128, 3 * LC], F32, tag="S")
                nc.scalar.activation(out=Ssig[:], in_=G[:, 0:3 * LC], func=AF.Sigmoid)
                Tg = tp.tile([128, LC], F32, tag="Tg")
                nc.scalar.activation(out=Tg[:], in_=G[:, 3 * LC:4 * LC], func=AF.Tanh)
                t1 = tp.tile([128, LC], F32, tag="t1")
                nc.vector.tensor_tensor(out=t1[:], in0=Ssig[:, 0:LC], in1=Tg[:], op=OP.mult)
                nc.vector.tensor_tensor(out=cst[:], in0=Ssig[:, LC:2 * LC], in1=cst[:], op=OP.mult)
                nc.vector.tensor_tensor(out=cst[:], in0=cst[:], in1=t1[:], op=OP.add)
                Tc = tp.tile([128, LC], F32, tag="Tc")
                nc.scalar.activation(out=Tc[:], in_=cst[:], func=AF.Tanh)
                nc.vector.tensor_tensor(out=hh[:, (t + 1) * LC:(t + 2) * LC],
                                        in0=Ssig[:, 2 * LC:3 * LC], in1=Tc[:], op=OP.mult)
                if t == W1 - 1:
                    blk = hh[:, (t + 1) * LC:(t + 2) * LC]
                    nc.vector.tensor_tensor(out=blk, in0=blk, in1=mH[:], op=OP.mult)
                    nc.vector.tensor_tensor(out=blk, in0=blk, in1=fH[:], op=OP.add)
                    nc.vector.tensor_tensor(out=cst[:], in0=cst[:], in1=mH[:], op=OP.mult)
                    nc.vector.tensor_tensor(out=cst[:], in0=cst[:], in1=fC[:], op=OP.add)
            hc = pp.tile([128, LEN1 * LC], F16)
            nc.vector.tensor_copy(out=hc[:], in_=hh[:, (W1 + 1) * LC:(S1 + 1) * LC])
            nc.sync.dma_start(hout[:], hc[:])
    nc.compile()
    return nc


# ---------------------------------------------------------------- L2: word
def build_l2():
    nc = _new_nc(8)
    # per-direction weight pack: 16 chunks of [128, 4*WH]:
    #   chunks 0-7  = wihT for word-embedding input dims (WD = 8*128)
    #   chunks 8-11 = wihT for char-feat input dims (512 = 4*128)
    #   chunks 12-15 = whhT (WH = 4*128)
    # each core ships its direction-group-rank's 4 consecutive chunks.
    embT = nc.dram_tensor("embT", [128, 8 * WIN], F16, kind="ExternalInput")
    cfT = nc.dram_tensor("cfT", [128, 4 * WIN], F16, kind="ExternalInput")
    wsh = nc.dram_tensor("wsh", [128, 4 * 4 * WH], F16, kind="ExternalInput")
    cpk = nc.dram_tensor("cpk", [128, 16 + 3 * 4 * LW + 24 + 6], F32, kind="ExternalInput")
    fpart = nc.dram_tensor("fpart", [512, 6], F32, kind="ExternalOutput")

    with tile.TileContext(nc) as tc:
        with tc.tile_pool(name="p", bufs=1) as pp, \
             tc.tile_pool(name="ps", bufs=2, space="PSUM") as psp, \
             tc.tile_pool(name="d", bufs=1, space="DRAM") as dp, \
             tc.tile_pool(name="tmp", bufs=2) as tp:
            # allgather the weight pack within each direction group
            bounce = dp.tile([128, 4 * 4 * WH], F16)
            gath = dp.tile([4 * 128 * 4 * 4 * WH], F16)
            nc.gpsimd.dma_start(bounce[:], wsh[:])
            nc.gpsimd.collective_compute(
                "AllGather", OP.bypass,
                replica_groups=[[0, 1, 2, 3], [4, 5, 6, 7]],
                ins=[bounce[:].opt()], outs=[gath[:].opt()])
            cpk_s = pp.tile([128, 16 + 3 * 4 * LW + 24 + 6], F32)
            nc.sync.dma_start(cpk_s[:], cpk[:])
            mH = cpk_s[:, 16:16 + 128]
            fH = cpk_s[:, 144:144 + 128]
            fC = cpk_s[:, 272:272 + 128]
            xpT = pp.tile([128, 16 * WIN], F32)
            whh_s = pp.tile([128, 4 * 4 * WH], F32)
            with tc.tile_pool(name="wih", bufs=1) as wp:
                embT_s = wp.tile([128, 8 * WIN], F16)
                nc.sync.dma_start(embT_s[:], embT[:])
                cfT_s = wp.tile([128, 4 * WIN], F16)
                nc.sync.dma_start(cfT_s[:], cfT[:])
                wih_s = wp.tile([128, 12 * 4 * WH], F16)
                SH = 4 * 4 * WH  # 8192 cols per rank shard
                for r in range(3):
                    nc.sync.dma_start(
                        wih_s[:, r * SH:(r + 1) * SH],
                        _dap(gath[:], [[SH, 128], [1, SH]], extra_off=r * 128 * SH))
                whh_s16 = wp.tile([128, SH], F16)
                nc.sync.dma_start(
                    whh_s16[:],
                    _dap(gath[:], [[SH, 128], [1, SH]], extra_off=3 * 128 * SH))
                nc.vector.tensor_copy(out=whh_s[:], in_=whh_s16[:])
                for g in range(16):
                    for (c0, cw) in ((0, 288), (288, 288)):
                        psx = psp.tile([128, 288], F32, tag="psx", space="PSUM")
                        for k in range(8):
                            nc.tensor.matmul(out=psx[:, :cw],
                                             lhsT=wih_s[:, k * 2048 + g * 128: k * 2048 + (g + 1) * 128],
                                             rhs=embT_s[:, k * WIN + c0: k * WIN + c0 + cw],
                                             start=(k == 0), stop=False)
                        for k in range(4):
                            nc.tensor.matmul(out=psx[:, :cw],
                                             lhsT=wih_s[:, (8 + k) * 2048 + g * 128: (8 + k) * 2048 + (g + 1) * 128],
                                             rhs=cfT_s[:, k * WIN + c0: k * WIN + c0 + cw],
                                             start=False, stop=(k == 3))
                        nc.vector.tensor_tensor(out=xpT[:, g * WIN + c0: g * WIN + c0 + cw],
                                                in0=psx[:, :cw],
                                                in1=cpk_s[:, g:g + 1].to_broadcast([128, cw]),
                                                op=OP.add)
            # scan
            hh = pp.tile([128, (S2 + 1) * 4 * LW], F32)
            cst = pp.tile([128, 4 * LW], F32)
            nc.vector.memset(hh[:, 0:4 * LW], 0.0)
            nc.vector.memset(cst[:], 0.0)
            for t in range(S2):
                gps = psp.tile([128, 16 * LW], F32, tag="g", space="PSUM")
                for m in range(16):
                    for k in range(4):
                        nc.tensor.matmul(out=gps[:, m * LW:(m + 1) * LW],
                                         lhsT=whh_s[:, k * 2048 + m * 128: k * 2048 + (m + 1) * 128],
                                         rhs=hh[:, t * 4 * LW + k * LW: t * 4 * LW + (k + 1) * LW],
                                         start=(k == 0), stop=(k == 3))
                G = tp.tile([128, 16 * LW], F32, tag="G")
                nc.vector.tensor_tensor(
                    out=_ap(G[:], [[LW, 16], [1, LW]]),
                    in0=_ap(gps[:], [[LW, 16], [1, LW]]),
                    in1=_ap(xpT[:], [[WIN, 16], [LEN2, LW]], extra_off=t),
                    op=OP.add)
                Ssig = tp.tile([128, 12 * LW], F32, tag="S")
                nc.scalar.activation(out=Ssig[:], in_=G[:, 0:12 * LW], func=AF.Sigmoid)
                Tg = tp.tile([128, 4 * LW], F32, tag="Tg")
                nc.scalar.activation(out=Tg[:], in_=G[:, 12 * LW:16 * LW], func=AF.Tanh)
                t1 = tp.tile([128, 4 * LW], F32, tag="t1")
                nc.vector.tensor_tensor(out=t1[:], in0=Ssig[:, 0:4 * LW], in1=Tg[:], op=OP.mult)
                nc.vector.tensor_tensor(out=cst[:], in0=Ssig[:, 4 * LW:8 * LW], in1=cst[:], op=OP.mult)
                nc.vector.tensor_tensor(out=cst[:], in0=cst[:], in1=t1[:], op=OP.add)
                Tc = tp.tile([128, 4 * LW], F32, tag="Tc")
                nc.scalar.activation(out=Tc[:], in_=cst[:], func=AF.Tanh)
                nc.vector.tensor_tensor(out=hh[:, (t + 1) * 4 * LW:(t + 2) * 4 * LW],
                                        in0=Ssig[:, 8 * LW:12 * LW], in1=Tc[:], op=OP.mult)
                if t == W2 - 1:
                    blk = hh[:, (t + 1) * 4 * LW:(t + 2) * 4 * LW]
                    nc.vector.tensor_tensor(out=blk, in0=blk, in1=mH[:], op=OP.mult)
                    nc.vector.tensor_tensor(out=blk, in0=blk, in1=fH[:], op=OP.add)
                    nc.vector.tensor_tensor(out=cst[:], in0=cst[:], in1=mH[:], op=OP.mult)
                    nc.vector.tensor_tensor(out=cst[:], in0=cst[:], in1=fC[:], op=OP.add)
            # repack post-warmup h (t-major) then feats partial
            hT = pp.tile([128, 4 * 512], F32)
            for k in range(4):
                nc.vector.tensor_copy(
                    out=_ap(hT[:], [[16, 32], [1, 16]], extra_off=k * 512),
                    in_=_ap(hh[:], [[1, 32], [4 * LW, 16]],
                            extra_off=(W2 + 1) * 4 * LW + k * LW))
            fp_s = pp.tile([128, 4 * 6], F32)
            for m in range(4):
                psf = psp.tile([128, 6], F32, tag="psf", space="PSUM")
                for k in range(4):
                    nc.tensor.matmul(out=psf[:],
                                     lhsT=hT[:, k * 512 + m * 128: k * 512 + (m + 1) * 128],
                                     rhs=cpk_s[:, 400 + k * 6:400 + (k + 1) * 6],
                                     start=(k == 0), stop=(k == 3))
                nc.vector.tensor_tensor(out=fp_s[:, m * 6:(m + 1) * 6], in0=psf[:],
                                        in1=cpk_s[:, 424:430], op=OP.add)
            nc.sync.dma_start(fpart[:].rearrange("(m p) s -> p m s", p=128),
                              fp_s[:].rearrange("p (m s) -> p m s", m=4))
    nc.compile()
    return nc


# ---------------------------------------------------------------- L3: viterbi
def build_l3():
    nc = _new_nc(1)
    fstack = nc.dram_tensor("fstack", [8 * 512, 6], F32, kind="ExternalInput")
    # c16: transR(36) | iotaM(36) | maskV(6) | fillV(6) | tstop(6)
    c16 = nc.dram_tensor("c16", [16, 90], F32, kind="ExternalInput")
    # c96: iotaI(36) | iotaJ(768) | uinit(6) | bmask(16)
    c96 = nc.dram_tensor("c96", [96, 826], F32, kind="ExternalInput")
    ids_o = nc.dram_tensor("ids_o", [T], I32, kind="ExternalOutput")

    with tile.TileContext(nc) as tc:
        with tc.tile_pool(name="p", bufs=1) as pp, \
             tc.tile_pool(name="ps", bufs=2, space="PSUM") as psp, \
             tc.tile_pool(name="d", bufs=1, space="DRAM") as dp, \
             tc.tile_pool(name="tmp", bufs=2) as tp:
            # sum the 8 partial feats
            Ff = pp.tile([128, 16 * 6], F32)
            Fb = pp.tile([128, 16 * 6], F32)
            for k in range(4):
                nc.sync.dma_start(Ff[32 * k:32 * (k + 1), :],
                                  fstack[:].rearrange("(c p a) s -> c p a s", c=8, p=32)[k])
                nc.sync.dma_start(Fb[32 * k:32 * (k + 1), :],
                                  fstack[:].rearrange("(c p a) s -> c p a s", c=8, p=32)[4 + k])
            F = pp.tile([128, 16 * 6], F32)
            nc.vector.tensor_tensor(out=F[:], in0=Ff[:], in1=Fb[:], op=OP.add)
            featsD = dp.tile([T * 6], F32)
            nc.sync.dma_start(featsD[:].rearrange("(p a) -> p a", p=128), F[:])
            # stage per-sublane feats windows
            fsub = pp.tile([16, SV * 6], F32)
            fD = featsD[:]
            for p in range(16):
                if p == 0:
                    nc.sync.dma_start(fsub[0:1, 0:WV * 6], _dap(fD, [[WV * 6, 1], [1, WV * 6]]))
                    nc.sync.dma_start(fsub[0:1, WV * 6:SV * 6], _dap(fD, [[LV * 6, 1], [1, LV * 6]]))
                else:
                    nc.sync.dma_start(fsub[p:p + 1, :],
                                      _dap(fD, [[SV * 6, 1], [1, SV * 6]], extra_off=(p * LV - WV) * 6))
            c16_s = pp.tile([16, 90], F32)
            nc.sync.dma_start(c16_s[:], c16[:])
            fv = pp.tile([16, 6], F32)
            nc.vector.memset(fv[:], 0.0)
            bpsH = pp.tile([16, LV * 6], F32)
            for t in range(SV):
                if t == WV:
                    nc.vector.tensor_tensor(out=fv[:], in0=fv[:], in1=c16_s[:, 72:78], op=OP.mult)
                    nc.vector.tensor_tensor(out=fv[:], in0=fv[:], in1=c16_s[:, 78:84], op=OP.add)
                tmp = tp.tile([16, 36], F32, tag="tmp")
                nc.vector.tensor_tensor(out=_ap(tmp[:], [[6, 6], [1, 6]]),
                                        in0=_ap(c16_s[:], [[6, 6], [1, 6]]),
                                        in1=_ap(fv[:], [[0, 6], [1, 6]]), op=OP.add)
                mx = tp.tile([16, 6], F32, tag="mx")
                nc.vector.tensor_reduce(out=mx[:], in_=_ap(tmp[:], [[6, 6], [1, 6]]),
                                        axis=AX.X, op=OP.max)
                eq = tp.tile([16, 36], F32, tag="eq")
                nc.vector.tensor_tensor(out=_ap(eq[:], [[6, 6], [1, 6]]),
                                        in0=_ap(tmp[:], [[6, 6], [1, 6]]),
                                        in1=_ap(mx[:], [[1, 6], [0, 6]]), op=OP.is_ge)
                nc.vector.tensor_tensor(out=eq[:], in0=eq[:], in1=c16_s[:, 36:72], op=OP.mult)
                if t >= WV:
                    nc.vector.tensor_reduce(out=bpsH[:, (t - WV) * 6:(t - WV + 1) * 6],
                                            in_=_ap(eq[:], [[6, 6], [1, 6]]), axis=AX.X, op=OP.min)
                nc.vector.tensor_tensor(out=fv[:], in0=mx[:], in1=fsub[:, t * 6:(t + 1) * 6], op=OP.add)
            # last-tag onehot
            av = pp.tile([16, 6], F32)
            nc.vector.tensor_tensor(out=av[:], in0=fv[:], in1=c16_s[:, 84:90], op=OP.add)
            am = pp.tile([16, 1], F32)
            nc.vector.tensor_reduce(out=am[:], in_=av[:], axis=AX.X, op=OP.max)
            ohf = pp.tile([16, 6], F32)
            nc.vector.tensor_tensor(out=ohf[:], in0=av[:], in1=am[:].to_broadcast([16, 6]), op=OP.is_ge)
            # replicate bps to 96 partitions
            bpsD = dp.tile([16 * LV * 6], F32)
            nc.sync.dma_start(bpsD[:].rearrange("(p a) -> p a", p=16), bpsH[:])
            bpsR = pp.tile([96, LV * 6], F32)
            for e in range(6):
                nc.sync.dma_start(bpsR[16 * e:16 * (e + 1), :],
                                  bpsD[:].rearrange("(p a) -> p a", p=16))
            c96_s = pp.tile([96, 826], F32)
            nc.sync.dma_start(c96_s[:], c96[:])
            uH = pp.tile([96, (LV + 1) * 6], F32)
            nc.vector.tensor_copy(out=uH[:, LV * 6:(LV + 1) * 6], in_=c96_s[:, 804:810])
            for tb in range(LV - 1, -1, -1):
                eqB = tp.tile([96, 36], F32, tag="eqB")
                nc.vector.tensor_tensor(out=_ap(eqB[:], [[6, 6], [1, 6]]),
                                        in0=_ap(bpsR[:], [[0, 6], [1, 6]], extra_off=tb * 6),
                                        in1=_ap(c96_s[:], [[6, 6], [1, 6]]), op=OP.is_equal)
                tB = tp.tile([96, 36], F32, tag="tB")
                nc.vector.tensor_tensor(out=_ap(tB[:], [[6, 6], [1, 6]]),
                                        in0=_ap(eqB[:], [[6, 6], [1, 6]]),
                                        in1=_ap(uH[:], [[0, 6], [1, 6]], extra_off=(tb + 1) * 6),
                                        op=OP.mult)
                nc.vector.tensor_reduce(out=uH[:, tb * 6:(tb + 1) * 6],
                                        in_=_ap(tB[:], [[6, 6], [1, 6]]), axis=AX.X, op=OP.max)
            # decode ids for all hypotheses
            idsA = pp.tile([96, LV], F32)
            tJ = pp.tile([96, 768], F32)
            nc.vector.tensor_tensor(out=tJ[:], in0=uH[:, 6:(LV + 1) * 6], in1=c96_s[:, 36:804], op=OP.mult)
            nc.vector.tensor_reduce(out=idsA[:], in_=_ap(tJ[:], [[6, LV], [1, 6]]), axis=AX.X, op=OP.max)
            # chunk maps flattened onto ONE partition: MT2 [1, 16*36] flat (c,j,e)
            uD = dp.tile([96 * 6], F32)
            nc.sync.dma_start(uD[:].rearrange("(p a) -> p a", p=96), uH[:, 0:6])
            MT2 = pp.tile([1, 16 * 36], F32)
            nc.sync.dma_start(MT2[:], _dap(uD[:], [[576, 1], [6, 16], [1, 6], [96, 6]]))
            # move last-tag onehot (row 15 of ohf) to partition 0
            ohfD = dp.tile([16 * 6], F32)
            nc.sync.dma_start(ohfD[:].rearrange("(p a) -> p a", p=16), ohf[:])
            # stitch on partition 0: ohSeq[:, c*6+e] = onehot(ids at end of chunk c)
            ohSeq = pp.tile([1, 16 * 6], F32)
            nc.sync.dma_start(ohSeq[0:1, 15 * 6:16 * 6],
                              _dap(ohfD[:], [[6, 1], [1, 6]], extra_off=15 * 6))
            for c in range(14, -1, -1):
                tS2 = tp.tile([1, 36], F32, tag="tS2")
                nc.vector.tensor_tensor(out=_ap(tS2[:], [[6, 6], [1, 6]]),
                                        in0=_ap(MT2[:], [[6, 6], [1, 6]], extra_off=(c + 1) * 36),
                                        in1=_ap(ohSeq[:], [[0, 6], [1, 6]], extra_off=(c + 1) * 6),
                                        op=OP.mult)
                nc.vector.tensor_reduce(out=ohSeq[0:1, c * 6:(c + 1) * 6],
                                        in_=_ap(tS2[:], [[6, 6], [1, 6]]), axis=AX.X, op=OP.max)
            ohD = dp.tile([16 * 6], F32)
            nc.sync.dma_start(ohD[:].rearrange("(p a) -> p a", p=1), ohSeq[:])
            selC = pp.tile([96, 1], F32)
            for e in range(6):
                nc.sync.dma_start(selC[16 * e:16 * (e + 1), :],
                                  _dap(ohD[:], [[6, 16], [1, 1]], extra_off=e))
            SEL = pp.tile([96, 16], F32)
            nc.vector.tensor_tensor(out=SEL[:], in0=selC[:].to_broadcast([96, 16]), in1=c96_s[:, 810:826], op=OP.mult)
            psi = psp.tile([16, LV], F32, tag="psi", space="PSUM")
            nc.tensor.matmul(out=psi[:], lhsT=SEL[:], rhs=idsA[:], start=True, stop=True)
            idsI = pp.tile([16, LV], I32)
            nc.vector.tensor_copy(out=idsI[:], in_=psi[:])
            nc.sync.dma_start(ids_o[:].rearrange("(p a) -> p a", p=16), idsI[:])
    nc.compile()
    return nc


# ---------------------------------------------------------------- host glue
_cache = {}


def _programs():
    if "l1" not in _cache:
        _cache["l1"] = build_l1()
        _cache["l2"] = build_l2()
        _cache["l3"] = build_l3()
    return _cache["l1"], _cache["l2"], _cache["l3"]


def _kernel_3launch(**inp):
    inp = {k: np.asarray(v) for k, v in inp.items()}
    nc1, nc2, nc3 = _programs()
    perf = {}

    chars = inp["chars"].astype(np.int32)
    words = inp["words"].astype(np.int32)
    ix = inp["ix_seq"].astype(np.int64)
    cemb = inp["char_embed"].astype(np.float32)
    wemb = inp["word_embed"].astype(np.float32)

    # ---------------- L1 inputs
    l1_dir = []
    for d, suf in ((0, "f"), (1, "b")):
        Wih = _reorder(inp[f"c_Wih_{suf}"], CH)
        Whh = _reorder(inp[f"c_Whh_{suf}"], CH)
        bias = _reorder(inp[f"c_bih_{suf}"] + inp[f"c_bhh_{suf}"], CH)
        l1_dir.append({
            "wih16": _chunkT(Wih.T.astype(np.float32), 2).astype(np.float16),
            "whh16": np.ascontiguousarray(Whh.T).astype(np.float16),
            "bias": np.ascontiguousarray(bias.reshape(4, 128).T.astype(np.float32)),
            "src": chars if d == 0 else chars[::-1],
        })
    in_maps1 = []
    for core in range(8):
        d, kk = core // 4, core % 4
        dd = l1_dir[d]
        pos = (2048 * kk - W1 + np.arange(NC1)).clip(0, C - 1)
        Xc = cemb[dd["src"][pos]]                      # [NC1, 256]
        xt16 = np.ascontiguousarray(
            Xc.T.reshape(2, 128, NC1).transpose(1, 0, 2).reshape(128, 2 * NC1)
        ).astype(np.float16)
        cpk1 = np.zeros((128, 4 + 3 * LC), np.float32)
        cpk1[:, 0:4] = dd["bias"]
        maskH = np.ones((128, LC), np.float32)
        fillH = np.zeros((128, LC), np.float32)
        fillC = np.zeros((128, LC), np.float32)
        if kk == 0:
            maskH[:, 0] = 0.0
            fillH[:, 0] = inp["c_h0"][d]
            fillC[:, 0] = inp["c_c0"][d]
        cpk1[:, 4:4 + LC] = maskH
        cpk1[:, 4 + LC:4 + 2 * LC] = fillH
        cpk1[:, 4 + 2 * LC:4 + 3 * LC] = fillC
        in_maps1.append({
            "xt16": xt16,
            "wih16": dd["wih16"],
            "whh16": dd["whh16"],
            "cpk": cpk1,
        })
    t0 = _time.time()
    r1 = run_bass_kernel_spmd(nc1, in_maps1, core_ids=list(range(8)),
                              trace=False, tmpdir=None)
    perf["l1_wall"] = _time.time() - t0
    # reassemble char hids: hout col = j*LC + l -> h at pos 2048*kk + 64*l + j
    chf = np.zeros((C, CH), np.float32)
    chb = np.zeros((C, CH), np.float32)
    for core in range(8):
        h = r1.results[core]["hout"].astype(np.float32)  # [128, LEN1*LC]
        d, kk = core // 4, core % 4
        hv = h.reshape(CH, LEN1, LC)  # [hid, j, l]
        pos = LEN1 * (LC * kk + np.arange(LC))[None, :] + np.arange(LEN1)[:, None]
        if d == 0:
            chf[pos.reshape(-1)] = hv.reshape(CH, -1).T
        else:
            chb[C - 1 - pos.reshape(-1)] = hv.reshape(CH, -1).T
    starts, ends = ix[:-1], ix[1:] - 1
    char_feats = np.concatenate(
        [chf[starts], chb[starts], chf[ends], chb[ends]], axis=1)  # [T, 512]

    # ---------------- L2 inputs
    l2_dir = []
    for d, suf in ((0, "f"), (1, "b")):
        Wih = _reorder(inp[f"w_Wih_{suf}"], WH)
        Whh = _reorder(inp[f"w_Whh_{suf}"], WH)
        bias = _reorder(inp[f"w_bih_{suf}"] + inp[f"w_bhh_{suf}"], WH)
        # 16-chunk pack: wihTwe (8) | wihTcf (4) | whhT (4), each [128, 2048]
        P = np.concatenate([
            _chunkT(np.ascontiguousarray(Wih[:, 512:].T), 8),
            _chunkT(np.ascontiguousarray(Wih[:, :512].T), 4),
            _chunkT(np.ascontiguousarray(Whh.T), 4),
        ], axis=1).astype(np.float16)            # [128, 16*2048]
        l2_dir.append({
            "P": P,
            "bias": np.ascontiguousarray(bias.reshape(16, 128).T.astype(np.float32)),
            "cf": char_feats if d == 0 else char_feats[::-1],
            "wsrc": words if d == 0 else words[::-1],
            "h2t": inp["hid2tag_W"][:, :WH] if d == 0 else inp["hid2tag_W"][:, WH:],
        })
    in_maps2 = []
    for core in range(8):
        d, kk = core // 4, core % 4
        dd = l2_dir[d]
        rows = (512 * kk - W2 + np.arange(WIN)).clip(0, T - 1)
        We = wemb[dd["wsrc"][rows]]                    # [WIN, 1024]
        embT16 = np.ascontiguousarray(
            We.T.reshape(8, 128, WIN).transpose(1, 0, 2).reshape(128, 8 * WIN)
        ).astype(np.float16)
        cfr = dd["cf"][rows]                           # [WIN, 512]
        cfT16 = np.ascontiguousarray(
            cfr.T.reshape(4, 128, WIN).transpose(1, 0, 2).reshape(128, 4 * WIN)
        ).astype(np.float16)
        cpk2 = np.zeros((128, 16 + 3 * 4 * LW + 24 + 6), np.float32)
        cpk2[:, 0:16] = dd["bias"]
        maskH = np.ones((128, 4 * LW), np.float32)
        fillH = np.zeros((128, 4 * LW), np.float32)
        fillC = np.zeros((128, 4 * LW), np.float32)
        if kk == 0:
            for k in range(4):
                maskH[:, k * LW] = 0.0
                fillH[:, k * LW] = inp["w_h0"][d][k * 128:(k + 1) * 128]
                fillC[:, k * LW] = inp["w_c0"][d][k * 128:(k + 1) * 128]
        cpk2[:, 16:144] = maskH
        cpk2[:, 144:272] = fillH
        cpk2[:, 272:400] = fillC
        cpk2[:, 400:424] = _chunkT(np.ascontiguousarray(dd["h2t"].T), 4)
        if d == 0:
            cpk2[:, 424:430] = inp["hid2tag_b"][None, :]
        in_maps2.append({
            "embT": embT16,
            "cfT": cfT16,
            "wsh": np.ascontiguousarray(dd["P"][:, kk * 8192:(kk + 1) * 8192]),
            "cpk": cpk2,
        })
    t0 = _time.time()
    r2 = run_bass_kernel_spmd(nc2, in_maps2, core_ids=list(range(8)),
                              trace=False, tmpdir=None)
    perf["l2_wall"] = _time.time() - t0
    fstack = np.zeros((8 * 512, 6), np.float32)
    for core in range(8):
        fp = r2.results[core]["fpart"]  # [512, 6] for global t block 512*kk
        d, kk = core // 4, core % 4
        if d == 0:
            fstack[512 * core:512 * (core + 1)] = fp
        else:
            fstack[512 * core:512 * (core + 1)] = fp[::-1]
    # bwd partials: core (4+kk) block covers reversed rows [512kk:512kk+512]
    # -> global t = T-1 - rev_t, i.e. global block [T-512(kk+1), T-512kk) reversed.
    bsec = fstack[4 * 512:].copy()
    fstack[4 * 512:] = 0
    for kk in range(4):
        blk = bsec[512 * kk:512 * (kk + 1)]
        g0 = T - 512 * (kk + 1)
        fstack[4 * 512 + g0:4 * 512 + g0 + 512] = blk

    # ---------------- L3 inputs
    trans = inp["transition"].astype(np.float32)
    c16 = np.zeros((16, 90), np.float32)
    c16[:, 0:36] = trans.reshape(1, 36)
    c16[:, 36:72] = (np.arange(36) % 6 - 6).astype(np.float32)[None, :]
    maskV = np.ones((16, 6), np.float32)
    maskV[0] = 0.0
    fillV = np.zeros((16, 6), np.float32)
    fv0 = np.full(6, NEG, np.float32)
    fv0[4] = 0.0
    fillV[0] = fv0
    c16[:, 72:78] = maskV
    c16[:, 78:84] = fillV
    c16[:, 84:90] = trans[:, 5][None, :]
    c96 = np.zeros((96, 826), np.float32)
    c96[:, 0:36] = (np.arange(36) // 6 - 6).astype(np.float32)[None, :]
    c96[:, 36:804] = (np.arange(768) % 6).astype(np.float32)[None, :]
    uinit = np.zeros((96, 6), np.float32)
    for e in range(6):
        uinit[16 * e:16 * (e + 1), e] = 1.0
    c96[:, 804:810] = uinit
    bmask = np.zeros((96, 16), np.float32)
    for e in range(6):
        for c in range(16):
            bmask[16 * e + c, c] = 1.0
    c96[:, 810:826] = bmask
    in_map3 = {"fstack": fstack, "c16": c16, "c96": c96}
    t0 = _time.time()
    r3 = run_bass_kernel_spmd(nc3, [in_map3], core_ids=[0],
                              trace=False, tmpdir=None)
    perf["l3_wall"] = _time.time() - t0
    _kernel_3launch.last_perf = perf
    _kernel_3launch.last_fstack = fstack
    return r3.results[0]["ids_o"].astype(np.int32)


_kernel_3launch.last_perf = {}


# ---------------------------------------------------------------- fused
# packed input layouts (cols)
OXT, OWIH, OWHH = 0, 4224, 5248
OEMB, OWSH, OC96 = 5760, 10368, 18560
NA = 19392                       # bigA f16 cols (19386 used)
OCPK1, OCPK2, OC16, OREV, OSEL = 0, 100, 530, 620, 748
NC32 = 752                       # cst f32 cols (750 used)


def build_fused():
    nc = _new_nc(8)
    bigA = nc.dram_tensor("bigA", [128, NA], F16, kind="ExternalInput")
    cst = nc.dram_tensor("cst", [128, NC32], F32, kind="ExternalInput")
    idx = nc.dram_tensor("idx", [4 * 640, 1], I32, kind="ExternalInput")
    ids_o = nc.dram_tensor("ids_o", [T], I32, kind="ExternalOutput")

    with tile.TileContext(nc) as tc:
        with tc.tile_pool(name="pp", bufs=1) as pp, \
             tc.tile_pool(name="dp", bufs=1, space="DRAM") as dp, \
             tc.tile_pool(name="tp", bufs=2) as tp:
            # ---- kick off the word-weight allgather early (independent)
            bounce_w = dp.tile([128, 16 * WH], F16)
            gathW = dp.tile([4 * 128 * 16 * WH], F16)
            nc.gpsimd.dma_start(bounce_w[:],
                                _dap(bigA[:], [[NA, 128], [1, 16 * WH]], extra_off=OWSH))
            nc.gpsimd.collective_compute(
                "AllGather", OP.bypass,
                replica_groups=[[0, 1, 2, 3], [4, 5, 6, 7]],
                ins=[bounce_w[:].opt()], outs=[gathW[:].opt()])
            ident = pp.tile([128, 128], F32)
            make_identity(nc, ident[:])
            cfT_s = pp.tile([128, 4 * WIN], F16)
            sel_s = pp.tile([128, 2], F32)
            nc.sync.dma_start(sel_s[:], _dap(cst[:], [[NC32, 128], [1, 2]], extra_off=OSEL))
            bounce_h = dp.tile([LEN1 * LC, 128], F16)
            # ================= phase 1: char LSTM =================
            with tc.tile_pool(name="p1", bufs=1) as p1, \
                 tc.tile_pool(name="ps1", bufs=2, space="PSUM") as psp:
                XT = p1.tile([128, 2 * NC1], F16)
                nc.sync.dma_start(XT[:], _dap(bigA[:], [[NA, 128], [1, 2 * NC1]], extra_off=OXT))
                wih_s = p1.tile([128, 2 * 4 * CH], F16)
                nc.sync.dma_start(wih_s[:], _dap(bigA[:], [[NA, 128], [1, 1024]], extra_off=OWIH))
                cpk_s = p1.tile([128, 4 + 3 * LC], F32)
                nc.sync.dma_start(cpk_s[:], _dap(cst[:], [[NC32, 128], [1, 100]], extra_off=OCPK1))
                mH = cpk_s[:, 4:4 + LC]
                fH = cpk_s[:, 4 + LC:4 + 2 * LC]
                fC = cpk_s[:, 4 + 2 * LC:4 + 3 * LC]
                xpT = p1.tile([128, 4 * NC1], F32)
                blocks = [(0, 512), (512, 512), (1024, 512), (1536, 512), (2048, 64)]
                for g in range(4):
                    for (c0, cw) in blocks:
                        psx = psp.tile([128, 512], F32, tag="psx", space="PSUM")
                        for k in range(2):
                            nc.tensor.matmul(out=psx[:, :cw],
                                             lhsT=wih_s[:, k * 512 + g * 128: k * 512 + (g + 1) * 128],
                                             rhs=XT[:, k * NC1 + c0: k * NC1 + c0 + cw],
                                             start=(k == 0), stop=(k == 1))
                        nc.vector.tensor_tensor(out=xpT[:, g * NC1 + c0: g * NC1 + c0 + cw],
                                                in0=psx[:, :cw],
                                                in1=cpk_s[:, g:g + 1].to_broadcast([128, cw]),
                                                op=OP.add)
                whh_s16 = p1.tile([128, 4 * CH], F16)
                nc.sync.dma_start(whh_s16[:], _dap(bigA[:], [[NA, 128], [1, 512]], extra_off=OWHH))
                whh_s = p1.tile([128, 4 * CH], F32)
                nc.vector.tensor_copy(out=whh_s[:], in_=whh_s16[:])
                hh = p1.tile([128, (S1 + 1) * LC], F32)
                cst1 = p1.tile([128, LC], F32)
                nc.vector.memset(hh[:, 0:LC], 0.0)
                nc.vector.memset(cst1[:], 0.0)
                for t in range(S1):
                    gps = psp.tile([128, 4 * LC], F32, tag="g", space="PSUM")
                    for g in range(4):
                        nc.tensor.matmul(out=gps[:, g * LC:(g + 1) * LC],
                                         lhsT=whh_s[:, g * 128:(g + 1) * 128],
                                         rhs=hh[:, t * LC:(t + 1) * LC],
                                         start=(g == 0), stop=(g == 3))
                    G = tp.tile([128, 4 * LC], F32, tag="G")
                    nc.vector.tensor_tensor(
                        out=_ap(G[:], [[LC, 4], [1, LC]]),
                        in0=_ap(gps[:], [[LC, 4], [1, LC]]),
                        in1=_ap(xpT[:], [[NC1, 4], [LEN1, LC]], extra_off=t),
                        op=OP.add)
                    Ssig = tp.tile([128, 3 * LC], F32, tag="S")
                    nc.scalar.activation(out=Ssig[:], in_=G[:, 0:3 * LC], func=AF.Sigmoid)
                    Tg = tp.tile([128, LC], F32, tag="Tg")
                    nc.scalar.activation(out=Tg[:], in_=G[:, 3 * LC:4 * LC], func=AF.Tanh)
                    t1 = tp.tile([128, LC], F32, tag="t1")
                    nc.vector.tensor_tensor(out=t1[:], in0=Ssig[:, 0:LC], in1=Tg[:], op=OP.mult)
                    nc.vector.tensor_tensor(out=cst1[:], in0=Ssig[:, LC:2 * LC], in1=cst1[:], op=OP.mult)
                    nc.vector.tensor_tensor(out=cst1[:], in0=cst1[:], in1=t1[:], op=OP.add)
                    Tc = tp.tile([128, LC], F32, tag="Tc")
                    nc.scalar.activation(out=Tc[:], in_=cst1[:], func=AF.Tanh)
                    nc.vector.tensor_tensor(out=hh[:, (t + 1) * LC:(t + 2) * LC],
                                            in0=Ssig[:, 2 * LC:3 * LC], in1=Tc[:], op=OP.mult)
                    if t == W1 - 1:
                        blk = hh[:, (t + 1) * LC:(t + 2) * LC]
                        nc.vector.tensor_tensor(out=blk, in0=blk, in1=mH[:], op=OP.mult)
                        nc.vector.tensor_tensor(out=blk, in0=blk, in1=fH[:], op=OP.add)
                        nc.vector.tensor_tensor(out=cst1[:], in0=cst1[:], in1=mH[:], op=OP.mult)
                        nc.vector.tensor_tensor(out=cst1[:], in0=cst1[:], in1=fC[:], op=OP.add)
                # transpose post-warmup h to row-major f16 and stage to DRAM
                for b in range(16):
                    pst = psp.tile([128, 128], F32, tag="tr", space="PSUM")
                    nc.tensor.transpose(out=pst[:],
                                        in_=hh[:, (W1 + 1) * LC + b * 128:(W1 + 1) * LC + (b + 1) * 128],
                                        identity=ident[:])
                    hr = tp.tile([128, 128], F16, tag="hr")
                    nc.vector.tensor_copy(out=hr[:], in_=pst[:])
                    nc.sync.dma_start(
                        _dap(bounce_h[:], [[128, 128], [1, 128]], extra_off=b * 128 * 128),
                        hr[:])
            # allgather char hiddens (8 cores): gathH[rank, r, hid]
            gathH = dp.tile([8 * LEN1 * LC * 128], F16)
            nc.gpsimd.collective_compute(
                "AllGather", OP.bypass,
                replica_groups=[list(range(8))],
                ins=[bounce_h[:].opt()], outs=[gathH[:].opt()])
            # ================= phase 2: build cfT from gathered char hiddens
            idxs = pp.tile([128, 20], I32)
            nc.sync.dma_start(idxs[:].rearrange("p (b o) -> p b o", b=20),
                              idx[:].rearrange("(b p) o -> p b o", p=128))
            with tc.tile_pool(name="p2", bufs=2) as p2, \
                 tc.tile_pool(name="ps2", bufs=2, space="PSUM") as psp:
                for grp in range(4):
                    for b in range(5):
                        cw = 128 if b < 4 else WIN - 512
                        g16 = p2.tile([128, 128], F16, tag="g16")
                        nc.gpsimd.indirect_dma_start(
                            out=g16[:], out_offset=None,
                            in_=_dap(gathH[:], [[128, 8 * LEN1 * LC], [1, 128]]),
                            in_offset=bass.IndirectOffsetOnAxis(
                                ap=idxs[:, grp * 5 + b: grp * 5 + b + 1], axis=0))
                        g32 = p2.tile([128, 128], F32, tag="g32")
                        nc.vector.tensor_copy(out=g32[:], in_=g16[:])
                        pst = psp.tile([128, 128], F32, tag="tr2", space="PSUM")
                        nc.tensor.transpose(out=pst[:], in_=g32[:], identity=ident[:])
                        nc.vector.tensor_copy(
                            out=cfT_s[:, grp * WIN + b * 128: grp * WIN + b * 128 + cw],
                            in_=pst[:, :cw])
            # ================= phase 3: word LSTM =================
            xpT2 = pp.tile([128, 16 * WIN], F32)
            whh2_s = pp.tile([128, 4 * 4 * WH], F32)
            cpk2_s = pp.tile([128, 430], F32)
            nc.sync.dma_start(cpk2_s[:], _dap(cst[:], [[NC32, 128], [1, 430]], extra_off=OCPK2))
            m2H = cpk2_s[:, 16:16 + 128]
            f2H = cpk2_s[:, 144:144 + 128]
            f2C = cpk2_s[:, 272:272 + 128]
            with tc.tile_pool(name="p3", bufs=1) as p3, \
                 tc.tile_pool(name="ps3", bufs=2, space="PSUM") as psp:
                embT_s = p3.tile([128, 8 * WIN], F16)
                nc.sync.dma_start(embT_s[:], _dap(bigA[:], [[NA, 128], [1, 8 * WIN]], extra_off=OEMB))
                wih2_s = p3.tile([128, 12 * 2048], F16)
                SH = 16 * WH
                for r in range(3):
                    nc.sync.dma_start(
                        wih2_s[:, r * SH:(r + 1) * SH],
                        _dap(gathW[:], [[SH, 128], [1, SH]], extra_off=r * 128 * SH))
                whh2_s16 = p3.tile([128, SH], F16)
                nc.sync.dma_start(
                    whh2_s16[:],
                    _dap(gathW[:], [[SH, 128], [1, SH]], extra_off=3 * 128 * SH))
                nc.vector.tensor_copy(out=whh2_s[:], in_=whh2_s16[:])
                for g in range(16):
                    for (c0, cw) in ((0, 288), (288, 288)):
                        psx = psp.tile([128, 288], F32, tag="psx2", space="PSUM")
                        for k in range(8):
                            nc.tensor.matmul(out=psx[:, :cw],
                                             lhsT=wih2_s[:, k * 2048 + g * 128: k * 2048 + (g + 1) * 128],
                                             rhs=embT_s[:, k * WIN + c0: k * WIN + c0 + cw],
                                             start=(k == 0), stop=False)
                        for k in range(4):
                            nc.tensor.matmul(out=psx[:, :cw],
                                             lhsT=wih2_s[:, (8 + k) * 2048 + g * 128: (8 + k) * 2048 + (g + 1) * 128],
                                             rhs=cfT_s[:, k * WIN + c0: k * WIN + c0 + cw],
                                             start=False, stop=(k == 3))
                        nc.vector.tensor_tensor(out=xpT2[:, g * WIN + c0: g * WIN + c0 + cw],
                                                in0=psx[:, :cw],
                                                in1=cpk2_s[:, g:g + 1].to_broadcast([128, cw]),
                                                op=OP.add)
            bounce_f = dp.tile([512, 6], F32)
            with tc.tile_pool(name="p4", bufs=1) as p4, \
                 tc.tile_pool(name="ps4", bufs=2, space="PSUM") as psp:
                hh2 = p4.tile([128, (S2 + 1) * 4 * LW], F32)
                cst2 = p4.tile([128, 4 * LW], F32)
                nc.vector.memset(hh2[:, 0:4 * LW], 0.0)
                nc.vector.memset(cst2[:], 0.0)
                for t in range(S2):
                    gps = psp.tile([128, 16 * LW], F32, tag="g2", space="PSUM")
                    for m in range(16):
                        for k in range(4):
                            nc.tensor.matmul(out=gps[:, m * LW:(m + 1) * LW],
                                             lhsT=whh2_s[:, k * 2048 + m * 128: k * 2048 + (m + 1) * 128],
                                             rhs=hh2[:, t * 4 * LW + k * LW: t * 4 * LW + (k + 1) * LW],
                                             start=(k == 0), stop=(k == 3))
                    G = tp.tile([128, 16 * LW], F32, tag="G2")
                    nc.vector.tensor_tensor(
                        out=_ap(G[:], [[LW, 16], [1, LW]]),
                        in0=_ap(gps[:], [[LW, 16], [1, LW]]),
                        in1=_ap(xpT2[:], [[WIN, 16], [LEN2, LW]], extra_off=t),
                        op=OP.add)
                    Ssig = tp.tile([128, 12 * LW], F32, tag="S2")
                    nc.scalar.activation(out=Ssig[:], in_=G[:, 0:12 * LW], func=AF.Sigmoid)
                    Tg = tp.tile([128, 4 * LW], F32, tag="Tg2")
                    nc.scalar.activation(out=Tg[:], in_=G[:, 12 * LW:16 * LW], func=AF.Tanh)
                    t1 = tp.tile([128, 4 * LW], F32, tag="t12")
                    nc.vector.tensor_tensor(out=t1[:], in0=Ssig[:, 0:4 * LW], in1=Tg[:], op=OP.mult)
                    nc.vector.tensor_tensor(out=cst2[:], in0=Ssig[:, 4 * LW:8 * LW], in1=cst2[:], op=OP.mult)
                    nc.vector.tensor_tensor(out=cst2[:], in0=cst2[:], in1=t1[:], op=OP.add)
                    Tc = tp.tile([128, 4 * LW], F32, tag="Tc2")
                    nc.scalar.activation(out=Tc[:], in_=cst2[:], func=AF.Tanh)
                    nc.vector.tensor_tensor(out=hh2[:, (t + 1) * 4 * LW:(t + 2) * 4 * LW],
                                            in0=Ssig[:, 8 * LW:12 * LW], in1=Tc[:], op=OP.mult)
                    if t == W2 - 1:
                        blk = hh2[:, (t + 1) * 4 * LW:(t + 2) * 4 * LW]
                        nc.vector.tensor_tensor(out=blk, in0=blk, in1=m2H[:], op=OP.mult)
                        nc.vector.tensor_tensor(out=blk, in0=blk, in1=f2H[:], op=OP.add)
                        nc.vector.tensor_tensor(out=cst2[:], in0=cst2[:], in1=m2H[:], op=OP.mult)
                        nc.vector.tensor_tensor(out=cst2[:], in0=cst2[:], in1=f2C[:], op=OP.add)
                # repack post-warmup h (t-major) then feats partial
                hT = p4.tile([128, 4 * 512], F32)
                for k in range(4):
                    nc.vector.tensor_copy(
                        out=_ap(hT[:], [[16, 32], [1, 16]], extra_off=k * 512),
                        in_=_ap(hh2[:], [[1, 32], [4 * LW, 16]],
                                extra_off=(W2 + 1) * 4 * LW + k * LW))
                fp_s = p4.tile([128, 4 * 6], F32)
                for m in range(4):
                    psf = psp.tile([128, 6], F32, tag="psf", space="PSUM")
                    for k in range(4):
                        nc.tensor.matmul(out=psf[:],
                                         lhsT=hT[:, k * 512 + m * 128: k * 512 + (m + 1) * 128],
                                         rhs=cpk2_s[:, 400 + k * 6:400 + (k + 1) * 6],
                                         start=(k == 0), stop=(k == 3))
                    nc.vector.tensor_tensor(out=fp_s[:, m * 6:(m + 1) * 6], in0=psf[:],
                                            in1=cpk2_s[:, 424:430], op=OP.add)
                # data-driven block reversal for bwd cores: psr = REV^T @ fp
                rev_s = p4.tile([128, 128], F32)
                nc.sync.dma_start(rev_s[:], _dap(cst[:], [[NC32, 128], [1, 128]], extra_off=OREV))
                psr = psp.tile([128, 24], F32, tag="psr", space="PSUM")
                nc.tensor.matmul(out=psr[:], lhsT=rev_s[:], rhs=fp_s[:], start=True, stop=True)
                tA = p4.tile([128, 24], F32)
                nc.vector.tensor_tensor(out=tA[:], in0=psr[:],
                                        in1=sel_s[:, 0:1].to_broadcast([128, 24]), op=OP.mult)
                fpB = p4.tile([128, 24], F32)
                for m in range(4):
                    nc.vector.tensor_copy(out=fpB[:, m * 6:(m + 1) * 6],
                                          in_=psr[:, (3 - m) * 6:(4 - m) * 6])
                nc.vector.tensor_tensor(out=fpB[:], in0=fpB[:],
                                        in1=sel_s[:, 1:2].to_broadcast([128, 24]), op=OP.mult)
                fpO = p4.tile([128, 24], F32)
                nc.vector.tensor_tensor(out=fpO[:], in0=tA[:], in1=fpB[:], op=OP.add)
                nc.sync.dma_start(bounce_f[:].rearrange("(m p) s -> p m s", p=128),
                                  fpO[:].rearrange("p (m s) -> p m s", m=4))
            gathF = dp.tile([8 * 512 * 6], F32)
            nc.gpsimd.collective_compute(
                "AllGather", OP.bypass,
                replica_groups=[list(range(8))],
                ins=[bounce_f[:].opt()], outs=[gathF[:].opt()])
            # ================= phase 5: viterbi (replicated on all cores)
            with tc.tile_pool(name="p5", bufs=1) as p5, \
                 tc.tile_pool(name="ps5", bufs=2, space="PSUM") as psp:
                Ff = p5.tile([128, 16 * 6], F32)
                Fb = p5.tile([128, 16 * 6], F32)
                for k in range(4):
                    nc.sync.dma_start(Ff[32 * k:32 * (k + 1), :],
                                      _dap(gathF[:], [[96, 32], [1, 96]], extra_off=k * 3072))
                    nc.sync.dma_start(Fb[32 * k:32 * (k + 1), :],
                                      _dap(gathF[:], [[96, 32], [1, 96]], extra_off=(7 - k) * 3072))
                F = p5.tile([128, 16 * 6], F32)
                nc.vector.tensor_tensor(out=F[:], in0=Ff[:], in1=Fb[:], op=OP.add)
                featsD = dp.tile([T * 6], F32)
                nc.sync.dma_start(featsD[:].rearrange("(p a) -> p a", p=128), F[:])
                fsub = p5.tile([16, SV * 6], F32)
                fD = featsD[:]
                for p in range(16):
                    if p == 0:
                        nc.sync.dma_start(fsub[0:1, 0:WV * 6], _dap(fD, [[WV * 6, 1], [1, WV * 6]]))
                        nc.sync.dma_start(fsub[0:1, WV * 6:SV * 6], _dap(fD, [[LV * 6, 1], [1, LV * 6]]))
                    else:
                        nc.sync.dma_start(fsub[p:p + 1, :],
                                          _dap(fD, [[SV * 6, 1], [1, SV * 6]], extra_off=(p * LV - WV) * 6))
                c16_s = p5.tile([16, 90], F32)
                nc.sync.dma_start(c16_s[:], _dap(cst[:], [[NC32, 16], [1, 90]], extra_off=OC16))
                fv = p5.tile([16, 6], F32)
                nc.vector.memset(fv[:], 0.0)
                bpsH = p5.tile([16, LV * 6], F32)
                for t in range(SV):
                    if t == WV:
                        nc.vector.tensor_tensor(out=fv[:], in0=fv[:], in1=c16_s[:, 72:78], op=OP.mult)
                        nc.vector.tensor_tensor(out=fv[:], in0=fv[:], in1=c16_s[:, 78:84], op=OP.add)
                    tmp = tp.tile([16, 36], F32, tag="tmp")
                    nc.vector.tensor_tensor(out=_ap(tmp[:], [[6, 6], [1, 6]]),
                                            in0=_ap(c16_s[:], [[6, 6], [1, 6]]),
                                            in1=_ap(fv[:], [[0, 6], [1, 6]]), op=OP.add)
                    mx = tp.tile([16, 6], F32, tag="mx")
                    nc.vector.tensor_reduce(out=mx[:], in_=_ap(tmp[:], [[6, 6], [1, 6]]),
                                            axis=AX.X, op=OP.max)
                    eq = tp.tile([16, 36], F32, tag="eq")
                    nc.vector.tensor_tensor(out=_ap(eq[:], [[6, 6], [1, 6]]),
                                            in0=_ap(tmp[:], [[6, 6], [1, 6]]),
                                            in1=_ap(mx[:], [[1, 6], [0, 6]]), op=OP.is_ge)
                    nc.vector.tensor_tensor(out=eq[:], in0=eq[:], in1=c16_s[:, 36:72], op=OP.mult)
                    if t >= WV:
                        nc.vector.tensor_reduce(out=bpsH[:, (t - WV) * 6:(t - WV + 1) * 6],
                                                in_=_ap(eq[:], [[6, 6], [1, 6]]), axis=AX.X, op=OP.min)
                    nc.vector.tensor_tensor(out=fv[:], in0=mx[:], in1=fsub[:, t * 6:(t + 1) * 6], op=OP.add)
                av = p5.tile([16, 6], F32)
                nc.vector.tensor_tensor(out=av[:], in0=fv[:], in1=c16_s[:, 84:90], op=OP.add)
                am = p5.tile([16, 1], F32)
                nc.vector.tensor_reduce(out=am[:], in_=av[:], axis=AX.X, op=OP.max)
                ohf = p5.tile([16, 6], F32)
                nc.vector.tensor_tensor(out=ohf[:], in0=av[:], in1=am[:].to_broadcast([16, 6]), op=OP.is_ge)
                bpsD = dp.tile([16 * LV * 6], F32)
                nc.sync.dma_start(bpsD[:].rearrange("(p a) -> p a", p=16), bpsH[:])
                bpsR = p5.tile([96, LV * 6], F32)
                for e in range(6):
                    nc.sync.dma_start(bpsR[16 * e:16 * (e + 1), :],
                                      bpsD[:].rearrange("(p a) -> p a", p=16))
                c96t = p5.tile([96, 826], F16)
                nc.sync.dma_start(c96t[:], _dap(bigA[:], [[NA, 96], [1, 826]], extra_off=OC96))
                c96_s = p5.tile([96, 826], F32)
                nc.vector.tensor_copy(out=c96_s[:], in_=c96t[:])
                uH = p5.tile([96, (LV + 1) * 6], F32)
                nc.vector.tensor_copy(out=uH[:, LV * 6:(LV + 1) * 6], in_=c96_s[:, 804:810])
                for tb in range(LV - 1, -1, -1):
                    eqB = tp.tile([96, 36], F32, tag="eqB")
                    nc.vector.tensor_tensor(out=_ap(eqB[:], [[6, 6], [1, 6]]),
                                            in0=_ap(bpsR[:], [[0, 6], [1, 6]], extra_off=tb * 6),
                                            in1=_ap(c96_s[:], [[6, 6], [1, 6]]), op=OP.is_equal)
                    tB = tp.tile([96, 36], F32, tag="tB")
                    nc.vector.tensor_tensor(out=_ap(tB[:], [[6, 6], [1, 6]]),
                                            in0=_ap(eqB[:], [[6, 6], [1, 6]]),
                                            in1=_ap(uH[:], [[0, 6], [1, 6]], extra_off=(tb + 1) * 6),
                                            op=OP.mult)
                    nc.vector.tensor_reduce(out=uH[:, tb * 6:(tb + 1) * 6],
                                            in_=_ap(tB[:], [[6, 6], [1, 6]]), axis=AX.X, op=OP.max)
                idsA = p5.tile([96, LV], F32)
                tJ = p5.tile([96, 768], F32)
                nc.vector.tensor_tensor(out=tJ[:], in0=uH[:, 6:(LV + 1) * 6], in1=c96_s[:, 36:804], op=OP.mult)
                nc.vector.tensor_reduce(out=idsA[:], in_=_ap(tJ[:], [[6, LV], [1, 6]]), axis=AX.X, op=OP.max)
                uD = dp.tile([96 * 6], F32)
                nc.sync.dma_start(uD[:].rearrange("(p a) -> p a", p=96), uH[:, 0:6])
                MT2 = p5.tile([1, 16 * 36], F32)
                nc.sync.dma_start(MT2[:], _dap(uD[:], [[576, 1], [6, 16], [1, 6], [96, 6]]))
                ohfD = dp.tile([16 * 6], F32)
                nc.sync.dma_start(ohfD[:].rearrange("(p a) -> p a", p=16), ohf[:])
                ohSeq = p5.tile([1, 16 * 6], F32)
                nc.sync.dma_start(ohSeq[0:1, 15 * 6:16 * 6],
                                  _dap(ohfD[:], [[6, 1], [1, 6]], extra_off=15 * 6))
                for c in range(14, -1, -1):
                    tS2 = tp.tile([1, 36], F32, tag="tS2")
                    nc.vector.tensor_tensor(out=_ap(tS2[:], [[6, 6], [1, 6]]),
                                            in0=_ap(MT2[:], [[6, 6], [1, 6]], extra_off=(c + 1) * 36),
                                            in1=_ap(ohSeq[:], [[0, 6], [1, 6]], extra_off=(c + 1) * 6),
                                            op=OP.mult)
                    nc.vector.tensor_reduce(out=ohSeq[0:1, c * 6:(c + 1) * 6],
                                            in_=_ap(tS2[:], [[6, 6], [1, 6]]), axis=AX.X, op=OP.max)
                ohD = dp.tile([16 * 6], F32)
                nc.sync.dma_start(ohD[:].rearrange("(p a) -> p a", p=1), ohSeq[:])
                selC = p5.tile([96, 1], F32)
                for e in range(6):
                    nc.sync.dma_start(selC[16 * e:16 * (e + 1), :],
                                      _dap(ohD[:], [[6, 16], [1, 1]], extra_off=e))
                SEL = p5.tile([96, 16], F32)
                nc.vector.tensor_tensor(out=SEL[:], in0=selC[:].to_broadcast([96, 16]),
                                        in1=c96_s[:, 810:826], op=OP.mult)
                psi = psp.tile([16, LV], F32, tag="psi", space="PSUM")
                nc.tensor.matmul(out=psi[:], lhsT=SEL[:], rhs=idsA[:], start=True, stop=True)
                idsI = p5.tile([16, LV], I32)
                nc.vector.tensor_copy(out=idsI[:], in_=psi[:])
                nc.sync.dma_start(ids_o[:].rearrange("(p a) -> p a", p=16), idsI[:])
    nc.compile()
    return nc


def _programs_fused():
    if "fused" not in _cache:
        _cache["fused"] = build_fused()
    return _cache["fused"]


def _cf_rows_fwd(p):
    kk2 = p // 2048
    pl = p % 2048
    return kk2 * 2048 + (pl % 64) * 32 + (pl // 64)


def _cf_rows_bwd(p):
    pb = (C - 1) - p
    kk2 = pb // 2048
    pl = pb % 2048
    return (4 + kk2) * 2048 + (pl % 64) * 32 + (pl // 64)


def kernel_fused(**inp):
    inp = {k: np.asarray(v) for k, v in inp.items()}
    ncf = _programs_fused()
    perf = {}

    chars = inp["chars"].astype(np.int32)
    words = inp["words"].astype(np.int32)
    ix = inp["ix_seq"].astype(np.int64)
    cemb = inp["char_embed"].astype(np.float32)
    wemb = inp["word_embed"].astype(np.float32)

    l1_dir = []
    for d, suf in ((0, "f"), (1, "b")):
        Wih = _reorder(inp[f"c_Wih_{suf}"], CH)
        Whh = _reorder(inp[f"c_Whh_{suf}"], CH)
        bias = _reorder(inp[f"c_bih_{suf}"] + inp[f"c_bhh_{suf}"], CH)
        l1_dir.append({
            "wih16": _chunkT(Wih.T.astype(np.float32), 2).astype(np.float16),
            "whh16": np.ascontiguousarray(Whh.T).astype(np.float16),
            "bias": np.ascontiguousarray(bias.reshape(4, 128).T.astype(np.float32)),
            "src": chars if d == 0 else chars[::-1],
        })
    l2_dir = []
    for d, suf in ((0, "f"), (1, "b")):
        Wih = _reorder(inp[f"w_Wih_{suf}"], WH)
        Whh = _reorder(inp[f"w_Whh_{suf}"], WH)
        bias = _reorder(inp[f"w_bih_{suf}"] + inp[f"w_bhh_{suf}"], WH)
        P = np.concatenate([
            _chunkT(np.ascontiguousarray(Wih[:, 512:].T), 8),
            _chunkT(np.ascontiguousarray(Wih[:, :512].T), 4),
            _chunkT(np.ascontiguousarray(Whh.T), 4),
        ], axis=1).astype(np.float16)
        l2_dir.append({
            "P": P,
            "bias": np.ascontiguousarray(bias.reshape(16, 128).T.astype(np.float32)),
            "wsrc": words if d == 0 else words[::-1],
            "h2t": inp["hid2tag_W"][:, :WH] if d == 0 else inp["hid2tag_W"][:, WH:],
        })
    # viterbi constant packs
    trans = inp["transition"].astype(np.float32)
    c16v = np.zeros((16, 90), np.float32)
    c16v[:, 0:36] = trans.reshape(1, 36)
    c16v[:, 36:72] = (np.arange(36) % 6 - 6).astype(np.float32)[None, :]
    maskV = np.ones((16, 6), np.float32)
    maskV[0] = 0.0
    fillV = np.zeros((16, 6), np.float32)
    fv0 = np.full(6, NEG, np.float32)
    fv0[4] = 0.0
    fillV[0] = fv0
    c16v[:, 72:78] = maskV
    c16v[:, 78:84] = fillV
    c16v[:, 84:90] = trans[:, 5][None, :]
    c96v = np.zeros((96, 826), np.float16)
    c96v[:, 0:36] = (np.arange(36) // 6 - 6).astype(np.float16)[None, :]
    c96v[:, 36:804] = (np.arange(768) % 6).astype(np.float16)[None, :]
    uinit = np.zeros((96, 6), np.float16)
    for e in range(6):
        uinit[16 * e:16 * (e + 1), e] = 1.0
    c96v[:, 804:810] = uinit
    bmask = np.zeros((96, 16), np.float16)
    for e in range(6):
        for c in range(16):
            bmask[16 * e + c, c] = 1.0
    c96v[:, 810:826] = bmask

    in_maps = []
    for core in range(8):
        d, kk = core // 4, core % 4
        d1 = l1_dir[d]
        d2 = l2_dir[d]
        bigA = np.zeros((128, NA), np.float16)
        cstv = np.zeros((128, NC32), np.float32)
        # char window
        pos = (2048 * kk - W1 + np.arange(NC1)).clip(0, C - 1)
        Xc = cemb[d1["src"][pos]]
        bigA[:, OXT:OXT + 2 * NC1] = np.ascontiguousarray(
            Xc.T.reshape(2, 128, NC1).transpose(1, 0, 2).reshape(128, 2 * NC1))
        bigA[:, OWIH:OWIH + 1024] = d1["wih16"]
        bigA[:, OWHH:OWHH + 512] = d1["whh16"]
        # word window
        rows = (512 * kk - W2 + np.arange(WIN)).clip(0, T - 1)
        We = wemb[d2["wsrc"][rows]]
        bigA[:, OEMB:OEMB + 8 * WIN] = np.ascontiguousarray(
            We.T.reshape(8, 128, WIN).transpose(1, 0, 2).reshape(128, 8 * WIN))
        bigA[:, OWSH:OWSH + 8192] = d2["P"][:, kk * 8192:(kk + 1) * 8192]
        bigA[0:96, OC96:OC96 + 826] = c96v
        # cst pack
        cpk1 = np.zeros((128, 100), np.float32)
        cpk1[:, 0:4] = d1["bias"]
        maskH = np.ones((128, LC), np.float32)
        fillH = np.zeros((128, LC), np.float32)
        fillC = np.zeros((128, LC), np.float32)
        if kk == 0:
            maskH[:, 0] = 0.0
            fillH[:, 0] = inp["c_h0"][d]
            fillC[:, 0] = inp["c_c0"][d]
        cpk1[:, 4:4 + LC] = maskH
        cpk1[:, 4 + LC:4 + 2 * LC] = fillH
        cpk1[:, 4 + 2 * LC:4 + 3 * LC] = fillC
        cstv[:, OCPK1:OCPK1 + 100] = cpk1
        cpk2 = np.zeros((128, 430), np.float32)
        cpk2[:, 0:16] = d2["bias"]
        maskH2 = np.ones((128, 4 * LW), np.float32)
        fillH2 = np.zeros((128, 4 * LW), np.float32)
        fillC2 = np.zeros((128, 4 * LW), np.float32)
        if kk == 0:
            for k in range(4):
                maskH2[:, k * LW] = 0.0
                fillH2[:, k * LW] = inp["w_h0"][d][k * 128:(k + 1) * 128]
                fillC2[:, k * LW] = inp["w_c0"][d][k * 128:(k + 1) * 128]
        cpk2[:, 16:144] = maskH2
        cpk2[:, 144:272] = fillH2
        cpk2[:, 272:400] = fillC2
        cpk2[:, 400:424] = _chunkT(np.ascontiguousarray(d2["h2t"].T), 4)
        if d == 0:
            cpk2[:, 424:430] = inp["hid2tag_b"][None, :]
        cstv[:, OCPK2:OCPK2 + 430] = cpk2
        cstv[0:16, OC16:OC16 + 90] = c16v
        rev = np.zeros((128, 128), np.float32)
        if d == 0:
            rev[np.arange(128), np.arange(128)] = 1.0
            cstv[:, OSEL] = 1.0
        else:
            rev[np.arange(128), 127 - np.arange(128)] = 1.0
            cstv[:, OSEL + 1] = 1.0
        cstv[:, OREV:OREV + 128] = rev
        # cf gather indices
        tglob = rows if d == 0 else (T - 1 - rows)
        st = ix[tglob].astype(np.int64)
        en = (ix[tglob + 1] - 1).astype(np.int64)
        idxv = np.zeros((4, 640), np.int32)
        idxv[0, :WIN] = _cf_rows_fwd(st)
        idxv[1, :WIN] = _cf_rows_bwd(st)
        idxv[2, :WIN] = _cf_rows_fwd(en)
        idxv[3, :WIN] = _cf_rows_bwd(en)
        in_maps.append({
            "bigA": bigA,
            "cst": cstv,
            "idx": idxv.reshape(-1, 1),
        })
    t0 = _time.time()
    r = run_bass_kernel_spmd(ncf, in_maps, core_ids=list(range(8)),
                             trace=False, tmpdir=None)
    perf["fused_wall"] = _time.time() - t0
    kernel_fused.last_perf = perf
    return r.results[0]["ids_o"].astype(np.int32)


kernel_fused.last_perf = {}

FUSED = True


def kernel_dispatch(**inp):
    if FUSED:
        out = kernel_fused(**inp)
        kernel.last_perf = kernel_fused.last_perf
        return out
    return _kernel_3launch(**inp)


def kernel(**inp):
    return kernel_dispatch(**inp)


kernel.last_perf = {}


# revision 11
# speedup vs baseline: 59.6502x; 1.3833x over previous
"""Trainium2 Bass kernel for nn_ConcatCharLSTM_LSTM_CRF.

Strategy (8 NeuronCores, SPMD, three device launches; host does the
inter-launch data movement). The dominant cost through this runtime path is
host->device transfer bandwidth (~35 MB/s effective), so the design
minimizes shipped bytes:
  - embeddings are gathered on host (pure data movement) and shipped as the
    per-core windows actually consumed, in fp16;
  - LSTM weights ship in fp16; the word-LSTM weight pack is sharded 4-ways
    per direction and reconstructed on-device with an AllGather collective;
  - matmuls against shipped fp16 data run in fp16 (fp32 PSUM accumulate);
    the recurrent path stays fp32.
Layers:
  L1: char BiLSTM. Per core: 32 chunk-lanes x 64 steps + 64 warmup steps
      (LSTM forget-gate contraction decays chunk-boundary state error).
      4 cores fwd + 4 bwd. Compact window: 2112 = 64 + 32*64 input columns,
      scan indexes lane l step t at column 64*l + t.
  L2: word BiLSTM, same scheme (512 cols + 64 warmup per core) + hid2tag
      partial feats.
  L3: Viterbi forward scan (16 time-chunks as partition sublanes with
      warmup) + exact chunked backtrace via one-hot map composition, 1 core.
"""

import os
import sys
import numpy as np
import time as _time

sys.path.insert(0, "/opt/trn_rl_repo")
os.environ.setdefault("JAX_PLATFORMS", "axon,cpu")
os.environ.setdefault("JAX_COMPILATION_CACHE_DIR", "/root/.cache/jax_bass")

from concourse import bass, mybir
from concourse import bacc
import concourse.tile as tile
from concourse.bass_utils import run_bass_kernel_spmd
from concourse.masks import make_identity

try:
    import jax
    jax.config.update("jax_compilation_cache_dir", "/root/.cache/jax_bass")
    jax.config.update("jax_persistent_cache_min_entry_size_bytes", -1)
    jax.config.update("jax_persistent_cache_min_compile_time_secs", 0.0)
except Exception:
    pass

F32 = mybir.dt.float32
F16 = mybir.dt.float16
I32 = mybir.dt.int32
AF = mybir.ActivationFunctionType
OP = mybir.AluOpType
AX = mybir.AxisListType

# problem constants
T, C, V, WD, CS, CD = 2048, 8192, 50000, 1024, 8000, 256
CH, WH = 128, 512            # per-direction hidden sizes
NEG = -10000.0

# chunking parameters
LC, LEN1, W1 = 32, 64, 64    # char: lanes/core, chunk len, warmup
S1 = LEN1 + W1               # char steps per core = 128
NC1 = LEN1 * LC + W1         # compact char cols per core = 2112
LW, LEN2, W2 = 32, 16, 64    # word
S2 = LEN2 + W2               # 80
WIN = LEN2 * LW + W2         # word per-core column window = 576
NV, LV, WV = 16, 128, 32     # viterbi chunks, chunk len, warmup
SV = LV + WV                 # 160

# gate reorder: torch (i,f,g,o) -> (i,f,o,g) so sigmoid cols are contiguous
PERM = (0, 1, 3, 2)


def _reorder(w, H):
    """reorder gate blocks of leading dim 4H from (i,f,g,o) to (i,f,o,g)."""
    blocks = [w[i * H:(i + 1) * H] for i in range(4)]
    return np.concatenate([blocks[p] for p in PERM], axis=0)


def _chunkT(a, nk):
    """[R, G] -> [128, nk*G] where col k*G+g holds a[k*128+p, g] (R = nk*128)."""
    R, G = a.shape
    assert R == nk * 128
    return np.ascontiguousarray(
        a.reshape(nk, 128, G).transpose(1, 0, 2).reshape(128, nk * G))


def _ap(ap, dims, extra_off=0):
    """Build an AP with custom free dims [[step,count],...] keeping partition dim."""
    return bass.AP(ap.tensor, ap.offset + extra_off, [list(ap.ap[0])] + [list(d) for d in dims])


def _dap(ap, dims, extra_off=0):
    """Build an AP replacing ALL dims (for DRAM tensors)."""
    return bass.AP(ap.tensor, ap.offset + extra_off, [list(d) for d in dims])


def _new_nc(num_devices):
    return bacc.Bacc("TRN2", target_bir_lowering=False, debug=False,
                     num_devices=num_devices)


# ---------------------------------------------------------------- L1: char
def build_l1():
    nc = _new_nc(8)
    xt16 = nc.dram_tensor("xt16", [128, 2 * NC1], F16, kind="ExternalInput")
    wih16 = nc.dram_tensor("wih16", [128, 2 * 4 * CH], F16, kind="ExternalInput")
    whh16 = nc.dram_tensor("whh16", [128, 4 * CH], F16, kind="ExternalInput")
    cpk = nc.dram_tensor("cpk", [128, 4 + 3 * LC], F32, kind="ExternalInput")
    hout = nc.dram_tensor("hout", [128, LEN1 * LC], F16, kind="ExternalOutput")

    with tile.TileContext(nc) as tc:
        with tc.tile_pool(name="p", bufs=1) as pp, \
             tc.tile_pool(name="ps", bufs=2, space="PSUM") as psp, \
             tc.tile_pool(name="tmp", bufs=2) as tp:
            XT = pp.tile([128, 2 * NC1], F16)
            nc.sync.dma_start(XT[:], xt16[:])
            wih_s = pp.tile([128, 2 * 4 * CH], F16)
            nc.sync.dma_start(wih_s[:], wih16[:])
            cpk_s = pp.tile([128, 4 + 3 * LC], F32)
            nc.sync.dma_start(cpk_s[:], cpk[:])
            mH = cpk_s[:, 4:4 + LC]
            fH = cpk_s[:, 4 + LC:4 + 2 * LC]
            fC = cpk_s[:, 4 + 2 * LC:4 + 3 * LC]
            # bulk xproj: xpT [128, 4*NC1] (gate-chunk major)
            xpT = pp.tile([128, 4 * NC1], F32)
            blocks = [(0, 512), (512, 512), (1024, 512), (1536, 512), (2048, 64)]
            for g in range(4):
                for (c0, cw) in blocks:
                    psx = psp.tile([128, 512], F32, tag="psx", space="PSUM")
                    for k in range(2):
                        nc.tensor.matmul(out=psx[:, :cw],
                                         lhsT=wih_s[:, k * 512 + g * 128: k * 512 + (g + 1) * 128],
                                         rhs=XT[:, k * NC1 + c0: k * NC1 + c0 + cw],
                                         start=(k == 0), stop=(k == 1))
                    nc.vector.tensor_tensor(out=xpT[:, g * NC1 + c0: g * NC1 + c0 + cw],
                                            in0=psx[:, :cw],
                                            in1=cpk_s[:, g:g + 1].to_broadcast([128, cw]),
                                            op=OP.add)
            # recurrent weights fp32
            whh_s16 = pp.tile([128, 4 * CH], F16)
            nc.sync.dma_start(whh_s16[:], whh16[:])
            whh_s = pp.tile([128, 4 * CH], F32)
            nc.vector.tensor_copy(out=whh_s[:], in_=whh_s16[:])
            # scan
            hh = pp.tile([128, (S1 + 1) * LC], F32)
            cst = pp.tile([128, LC], F32)
            nc.vector.memset(hh[:, 0:LC], 0.0)
            nc.vector.memset(cst[:], 0.0)
            for t in range(S1):
                gps = psp.tile([128, 4 * LC], F32, tag="g", space="PSUM")
                for g in range(4):
                    nc.tensor.matmul(out=gps[:, g * LC:(g + 1) * LC],
                                     lhsT=whh_s[:, g * 128:(g + 1) * 128],
                                     rhs=hh[:, t * LC:(t + 1) * LC],
                                     start=(g == 0), stop=(g == 3))
                G = tp.tile([128, 4 * LC], F32, tag="G")
                nc.vector.tensor_tensor(
                    out=_ap(G[:], [[LC, 4], [1, LC]]),
                    in0=_ap(gps[:], [[LC, 4], [1, LC]]),
                    in1=_ap(xpT[:], [[NC1, 4], [LEN1, LC]], extra_off=t),
                    op=OP.add)
                Ssig = tp.tile([128, 3 * LC], F32, tag="S")
                nc.scalar.activation(out=Ssig[:], in_=G[:, 0:3 * LC], func=AF.Sigmoid)
                Tg = tp.tile([128, LC], F32, tag="Tg")
                nc.scalar.activation(out=Tg[:], in_=G[:, 3 * LC:4 * LC], func=AF.Tanh)
                t1 = tp.tile([128, LC], F32, tag="t1")
                nc.vector.tensor_tensor(out=t1[:], in0=Ssig[:, 0:LC], in1=Tg[:], op=OP.mult)
                nc.vector.tensor_tensor(out=cst[:], in0=Ssig[:, LC:2 * LC], in1=cst[:], op=OP.mult)
                nc.vector.tensor_tensor(out=cst[:], in0=cst[:], in1=t1[:], op=OP.add)
                Tc = tp.tile([128, LC], F32, tag="Tc")
                nc.scalar.activation(out=Tc[:], in_=cst[:], func=AF.Tanh)
                nc.vector.tensor_tensor(out=hh[:, (t + 1) * LC:(t + 2) * LC],
                                        in0=Ssig[:, 2 * LC:3 * LC], in1=Tc[:], op=OP.mult)
                if t == W1 - 1:
                    blk = hh[:, (t + 1) * LC:(t + 2) * LC]
                    nc.vector.tensor_tensor(out=blk, in0=blk, in1=mH[:], op=OP.mult)
                    nc.vector.tensor_tensor(out=blk, in0=blk, in1=fH[:], op=OP.add)
                    nc.vector.tensor_tensor(out=cst[:], in0=cst[:], in1=mH[:], op=OP.mult)
                    nc.vector.tensor_tensor(out=cst[:], in0=cst[:], in1=fC[:], op=OP.add)
            hc = pp.tile([128, LEN1 * LC], F16)
            nc.vector.tensor_copy(out=hc[:], in_=hh[:, (W1 + 1) * LC:(S1 + 1) * LC])
            nc.sync.dma_start(hout[:], hc[:])
    nc.compile()
    return nc


# ---------------------------------------------------------------- L2: word
def build_l2():
    nc = _new_nc(8)
    # per-direction weight pack: 16 chunks of [128, 4*WH]:
    #   chunks 0-7  = wihT for word-embedding input dims (WD = 8*128)
    #   chunks 8-11 = wihT for char-feat input dims (512 = 4*128)
    #   chunks 12-15 = whhT (WH = 4*128)
    # each core ships its direction-group-rank's 4 consecutive chunks.
    embT = nc.dram_tensor("embT", [128, 8 * WIN], F16, kind="ExternalInput")
    cfT = nc.dram_tensor("cfT", [128, 4 * WIN], F16, kind="ExternalInput")
    wsh = nc.dram_tensor("wsh", [128, 4 * 4 * WH], F16, kind="ExternalInput")
    cpk = nc.dram_tensor("cpk", [128, 16 + 3 * 4 * LW + 24 + 6], F32, kind="ExternalInput")
    fpart = nc.dram_tensor("fpart", [512, 6], F32, kind="ExternalOutput")

    with tile.TileContext(nc) as tc:
        with tc.tile_pool(name="p", bufs=1) as pp, \
             tc.tile_pool(name="ps", bufs=2, space="PSUM") as psp, \
             tc.tile_pool(name="d", bufs=1, space="DRAM") as dp, \
             tc.tile_pool(name="tmp", bufs=2) as tp:
            # allgather the weight pack within each direction group
            bounce = dp.tile([128, 4 * 4 * WH], F16)
            gath = dp.tile([4 * 128 * 4 * 4 * WH], F16)
            nc.gpsimd.dma_start(bounce[:], wsh[:])
            nc.gpsimd.collective_compute(
                "AllGather", OP.bypass,
                replica_groups=[[0, 1, 2, 3], [4, 5, 6, 7]],
                ins=[bounce[:].opt()], outs=[gath[:].opt()])
            cpk_s = pp.tile([128, 16 + 3 * 4 * LW + 24 + 6], F32)
            nc.sync.dma_start(cpk_s[:], cpk[:])
            mH = cpk_s[:, 16:16 + 128]
            fH = cpk_s[:, 144:144 + 128]
            fC = cpk_s[:, 272:272 + 128]
            xpT = pp.tile([128, 16 * WIN], F32)
            whh_s = pp.tile([128, 4 * 4 * WH], F32)
            with tc.tile_pool(name="wih", bufs=1) as wp:
                embT_s = wp.tile([128, 8 * WIN], F16)
                nc.sync.dma_start(embT_s[:], embT[:])
                cfT_s = wp.tile([128, 4 * WIN], F16)
                nc.sync.dma_start(cfT_s[:], cfT[:])
                wih_s = wp.tile([128, 12 * 4 * WH], F16)
                SH = 4 * 4 * WH  # 8192 cols per rank shard
                for r in range(3):
                    nc.sync.dma_start(
                        wih_s[:, r * SH:(r + 1) * SH],
                        _dap(gath[:], [[SH, 128], [1, SH]], extra_off=r * 128 * SH))
                whh_s16 = wp.tile([128, SH], F16)
                nc.sync.dma_start(
                    whh_s16[:],
                    _dap(gath[:], [[SH, 128], [1, SH]], extra_off=3 * 128 * SH))
                nc.vector.tensor_copy(out=whh_s[:], in_=whh_s16[:])
                for g in range(16):
                    for (c0, cw) in ((0, 288), (288, 288)):
                        psx = psp.tile([128, 288], F32, tag="psx", space="PSUM")
                        for k in range(8):
                            nc.tensor.matmul(out=psx[:, :cw],
                                             lhsT=wih_s[:, k * 2048 + g * 128: k * 2048 + (g + 1) * 128],
                                             rhs=embT_s[:, k * WIN + c0: k * WIN + c0 + cw],
                                             start=(k == 0), stop=False)
                        for k in range(4):
                            nc.tensor.matmul(out=psx[:, :cw],
                                             lhsT=wih_s[:, (8 + k) * 2048 + g * 128: (8 + k) * 2048 + (g + 1) * 128],
                                             rhs=cfT_s[:, k * WIN + c0: k * WIN + c0 + cw],
                                             start=False, stop=(k == 3))
                        nc.vector.tensor_tensor(out=xpT[:, g * WIN + c0: g * WIN + c0 + cw],
                                                in0=psx[:, :cw],
                                                in1=cpk_s[:, g:g + 1].to_broadcast([128, cw]),
                                                op=OP.add)
            # scan
            hh = pp.tile([128, (S2 + 1) * 4 * LW], F32)
            cst = pp.tile([128, 4 * LW], F32)
            nc.vector.memset(hh[:, 0:4 * LW], 0.0)
            nc.vector.memset(cst[:], 0.0)
            for t in range(S2):
                gps = psp.tile([128, 16 * LW], F32, tag="g", space="PSUM")
                for m in range(16):
                    for k in range(4):
                        nc.tensor.matmul(out=gps[:, m * LW:(m + 1) * LW],
                                         lhsT=whh_s[:, k * 2048 + m * 128: k * 2048 + (m + 1) * 128],
                                         rhs=hh[:, t * 4 * LW + k * LW: t * 4 * LW + (k + 1) * LW],
                                         start=(k == 0), stop=(k == 3))
                G = tp.tile([128, 16 * LW], F32, tag="G")
                nc.vector.tensor_tensor(
                    out=_ap(G[:], [[LW, 16], [1, LW]]),
                    in0=_ap(gps[:], [[LW, 16], [1, LW]]),
                    in1=_ap(xpT[:], [[WIN, 16], [LEN2, LW]], extra_off=t),
                    op=OP.add)
                Ssig = tp.tile([128, 12 * LW], F32, tag="S")
                nc.scalar.activation(out=Ssig[:], in_=G[:, 0:12 * LW], func=AF.Sigmoid)
                Tg = tp.tile([128, 4 * LW], F32, tag="Tg")
                nc.scalar.activation(out=Tg[:], in_=G[:, 12 * LW:16 * LW], func=AF.Tanh)
                t1 = tp.tile([128, 4 * LW], F32, tag="t1")
                nc.vector.tensor_tensor(out=t1[:], in0=Ssig[:, 0:4 * LW], in1=Tg[:], op=OP.mult)
                nc.vector.tensor_tensor(out=cst[:], in0=Ssig[:, 4 * LW:8 * LW], in1=cst[:], op=OP.mult)
                nc.vector.tensor_tensor(out=cst[:], in0=cst[:], in1=t1[:], op=OP.add)
                Tc = tp.tile([128, 4 * LW], F32, tag="Tc")
                nc.scalar.activation(out=Tc[:], in_=cst[:], func=AF.Tanh)
                nc.vector.tensor_tensor(out=hh[:, (t + 1) * 4 * LW:(t + 2) * 4 * LW],
                                        in0=Ssig[:, 8 * LW:12 * LW], in1=Tc[:], op=OP.mult)
                if t == W2 - 1:
                    blk = hh[:, (t + 1) * 4 * LW:(t + 2) * 4 * LW]
                    nc.vector.tensor_tensor(out=blk, in0=blk, in1=mH[:], op=OP.mult)
                    nc.vector.tensor_tensor(out=blk, in0=blk, in1=fH[:], op=OP.add)
                    nc.vector.tensor_tensor(out=cst[:], in0=cst[:], in1=mH[:], op=OP.mult)
                    nc.vector.tensor_tensor(out=cst[:], in0=cst[:], in1=fC[:], op=OP.add)
            # repack post-warmup h (t-major) then feats partial
            hT = pp.tile([128, 4 * 512], F32)
            for k in range(4):
                nc.vector.tensor_copy(
                    out=_ap(hT[:], [[16, 32], [1, 16]], extra_off=k * 512),
                    in_=_ap(hh[:], [[1, 32], [4 * LW, 16]],
                            extra_off=(W2 + 1) * 4 * LW + k * LW))
            fp_s = pp.tile([128, 4 * 6], F32)
            for m in range(4):
                psf = psp.tile([128, 6], F32, tag="psf", space="PSUM")
                for k in range(4):
                    nc.tensor.matmul(out=psf[:],
                                     lhsT=hT[:, k * 512 + m * 128: k * 512 + (m + 1) * 128],
                                     rhs=cpk_s[:, 400 + k * 6:400 + (k + 1) * 6],
                                     start=(k == 0), stop=(k == 3))
                nc.vector.tensor_tensor(out=fp_s[:, m * 6:(m + 1) * 6], in0=psf[:],
                                        in1=cpk_s[:, 424:430], op=OP.add)
            nc.sync.dma_start(fpart[:].rearrange("(m p) s -> p m s", p=128),
                              fp_s[:].rearrange("p (m s) -> p m s", m=4))
    nc.compile()
    return nc


# ---------------------------------------------------------------- L3: viterbi
def build_l3():
    nc = _new_nc(1)
    fstack = nc.dram_tensor("fstack", [8 * 512, 6], F32, kind="ExternalInput")
    # c16: transR(36) | iotaM(36) | maskV(6) | fillV(6) | tstop(6)
    c16 = nc.dram_tensor("c16", [16, 90], F32, kind="ExternalInput")
    # c96: iotaI(36) | iotaJ(768) | uinit(6) | bmask(16)
    c96 = nc.dram_tensor("c96", [96, 826], F32, kind="ExternalInput")
    ids_o = nc.dram_tensor("ids_o", [T], I32, kind="ExternalOutput")

    with tile.TileContext(nc) as tc:
        with tc.tile_pool(name="p", bufs=1) as pp, \
             tc.tile_pool(name="ps", bufs=2, space="PSUM") as psp, \
             tc.tile_pool(name="d", bufs=1, space="DRAM") as dp, \
             tc.tile_pool(name="tmp", bufs=2) as tp:
            # sum the 8 partial feats
            Ff = pp.tile([128, 16 * 6], F32)
            Fb = pp.tile([128, 16 * 6], F32)
            for k in range(4):
                nc.sync.dma_start(Ff[32 * k:32 * (k + 1), :],
                                  fstack[:].rearrange("(c p a) s -> c p a s", c=8, p=32)[k])
                nc.sync.dma_start(Fb[32 * k:32 * (k + 1), :],
                                  fstack[:].rearrange("(c p a) s -> c p a s", c=8, p=32)[4 + k])
            F = pp.tile([128, 16 * 6], F32)
            nc.vector.tensor_tensor(out=F[:], in0=Ff[:], in1=Fb[:], op=OP.add)
            featsD = dp.tile([T * 6], F32)
            nc.sync.dma_start(featsD[:].rearrange("(p a) -> p a", p=128), F[:])
            # stage per-sublane feats windows
            fsub = pp.tile([16, SV * 6], F32)
            fD = featsD[:]
            for p in range(16):
                if p == 0:
                    nc.sync.dma_start(fsub[0:1, 0:WV * 6], _dap(fD, [[WV * 6, 1], [1, WV * 6]]))
                    nc.sync.dma_start(fsub[0:1, WV * 6:SV * 6], _dap(fD, [[LV * 6, 1], [1, LV * 6]]))
                else:
                    nc.sync.dma_start(fsub[p:p + 1, :],
                                      _dap(fD, [[SV * 6, 1], [1, SV * 6]], extra_off=(p * LV - WV) * 6))
            c16_s = pp.tile([16, 90], F32)
            nc.sync.dma_start(c16_s[:], c16[:])
            fv = pp.tile([16, 6], F32)
            nc.vector.memset(fv[:], 0.0)
            bpsH = pp.tile([16, LV * 6], F32)
            for t in range(SV):
                if t == WV:
                    nc.vector.tensor_tensor(out=fv[:], in0=fv[:], in1=c16_s[:, 72:78], op=OP.mult)
                    nc.vector.tensor_tensor(out=fv[:], in0=fv[:], in1=c16_s[:, 78:84], op=OP.add)
                tmp = tp.tile([16, 36], F32, tag="tmp")
                nc.vector.tensor_tensor(out=_ap(tmp[:], [[6, 6], [1, 6]]),
                                        in0=_ap(c16_s[:], [[6, 6], [1, 6]]),
                                        in1=_ap(fv[:], [[0, 6], [1, 6]]), op=OP.add)
                mx = tp.tile([16, 6], F32, tag="mx")
                nc.vector.tensor_reduce(out=mx[:], in_=_ap(tmp[:], [[6, 6], [1, 6]]),
                                        axis=AX.X, op=OP.max)
                eq = tp.tile([16, 36], F32, tag="eq")
                nc.vector.tensor_tensor(out=_ap(eq[:], [[6, 6], [1, 6]]),
                                        in0=_ap(tmp[:], [[6, 6], [1, 6]]),
                                        in1=_ap(mx[:], [[1, 6], [0, 6]]), op=OP.is_ge)
                nc.vector.tensor_tensor(out=eq[:], in0=eq[:], in1=c16_s[:, 36:72], op=OP.mult)
                if t >= WV:
                    nc.vector.tensor_reduce(out=bpsH[:, (t - WV) * 6:(t - WV + 1) * 6],
                                            in_=_ap(eq[:], [[6, 6], [1, 6]]), axis=AX.X, op=OP.min)
                nc.vector.tensor_tensor(out=fv[:], in0=mx[:], in1=fsub[:, t * 6:(t + 1) * 6], op=OP.add)
            # last-tag onehot
            av = pp.tile([16, 6], F32)
            nc.vector.tensor_tensor(out=av[:], in0=fv[:], in1=c16_s[:, 84:90], op=OP.add)
            am = pp.tile([16, 1], F32)
            nc.vector.tensor_reduce(out=am[:], in_=av[:], axis=AX.X, op=OP.max)
            ohf = pp.tile([16, 6], F32)
            nc.vector.tensor_tensor(out=ohf[:], in0=av[:], in1=am[:].to_broadcast([16, 6]), op=OP.is_ge)
            # replicate bps to 96 partitions
            bpsD = dp.tile([16 * LV * 6], F32)
            nc.sync.dma_start(bpsD[:].rearrange("(p a) -> p a", p=16), bpsH[:])
            bpsR = pp.tile([96, LV * 6], F32)
            for e in range(6):
                nc.sync.dma_start(bpsR[16 * e:16 * (e + 1), :],
                                  bpsD[:].rearrange("(p a) -> p a", p=16))
            c96_s = pp.tile([96, 826], F32)
            nc.sync.dma_start(c96_s[:], c96[:])
            uH = pp.tile([96, (LV + 1) * 6], F32)
            nc.vector.tensor_copy(out=uH[:, LV * 6:(LV + 1) * 6], in_=c96_s[:, 804:810])
            for tb in range(LV - 1, -1, -1):
                eqB = tp.tile([96, 36], F32, tag="eqB")
                nc.vector.tensor_tensor(out=_ap(eqB[:], [[6, 6], [1, 6]]),
                                        in0=_ap(bpsR[:], [[0, 6], [1, 6]], extra_off=tb * 6),
                                        in1=_ap(c96_s[:], [[6, 6], [1, 6]]), op=OP.is_equal)
                tB = tp.tile([96, 36], F32, tag="tB")
                nc.vector.tensor_tensor(out=_ap(tB[:], [[6, 6], [1, 6]]),
                                        in0=_ap(eqB[:], [[6, 6], [1, 6]]),
                                        in1=_ap(uH[:], [[0, 6], [1, 6]], extra_off=(tb + 1) * 6),
                                        op=OP.mult)
                nc.vector.tensor_reduce(out=uH[:, tb * 6:(tb + 1) * 6],
                                        in_=_ap(tB[:], [[6, 6], [1, 6]]), axis=AX.X, op=OP.max)
            # decode ids for all hypotheses
            idsA = pp.tile([96, LV], F32)
            tJ = pp.tile([96, 768], F32)
            nc.vector.tensor_tensor(out=tJ[:], in0=uH[:, 6:(LV + 1) * 6], in1=c96_s[:, 36:804], op=OP.mult)
            nc.vector.tensor_reduce(out=idsA[:], in_=_ap(tJ[:], [[6, LV], [1, 6]]), axis=AX.X, op=OP.max)
            # chunk maps flattened onto ONE partition: MT2 [1, 16*36] flat (c,j,e)
            uD = dp.tile([96 * 6], F32)
            nc.sync.dma_start(uD[:].rearrange("(p a) -> p a", p=96), uH[:, 0:6])
            MT2 = pp.tile([1, 16 * 36], F32)
            nc.sync.dma_start(MT2[:], _dap(uD[:], [[576, 1], [6, 16], [1, 6], [96, 6]]))
            # move last-tag onehot (row 15 of ohf) to partition 0
            ohfD = dp.tile([16 * 6], F32)
            nc.sync.dma_start(ohfD[:].rearrange("(p a) -> p a", p=16), ohf[:])
            # stitch on partition 0: ohSeq[:, c*6+e] = onehot(ids at end of chunk c)
            ohSeq = pp.tile([1, 16 * 6], F32)
            nc.sync.dma_start(ohSeq[0:1, 15 * 6:16 * 6],
                              _dap(ohfD[:], [[6, 1], [1, 6]], extra_off=15 * 6))
            for c in range(14, -1, -1):
                tS2 = tp.tile([1, 36], F32, tag="tS2")
                nc.vector.tensor_tensor(out=_ap(tS2[:], [[6, 6], [1, 6]]),
                                        in0=_ap(MT2[:], [[6, 6], [1, 6]], extra_off=(c + 1) * 36),
                                        in1=_ap(ohSeq[:], [[0, 6], [1, 6]], extra_off=(c + 1) * 6),
                                        op=OP.mult)
                nc.vector.tensor_reduce(out=ohSeq[0:1, c * 6:(c + 1) * 6],
                                        in_=_ap(tS2[:], [[6, 6], [1, 6]]), axis=AX.X, op=OP.max)
            ohD = dp.tile([16 * 6], F32)
            nc.sync.dma_start(ohD[:].rearrange("(p a) -> p a", p=1), ohSeq[:])
            selC = pp.tile([96, 1], F32)
            for e in range(6):
                nc.sync.dma_start(selC[16 * e:16 * (e + 1), :],
                                  _dap(ohD[:], [[6, 16], [1, 1]], extra_off=e))
            SEL = pp.tile([96, 16], F32)
            nc.vector.tensor_tensor(out=SEL[:], in0=selC[:].to_broadcast([96, 16]), in1=c96_s[:, 810:826], op=OP.mult)
            psi = psp.tile([16, LV], F32, tag="psi", space="PSUM")
            nc.tensor.matmul(out=psi[:], lhsT=SEL[:], rhs=idsA[:], start=True, stop=True)
            idsI = pp.tile([16, LV], I32)
            nc.vector.tensor_copy(out=idsI[:], in_=psi[:])
            nc.sync.dma_start(ids_o[:].rearrange("(p a) -> p a", p=16), idsI[:])
    nc.compile()
    return nc


# ---------------------------------------------------------------- host glue
_cache = {}


def _programs():
    if "l1" not in _cache:
        _cache["l1"] = build_l1()
        _cache["l2"] = build_l2()
        _cache["l3"] = build_l3()
    return _cache["l1"], _cache["l2"], _cache["l3"]


def _kernel_3launch(**inp):
    inp = {k: np.asarray(v) for k, v in inp.items()}
    nc1, nc2, nc3 = _programs()
    perf = {}

    chars = inp["chars"].astype(np.int32)
    words = inp["words"].astype(np.int32)
    ix = inp["ix_seq"].astype(np.int64)
    cemb = inp["char_embed"].astype(np.float32)
    wemb = inp["word_embed"].astype(np.float32)

    # ---------------- L1 inputs
    l1_dir = []
    for d, suf in ((0, "f"), (1, "b")):
        Wih = _reorder(inp[f"c_Wih_{suf}"], CH)
        Whh = _reorder(inp[f"c_Whh_{suf}"], CH)
        bias = _reorder(inp[f"c_bih_{suf}"] + inp[f"c_bhh_{suf}"], CH)
        l1_dir.append({
            "wih16": _chunkT(Wih.T.astype(np.float32), 2).astype(np.float16),
            "whh16": np.ascontiguousarray(Whh.T).astype(np.float16),
            "bias": np.ascontiguousarray(bias.reshape(4, 128).T.astype(np.float32)),
            "src": chars if d == 0 else chars[::-1],
        })
    in_maps1 = []
    for core in range(8):
        d, kk = core // 4, core % 4
        dd = l1_dir[d]
        pos = (2048 * kk - W1 + np.arange(NC1)).clip(0, C - 1)
        Xc = cemb[dd["src"][pos]]                      # [NC1, 256]
        xt16 = np.ascontiguousarray(
            Xc.T.reshape(2, 128, NC1).transpose(1, 0, 2).reshape(128, 2 * NC1)
        ).astype(np.float16)
        cpk1 = np.zeros((128, 4 + 3 * LC), np.float32)
        cpk1[:, 0:4] = dd["bias"]
        maskH = np.ones((128, LC), np.float32)
        fillH = np.zeros((128, LC), np.float32)
        fillC = np.zeros((128, LC), np.float32)
        if kk == 0:
            maskH[:, 0] = 0.0
            fillH[:, 0] = inp["c_h0"][d]
            fillC[:, 0] = inp["c_c0"][d]
        cpk1[:, 4:4 + LC] = maskH
        cpk1[:, 4 + LC:4 + 2 * LC] = fillH
        cpk1[:, 4 + 2 * LC:4 + 3 * LC] = fillC
        in_maps1.append({
            "xt16": xt16,
            "wih16": dd["wih16"],
            "whh16": dd["whh16"],
            "cpk": cpk1,
        })
    t0 = _time.time()
    r1 = run_bass_kernel_spmd(nc1, in_maps1, core_ids=list(range(8)),
                              trace=False, tmpdir=None)
    perf["l1_wall"] = _time.time() - t0
    # reassemble char hids: hout col = j*LC + l -> h at pos 2048*kk + 64*l + j
    chf = np.zeros((C, CH), np.float32)
    chb = np.zeros((C, CH), np.float32)
    for core in range(8):
        h = r1.results[core]["hout"].astype(np.float32)  # [128, LEN1*LC]
        d, kk = core // 4, core % 4
        hv = h.reshape(CH, LEN1, LC)  # [hid, j, l]
        pos = LEN1 * (LC * kk + np.arange(LC))[None, :] + np.arange(LEN1)[:, None]
        if d == 0:
            chf[pos.reshape(-1)] = hv.reshape(CH, -1).T
        else:
            chb[C - 1 - pos.reshape(-1)] = hv.reshape(CH, -1).T
    starts, ends = ix[:-1], ix[1:] - 1
    char_feats = np.concatenate(
        [chf[starts], chb[starts], chf[ends], chb[ends]], axis=1)  # [T, 512]

    # ---------------- L2 inputs
    CT16 = cemb[chars].astype(np.float16)          # [8192, 256]
    WT16 = wemb[words].astype(np.float16)          # [2048, 1024]
    l2_dir = []
    for d, suf in ((0, "f"), (1, "b")):
        Wih = _reorder(inp[f"w_Wih_{suf}"], WH)
        Whh = _reorder(inp[f"w_Whh_{suf}"], WH)
        bias = _reorder(inp[f"w_bih_{suf}"] + inp[f"w_bhh_{suf}"], WH)
        # 16-chunk pack: wihTwe (8) | wihTcf (4) | whhT (4), each [128, 2048]
        P = np.concatenate([
            _chunkT(np.ascontiguousarray(Wih[:, 512:].T), 8),
            _chunkT(np.ascontiguousarray(Wih[:, :512].T), 4),
            _chunkT(np.ascontiguousarray(Whh.T), 4),
        ], axis=1).astype(np.float16)            # [128, 16*2048]
        l2_dir.append({
            "P": P,
            "bias": np.ascontiguousarray(bias.reshape(16, 128).T.astype(np.float32)),
            "cf": char_feats if d == 0 else char_feats[::-1],
            "wsrc": words if d == 0 else words[::-1],
            "h2t": inp["hid2tag_W"][:, :WH] if d == 0 else inp["hid2tag_W"][:, WH:],
        })
    in_maps2 = []
    for core in range(8):
        d, kk = core // 4, core % 4
        dd = l2_dir[d]
        rows = (512 * kk - W2 + np.arange(WIN)).clip(0, T - 1)
        We = wemb[dd["wsrc"][rows]]                    # [WIN, 1024]
        embT16 = np.ascontiguousarray(
            We.T.reshape(8, 128, WIN).transpose(1, 0, 2).reshape(128, 8 * WIN)
        ).astype(np.float16)
        cfr = dd["cf"][rows]                           # [WIN, 512]
        cfT16 = np.ascontiguousarray(
            cfr.T.reshape(4, 128, WIN).transpose(1, 0, 2).reshape(128, 4 * WIN)
        ).astype(np.float16)
        cpk2 = np.zeros((128, 16 + 3 * 4 * LW + 24 + 6), np.float32)
        cpk2[:, 0:16] = dd["bias"]
        maskH = np.ones((128, 4 * LW), np.float32)
        fillH = np.zeros((128, 4 * LW), np.float32)
        fillC = np.zeros((128, 4 * LW), np.float32)
        if kk == 0:
            for k in range(4):
                maskH[:, k * LW] = 0.0
                fillH[:, k * LW] = inp["w_h0"][d][k * 128:(k + 1) * 128]
                fillC[:, k * LW] = inp["w_c0"][d][k * 128:(k + 1) * 128]
        cpk2[:, 16:144] = maskH
        cpk2[:, 144:272] = fillH
        cpk2[:, 272:400] = fillC
        cpk2[:, 400:424] = _chunkT(np.ascontiguousarray(dd["h2t"].T), 4)
        if d == 0:
            cpk2[:, 424:430] = inp["hid2tag_b"][None, :]
        in_maps2.append({
            "embT": embT16,
            "cfT": cfT16,
            "wsh": np.ascontiguousarray(dd["P"][:, kk * 8192:(kk + 1) * 8192]),
            "cpk": cpk2,
        })
    t0 = _time.time()
    r2 = run_bass_kernel_spmd(nc2, in_maps2, core_ids=list(range(8)),
                              trace=False, tmpdir=None)
    perf["l2_wall"] = _time.time() - t0
    fstack = np.zeros((8 * 512, 6), np.float32)
    for core in range(8):
        fp = r2.results[core]["fpart"]  # [512, 6] for global t block 512*kk
        d, kk = core // 4, core % 4
        if d == 0:
            fstack[512 * core:512 * (core + 1)] = fp
        else:
            fstack[512 * core:512 * (core + 1)] = fp[::-1]
    # bwd partials: core (4+kk) block covers reversed rows [512kk:512kk+512]
    # -> global t = T-1 - rev_t, i.e. global block [T-512(kk+1), T-512kk) reversed.
    bsec = fstack[4 * 512:].copy()
    fstack[4 * 512:] = 0
    for kk in range(4):
        blk = bsec[512 * kk:512 * (kk + 1)]
        g0 = T - 512 * (kk + 1)
        fstack[4 * 512 + g0:4 * 512 + g0 + 512] = blk

    # ---------------- L3 inputs
    trans = inp["transition"].astype(np.float32)
    c16 = np.zeros((16, 90), np.float32)
    c16[:, 0:36] = trans.reshape(1, 36)
    c16[:, 36:72] = (np.arange(36) % 6 - 6).astype(np.float32)[None, :]
    maskV = np.ones((16, 6), np.float32)
    maskV[0] = 0.0
    fillV = np.zeros((16, 6), np.float32)
    fv0 = np.full(6, NEG, np.float32)
    fv0[4] = 0.0
    fillV[0] = fv0
    c16[:, 72:78] = maskV
    c16[:, 78:84] = fillV
    c16[:, 84:90] = trans[:, 5][None, :]
    c96 = np.zeros((96, 826), np.float32)
    c96[:, 0:36] = (np.arange(36) // 6 - 6).astype(np.float32)[None, :]
    c96[:, 36:804] = (np.arange(768) % 6).astype(np.float32)[None, :]
    uinit = np.zeros((96, 6), np.float32)
    for e in range(6):
        uinit[16 * e:16 * (e + 1), e] = 1.0
    c96[:, 804:810] = uinit
    bmask = np.zeros((96, 16), np.float32)
    for e in range(6):
        for c in range(16):
            bmask[16 * e + c, c] = 1.0
    c96[:, 810:826] = bmask
    in_map3 = {"fstack": fstack, "c16": c16, "c96": c96}
    t0 = _time.time()
    r3 = run_bass_kernel_spmd(nc3, [in_map3], core_ids=[0],
                              trace=False, tmpdir=None)
    perf["l3_wall"] = _time.time() - t0
    _kernel_3launch.last_perf = perf
    _kernel_3launch.last_fstack = fstack
    return r3.results[0]["ids_o"].astype(np.int32)


_kernel_3launch.last_perf = {}


# ---------------------------------------------------------------- fused
# packed input layouts (cols)
OWSH = 0                         # word-weight shard (8192) + char-weight shard (384)
NSH = 8576                       # wsh cols per core
OCT, OWT, OC96 = 8576, 10624, 12672
NA = 13504                       # bigA f16 cols (13498 used)
OCPK1, OCPK2, OC16, OREV, OSEL = 0, 100, 530, 620, 748
NC32 = 752                       # cst f32 cols (750 used)
NIDX = 4 * 640 + 17 * 128 + 5 * 128   # cf groups | char window | word window


def build_fused():
    nc = _new_nc(8)
    bigA = nc.dram_tensor("bigA", [128, NA], F16, kind="ExternalInput")
    cst = nc.dram_tensor("cst", [128, NC32], F32, kind="ExternalInput")
    idx = nc.dram_tensor("idx", [NIDX, 1], I32, kind="ExternalInput")
    ids_o = nc.dram_tensor("ids_o", [T], I32, kind="ExternalOutput")

    with tile.TileContext(nc) as tc:
        with tc.tile_pool(name="pp", bufs=1) as pp, \
             tc.tile_pool(name="dp", bufs=1, space="DRAM") as dp, \
             tc.tile_pool(name="tp", bufs=2) as tp:
            # ---- kick off the table/weight allgathers early (input-only deps)
            bounce_ct = dp.tile([128, 2048], F16)
            gathCT = dp.tile([8 * 128 * 2048], F16)
            nc.gpsimd.dma_start(bounce_ct[:],
                                _dap(bigA[:], [[NA, 128], [1, 2048]], extra_off=OCT))
            nc.gpsimd.collective_compute(
                "AllGather", OP.bypass,
                replica_groups=[list(range(8))],
                ins=[bounce_ct[:].opt()], outs=[gathCT[:].opt()])
            bounce_wt = dp.tile([128, 2048], F16)
            gathWT = dp.tile([8 * 128 * 2048], F16)
            nc.gpsimd.dma_start(bounce_wt[:],
                                _dap(bigA[:], [[NA, 128], [1, 2048]], extra_off=OWT))
            nc.gpsimd.collective_compute(
                "AllGather", OP.bypass,
                replica_groups=[list(range(8))],
                ins=[bounce_wt[:].opt()], outs=[gathWT[:].opt()])
            bounce_w = dp.tile([128, NSH], F16)
            gathW = dp.tile([4 * 128 * NSH], F16)
            nc.gpsimd.dma_start(bounce_w[:],
                                _dap(bigA[:], [[NA, 128], [1, NSH]], extra_off=OWSH))
            nc.gpsimd.collective_compute(
                "AllGather", OP.bypass,
                replica_groups=[[0, 1, 2, 3], [4, 5, 6, 7]],
                ins=[bounce_w[:].opt()], outs=[gathW[:].opt()])
            ident = pp.tile([128, 128], F32)
            make_identity(nc, ident[:])
            cfT_s = pp.tile([128, 4 * WIN], F16)
            sel_s = pp.tile([128, 2], F32)
            nc.sync.dma_start(sel_s[:], _dap(cst[:], [[NC32, 128], [1, 2]], extra_off=OSEL))
            idxs = pp.tile([128, NIDX // 128], I32)
            nc.sync.dma_start(idxs[:].rearrange("p (b o) -> p b o", b=NIDX // 128),
                              idx[:].rearrange("(b p) o -> p b o", p=128))
            bounce_h = dp.tile([LEN1 * LC, 128], F16)
            # ================= phase 1: char LSTM =================
            with tc.tile_pool(name="p1", bufs=1) as p1, \
                 tc.tile_pool(name="ps1", bufs=2, space="PSUM") as psp:
                XT = p1.tile([128, 2 * NC1], F16)
                for b in range(17):
                    cw = 128 if b < 16 else NC1 - 2048
                    g16 = tp.tile([128, 256], F16, tag="gct")
                    nc.gpsimd.indirect_dma_start(
                        out=g16[:], out_offset=None,
                        in_=_dap(gathCT[:], [[256, 8192], [1, 256]]),
                        in_offset=bass.IndirectOffsetOnAxis(
                            ap=idxs[:, 20 + b:21 + b], axis=0))
                    g32 = tp.tile([128, 256], F32, tag="gct32")
                    nc.vector.tensor_copy(out=g32[:], in_=g16[:])
                    for d2 in range(2):
                        pst = psp.tile([128, 128], F32, tag="trct", space="PSUM")
                        nc.tensor.transpose(out=pst[:], in_=g32[:, d2 * 128:(d2 + 1) * 128],
                                            identity=ident[:])
                        nc.vector.tensor_copy(
                            out=XT[:, d2 * NC1 + b * 128: d2 * NC1 + b * 128 + cw],
                            in_=pst[:, :cw])
                cw_s = p1.tile([128, 1536], F16)
                for r in range(4):
                    nc.sync.dma_start(
                        cw_s[:, r * 384:(r + 1) * 384],
                        _dap(gathW[:], [[NSH, 128], [1, 384]],
                             extra_off=r * 128 * NSH + 8192))
                wih_s = cw_s[:, 0:1024]
                cpk_s = p1.tile([128, 4 + 3 * LC], F32)
                nc.sync.dma_start(cpk_s[:], _dap(cst[:], [[NC32, 128], [1, 100]], extra_off=OCPK1))
                mH = cpk_s[:, 4:4 + LC]
                fH = cpk_s[:, 4 + LC:4 + 2 * LC]
                fC = cpk_s[:, 4 + 2 * LC:4 + 3 * LC]
                xpT = p1.tile([128, 4 * NC1], F32)
                blocks = [(0, 512), (512, 512), (1024, 512), (1536, 512), (2048, 64)]
                for g in range(4):
                    for (c0, cw) in blocks:
                        psx = psp.tile([128, 512], F32, tag="psx", space="PSUM")
                        for k in range(2):
                            nc.tensor.matmul(out=psx[:, :cw],
                                             lhsT=wih_s[:, k * 512 + g * 128: k * 512 + (g + 1) * 128],
                                             rhs=XT[:, k * NC1 + c0: k * NC1 + c0 + cw],
                                             start=(k == 0), stop=(k == 1))
                        nc.vector.tensor_tensor(out=xpT[:, g * NC1 + c0: g * NC1 + c0 + cw],
                                                in0=psx[:, :cw],
                                                in1=cpk_s[:, g:g + 1].to_broadcast([128, cw]),
                                                op=OP.add)
                whh_s = p1.tile([128, 4 * CH], F32)
                nc.vector.tensor_copy(out=whh_s[:], in_=cw_s[:, 1024:1536])
                hh = p1.tile([128, (S1 + 1) * LC], F32)
                cst1 = p1.tile([128, LC], F32)
                nc.vector.memset(hh[:, 0:LC], 0.0)
                nc.vector.memset(cst1[:], 0.0)
                for t in range(S1):
                    gps = psp.tile([128, 4 * LC], F32, tag="g", space="PSUM")
                    for g in range(4):
                        nc.tensor.matmul(out=gps[:, g * LC:(g + 1) * LC],
                                         lhsT=whh_s[:, g * 128:(g + 1) * 128],
                                         rhs=hh[:, t * LC:(t + 1) * LC],
                                         start=(g == 0), stop=(g == 3))
                    G = tp.tile([128, 4 * LC], F32, tag="G")
                    nc.vector.tensor_tensor(
                        out=_ap(G[:], [[LC, 4], [1, LC]]),
                        in0=_ap(gps[:], [[LC, 4], [1, LC]]),
                        in1=_ap(xpT[:], [[NC1, 4], [LEN1, LC]], extra_off=t),
                        op=OP.add)
                    Ssig = tp.tile([128, 3 * LC], F32, tag="S")
                    nc.scalar.activation(out=Ssig[:], in_=G[:, 0:3 * LC], func=AF.Sigmoid)
                    Tg = tp.tile([128, LC], F32, tag="Tg")
                    nc.scalar.activation(out=Tg[:], in_=G[:, 3 * LC:4 * LC], func=AF.Tanh)
                    t1 = tp.tile([128, LC], F32, tag="t1")
                    nc.vector.tensor_tensor(out=t1[:], in0=Ssig[:, 0:LC], in1=Tg[:], op=OP.mult)
                    nc.vector.tensor_tensor(out=cst1[:], in0=Ssig[:, LC:2 * LC], in1=cst1[:], op=OP.mult)
                    nc.vector.tensor_tensor(out=cst1[:], in0=cst1[:], in1=t1[:], op=OP.add)
                    Tc = tp.tile([128, LC], F32, tag="Tc")
                    nc.scalar.activation(out=Tc[:], in_=cst1[:], func=AF.Tanh)
                    nc.vector.tensor_tensor(out=hh[:, (t + 1) * LC:(t + 2) * LC],
                                            in0=Ssig[:, 2 * LC:3 * LC], in1=Tc[:], op=OP.mult)
                    if t == W1 - 1:
                        blk = hh[:, (t + 1) * LC:(t + 2) * LC]
                        nc.vector.tensor_tensor(out=blk, in0=blk, in1=mH[:], op=OP.mult)
                        nc.vector.tensor_tensor(out=blk, in0=blk, in1=fH[:], op=OP.add)
                        nc.vector.tensor_tensor(out=cst1[:], in0=cst1[:], in1=mH[:], op=OP.mult)
                        nc.vector.tensor_tensor(out=cst1[:], in0=cst1[:], in1=fC[:], op=OP.add)
                # transpose post-warmup h to row-major f16 and stage to DRAM
                for b in range(16):
                    pst = psp.tile([128, 128], F32, tag="tr", space="PSUM")
                    nc.tensor.transpose(out=pst[:],
                                        in_=hh[:, (W1 + 1) * LC + b * 128:(W1 + 1) * LC + (b + 1) * 128],
                                        identity=ident[:])
                    hr = tp.tile([128, 128], F16, tag="hr")
                    nc.vector.tensor_copy(out=hr[:], in_=pst[:])
                    nc.sync.dma_start(
                        _dap(bounce_h[:], [[128, 128], [1, 128]], extra_off=b * 128 * 128),
                        hr[:])
            # allgather char hiddens (8 cores): gathH[rank, r, hid]
            gathH = dp.tile([8 * LEN1 * LC * 128], F16)
            nc.gpsimd.collective_compute(
                "AllGather", OP.bypass,
                replica_groups=[list(range(8))],
                ins=[bounce_h[:].opt()], outs=[gathH[:].opt()])
            # ================= phase 2: build cfT from gathered char hiddens
            with tc.tile_pool(name="p2", bufs=2) as p2, \
                 tc.tile_pool(name="ps2", bufs=2, space="PSUM") as psp:
                for grp in range(4):
                    for b in range(5):
                        cw = 128 if b < 4 else WIN - 512
                        g16 = p2.tile([128, 128], F16, tag="g16")
                        nc.gpsimd.indirect_dma_start(
                            out=g16[:], out_offset=None,
                            in_=_dap(gathH[:], [[128, 8 * LEN1 * LC], [1, 128]]),
                            in_offset=bass.IndirectOffsetOnAxis(
                                ap=idxs[:, grp * 5 + b: grp * 5 + b + 1], axis=0))
                        g32 = p2.tile([128, 128], F32, tag="g32")
                        nc.vector.tensor_copy(out=g32[:], in_=g16[:])
                        pst = psp.tile([128, 128], F32, tag="tr2", space="PSUM")
                        nc.tensor.transpose(out=pst[:], in_=g32[:], identity=ident[:])
                        nc.vector.tensor_copy(
                            out=cfT_s[:, grp * WIN + b * 128: grp * WIN + b * 128 + cw],
                            in_=pst[:, :cw])
            # ================= phase 3: word LSTM =================
            xpT2 = pp.tile([128, 16 * WIN], F32)
            whh2_s = pp.tile([128, 4 * 4 * WH], F32)
            cpk2_s = pp.tile([128, 430], F32)
            nc.sync.dma_start(cpk2_s[:], _dap(cst[:], [[NC32, 128], [1, 430]], extra_off=OCPK2))
            m2H = cpk2_s[:, 16:16 + 128]
            f2H = cpk2_s[:, 144:144 + 128]
            f2C = cpk2_s[:, 272:272 + 128]
            with tc.tile_pool(name="p3", bufs=1) as p3, \
                 tc.tile_pool(name="ps3", bufs=2, space="PSUM") as psp:
                embT_s = p3.tile([128, 8 * WIN], F16)
                for b in range(5):
                    cw = 128 if b < 4 else WIN - 512
                    g16 = tp.tile([128, 1024], F16, tag="gwt")
                    nc.gpsimd.indirect_dma_start(
                        out=g16[:], out_offset=None,
                        in_=_dap(gathWT[:], [[1024, 2048], [1, 1024]]),
                        in_offset=bass.IndirectOffsetOnAxis(
                            ap=idxs[:, 37 + b:38 + b], axis=0))
                    g32 = tp.tile([128, 1024], F32, tag="gwt32")
                    nc.vector.tensor_copy(out=g32[:], in_=g16[:])
                    for d2 in range(8):
                        pst = psp.tile([128, 128], F32, tag="trwt", space="PSUM")
                        nc.tensor.transpose(out=pst[:], in_=g32[:, d2 * 128:(d2 + 1) * 128],
                                            identity=ident[:])
                        nc.vector.tensor_copy(
                            out=embT_s[:, d2 * WIN + b * 128: d2 * WIN + b * 128 + cw],
                            in_=pst[:, :cw])
                wih2_s = p3.tile([128, 12 * 2048], F16)
                SH = 16 * WH
                for r in range(3):
                    nc.sync.dma_start(
                        wih2_s[:, r * SH:(r + 1) * SH],
                        _dap(gathW[:], [[NSH, 128], [1, SH]], extra_off=r * 128 * NSH))
                whh2_s16 = p3.tile([128, SH], F16)
                nc.sync.dma_start(
                    whh2_s16[:],
                    _dap(gathW[:], [[NSH, 128], [1, SH]], extra_off=3 * 128 * NSH))
                nc.vector.tensor_copy(out=whh2_s[:], in_=whh2_s16[:])
                for g in range(16):
                    for (c0, cw) in ((0, 288), (288, 288)):
                        psx = psp.tile([128, 288], F32, tag="psx2", space="PSUM")
                        for k in range(8):
                            nc.tensor.matmul(out=psx[:, :cw],
                                             lhsT=wih2_s[:, k * 2048 + g * 128: k * 2048 + (g + 1) * 128],
                                             rhs=embT_s[:, k * WIN + c0: k * WIN + c0 + cw],
                                             start=(k == 0), stop=False)
                        for k in range(4):
                            nc.tensor.matmul(out=psx[:, :cw],
                                             lhsT=wih2_s[:, (8 + k) * 2048 + g * 128: (8 + k) * 2048 + (g + 1) * 128],
                                             rhs=cfT_s[:, k * WIN + c0: k * WIN + c0 + cw],
                                             start=False, stop=(k == 3))
                        nc.vector.tensor_tensor(out=xpT2[:, g * WIN + c0: g * WIN + c0 + cw],
                                                in0=psx[:, :cw],
                                                in1=cpk2_s[:, g:g + 1].to_broadcast([128, cw]),
                                                op=OP.add)
            bounce_f = dp.tile([512, 6], F32)
            with tc.tile_pool(name="p4", bufs=1) as p4, \
                 tc.tile_pool(name="ps4", bufs=2, space="PSUM") as psp:
                hh2 = p4.tile([128, (S2 + 1) * 4 * LW], F32)
                cst2 = p4.tile([128, 4 * LW], F32)
                nc.vector.memset(hh2[:, 0:4 * LW], 0.0)
                nc.vector.memset(cst2[:], 0.0)
                for t in range(S2):
                    gps = psp.tile([128, 16 * LW], F32, tag="g2", space="PSUM")
                    for m in range(16):
                        for k in range(4):
                            nc.tensor.matmul(out=gps[:, m * LW:(m + 1) * LW],
                                             lhsT=whh2_s[:, k * 2048 + m * 128: k * 2048 + (m + 1) * 128],
                                             rhs=hh2[:, t * 4 * LW + k * LW: t * 4 * LW + (k + 1) * LW],
                                             start=(k == 0), stop=(k == 3))
                    G = tp.tile([128, 16 * LW], F32, tag="G2")
                    nc.vector.tensor_tensor(
                        out=_ap(G[:], [[LW, 16], [1, LW]]),
                        in0=_ap(gps[:], [[LW, 16], [1, LW]]),
                        in1=_ap(xpT2[:], [[WIN, 16], [LEN2, LW]], extra_off=t),
                        op=OP.add)
                    Ssig = tp.tile([128, 12 * LW], F32, tag="S2")
                    nc.scalar.activation(out=Ssig[:], in_=G[:, 0:12 * LW], func=AF.Sigmoid)
                    Tg = tp.tile([128, 4 * LW], F32, tag="Tg2")
                    nc.scalar.activation(out=Tg[:], in_=G[:, 12 * LW:16 * LW], func=AF.Tanh)
                    t1 = tp.tile([128, 4 * LW], F32, tag="t12")
                    nc.vector.tensor_tensor(out=t1[:], in0=Ssig[:, 0:4 * LW], in1=Tg[:], op=OP.mult)
                    nc.vector.tensor_tensor(out=cst2[:], in0=Ssig[:, 4 * LW:8 * LW], in1=cst2[:], op=OP.mult)
                    nc.vector.tensor_tensor(out=cst2[:], in0=cst2[:], in1=t1[:], op=OP.add)
                    Tc = tp.tile([128, 4 * LW], F32, tag="Tc2")
                    nc.scalar.activation(out=Tc[:], in_=cst2[:], func=AF.Tanh)
                    nc.vector.tensor_tensor(out=hh2[:, (t + 1) * 4 * LW:(t + 2) * 4 * LW],
                                            in0=Ssig[:, 8 * LW:12 * LW], in1=Tc[:], op=OP.mult)
                    if t == W2 - 1:
                        blk = hh2[:, (t + 1) * 4 * LW:(t + 2) * 4 * LW]
                        nc.vector.tensor_tensor(out=blk, in0=blk, in1=m2H[:], op=OP.mult)
                        nc.vector.tensor_tensor(out=blk, in0=blk, in1=f2H[:], op=OP.add)
                        nc.vector.tensor_tensor(out=cst2[:], in0=cst2[:], in1=m2H[:], op=OP.mult)
                        nc.vector.tensor_tensor(out=cst2[:], in0=cst2[:], in1=f2C[:], op=OP.add)
                # repack post-warmup h (t-major) then feats partial
                hT = p4.tile([128, 4 * 512], F32)
                for k in range(4):
                    nc.vector.tensor_copy(
                        out=_ap(hT[:], [[16, 32], [1, 16]], extra_off=k * 512),
                        in_=_ap(hh2[:], [[1, 32], [4 * LW, 16]],
                                extra_off=(W2 + 1) * 4 * LW + k * LW))
                fp_s = p4.tile([128, 4 * 6], F32)
                for m in range(4):
                    psf = psp.tile([128, 6], F32, tag="psf", space="PSUM")
                    for k in range(4):
                        nc.tensor.matmul(out=psf[:],
                                         lhsT=hT[:, k * 512 + m * 128: k * 512 + (m + 1) * 128],
                                         rhs=cpk2_s[:, 400 + k * 6:400 + (k + 1) * 6],
                                         start=(k == 0), stop=(k == 3))
                    nc.vector.tensor_tensor(out=fp_s[:, m * 6:(m + 1) * 6], in0=psf[:],
                                            in1=cpk2_s[:, 424:430], op=OP.add)
                # data-driven block reversal for bwd cores: psr = REV^T @ fp
                rev_s = p4.tile([128, 128], F32)
                nc.sync.dma_start(rev_s[:], _dap(cst[:], [[NC32, 128], [1, 128]], extra_off=OREV))
                psr = psp.tile([128, 24], F32, tag="psr", space="PSUM")
                nc.tensor.matmul(out=psr[:], lhsT=rev_s[:], rhs=fp_s[:], start=True, stop=True)
                tA = p4.tile([128, 24], F32)
                nc.vector.tensor_tensor(out=tA[:], in0=psr[:],
                                        in1=sel_s[:, 0:1].to_broadcast([128, 24]), op=OP.mult)
                fpB = p4.tile([128, 24], F32)
                for m in range(4):
                    nc.vector.tensor_copy(out=fpB[:, m * 6:(m + 1) * 6],
                                          in_=psr[:, (3 - m) * 6:(4 - m) * 6])
                nc.vector.tensor_tensor(out=fpB[:], in0=fpB[:],
                                        in1=sel_s[:, 1:2].to_broadcast([128, 24]), op=OP.mult)
                fpO = p4.tile([128, 24], F32)
                nc.vector.tensor_tensor(out=fpO[:], in0=tA[:], in1=fpB[:], op=OP.add)
                nc.sync.dma_start(bounce_f[:].rearrange("(m p) s -> p m s", p=128),
                                  fpO[:].rearrange("p (m s) -> p m s", m=4))
            gathF = dp.tile([8 * 512 * 6], F32)
            nc.gpsimd.collective_compute(
                "AllGather", OP.bypass,
                replica_groups=[list(range(8))],
                ins=[bounce_f[:].opt()], outs=[gathF[:].opt()])
            # ================= phase 5: viterbi (replicated on all cores)
            with tc.tile_pool(name="p5", bufs=1) as p5, \
                 tc.tile_pool(name="ps5", bufs=2, space="PSUM") as psp:
                Ff = p5.tile([128, 16 * 6], F32)
                Fb = p5.tile([128, 16 * 6], F32)
                for k in range(4):
                    nc.sync.dma_start(Ff[32 * k:32 * (k + 1), :],
                                      _dap(gathF[:], [[96, 32], [1, 96]], extra_off=k * 3072))
                    nc.sync.dma_start(Fb[32 * k:32 * (k + 1), :],
                                      _dap(gathF[:], [[96, 32], [1, 96]], extra_off=(7 - k) * 3072))
                F = p5.tile([128, 16 * 6], F32)
                nc.vector.tensor_tensor(out=F[:], in0=Ff[:], in1=Fb[:], op=OP.add)
                featsD = dp.tile([T * 6], F32)
                nc.sync.dma_start(featsD[:].rearrange("(p a) -> p a", p=128), F[:])
                fsub = p5.tile([16, SV * 6], F32)
                fD = featsD[:]
                for p in range(16):
                    if p == 0:
                        nc.sync.dma_start(fsub[0:1, 0:WV * 6], _dap(fD, [[WV * 6, 1], [1, WV * 6]]))
                        nc.sync.dma_start(fsub[0:1, WV * 6:SV * 6], _dap(fD, [[LV * 6, 1], [1, LV * 6]]))
                    else:
                        nc.sync.dma_start(fsub[p:p + 1, :],
                                          _dap(fD, [[SV * 6, 1], [1, SV * 6]], extra_off=(p * LV - WV) * 6))
                c16_s = p5.tile([16, 90], F32)
                nc.sync.dma_start(c16_s[:], _dap(cst[:], [[NC32, 16], [1, 90]], extra_off=OC16))
                fv = p5.tile([16, 6], F32)
                nc.vector.memset(fv[:], 0.0)
                bpsH = p5.tile([16, LV * 6], F32)
                for t in range(SV):
                    if t == WV:
                        nc.vector.tensor_tensor(out=fv[:], in0=fv[:], in1=c16_s[:, 72:78], op=OP.mult)
                        nc.vector.tensor_tensor(out=fv[:], in0=fv[:], in1=c16_s[:, 78:84], op=OP.add)
                    tmp = tp.tile([16, 36], F32, tag="tmp")
                    nc.vector.tensor_tensor(out=_ap(tmp[:], [[6, 6], [1, 6]]),
                                            in0=_ap(c16_s[:], [[6, 6], [1, 6]]),
                                            in1=_ap(fv[:], [[0, 6], [1, 6]]), op=OP.add)
                    mx = tp.tile([16, 6], F32, tag="mx")
                    nc.vector.tensor_reduce(out=mx[:], in_=_ap(tmp[:], [[6, 6], [1, 6]]),
                                            axis=AX.X, op=OP.max)
                    eq = tp.tile([16, 36], F32, tag="eq")
                    nc.vector.tensor_tensor(out=_ap(eq[:], [[6, 6], [1, 6]]),
                                            in0=_ap(tmp[:], [[6, 6], [1, 6]]),
                                            in1=_ap(mx[:], [[1, 6], [0, 6]]), op=OP.is_ge)
                    nc.vector.tensor_tensor(out=eq[:], in0=eq[:], in1=c16_s[:, 36:72], op=OP.mult)
                    if t >= WV:
                        nc.vector.tensor_reduce(out=bpsH[:, (t - WV) * 6:(t - WV + 1) * 6],
                                                in_=_ap(eq[:], [[6, 6], [1, 6]]), axis=AX.X, op=OP.min)
                    nc.vector.tensor_tensor(out=fv[:], in0=mx[:], in1=fsub[:, t * 6:(t + 1) * 6], op=OP.add)
                av = p5.tile([16, 6], F32)
                nc.vector.tensor_tensor(out=av[:], in0=fv[:], in1=c16_s[:, 84:90], op=OP.add)
                am = p5.tile([16, 1], F32)
                nc.vector.tensor_reduce(out=am[:], in_=av[:], axis=AX.X, op=OP.max)
                ohf = p5.tile([16, 6], F32)
                nc.vector.tensor_tensor(out=ohf[:], in0=av[:], in1=am[:].to_broadcast([16, 6]), op=OP.is_ge)
                bpsD = dp.tile([16 * LV * 6], F32)
                nc.sync.dma_start(bpsD[:].rearrange("(p a) -> p a", p=16), bpsH[:])
                bpsR = p5.tile([96, LV * 6], F32)
                for e in range(6):
                    nc.sync.dma_start(bpsR[16 * e:16 * (e + 1), :],
                                      bpsD[:].rearrange("(p a) -> p a", p=16))
                c96t = p5.tile([96, 826], F16)
                nc.sync.dma_start(c96t[:], _dap(bigA[:], [[NA, 96], [1, 826]], extra_off=OC96))
                c96_s = p5.tile([96, 826], F32)
                nc.vector.tensor_copy(out=c96_s[:], in_=c96t[:])
                uH = p5.tile([96, (LV + 1) * 6], F32)
                nc.vector.tensor_copy(out=uH[:, LV * 6:(LV + 1) * 6], in_=c96_s[:, 804:810])
                for tb in range(LV - 1, -1, -1):
                    eqB = tp.tile([96, 36], F32, tag="eqB")
                    nc.vector.tensor_tensor(out=_ap(eqB[:], [[6, 6], [1, 6]]),
                                            in0=_ap(bpsR[:], [[0, 6], [1, 6]], extra_off=tb * 6),
                                            in1=_ap(c96_s[:], [[6, 6], [1, 6]]), op=OP.is_equal)
                    tB = tp.tile([96, 36], F32, tag="tB")
                    nc.vector.tensor_tensor(out=_ap(tB[:], [[6, 6], [1, 6]]),
                                            in0=_ap(eqB[:], [[6, 6], [1, 6]]),
                                            in1=_ap(uH[:], [[0, 6], [1, 6]], extra_off=(tb + 1) * 6),
                                            op=OP.mult)
                    nc.vector.tensor_reduce(out=uH[:, tb * 6:(tb + 1) * 6],
                                            in_=_ap(tB[:], [[6, 6], [1, 6]]), axis=AX.X, op=OP.max)
                idsA = p5.tile([96, LV], F32)
                tJ = p5.tile([96, 768], F32)
                nc.vector.tensor_tensor(out=tJ[:], in0=uH[:, 6:(LV + 1) * 6], in1=c96_s[:, 36:804], op=OP.mult)
                nc.vector.tensor_reduce(out=idsA[:], in_=_ap(tJ[:], [[6, LV], [1, 6]]), axis=AX.X, op=OP.max)
                uD = dp.tile([96 * 6], F32)
                nc.sync.dma_start(uD[:].rearrange("(p a) -> p a", p=96), uH[:, 0:6])
                MT2 = p5.tile([1, 16 * 36], F32)
                nc.sync.dma_start(MT2[:], _dap(uD[:], [[576, 1], [6, 16], [1, 6], [96, 6]]))
                ohfD = dp.tile([16 * 6], F32)
                nc.sync.dma_start(ohfD[:].rearrange("(p a) -> p a", p=16), ohf[:])
                ohSeq = p5.tile([1, 16 * 6], F32)
                nc.sync.dma_start(ohSeq[0:1, 15 * 6:16 * 6],
                                  _dap(ohfD[:], [[6, 1], [1, 6]], extra_off=15 * 6))
                for c in range(14, -1, -1):
                    tS2 = tp.tile([1, 36], F32, tag="tS2")
                    nc.vector.tensor_tensor(out=_ap(tS2[:], [[6, 6], [1, 6]]),
                                            in0=_ap(MT2[:], [[6, 6], [1, 6]], extra_off=(c + 1) * 36),
                                            in1=_ap(ohSeq[:], [[0, 6], [1, 6]], extra_off=(c + 1) * 6),
                                            op=OP.mult)
                    nc.vector.tensor_reduce(out=ohSeq[0:1, c * 6:(c + 1) * 6],
                                            in_=_ap(tS2[:], [[6, 6], [1, 6]]), axis=AX.X, op=OP.max)
                ohD = dp.tile([16 * 6], F32)
                nc.sync.dma_start(ohD[:].rearrange("(p a) -> p a", p=1), ohSeq[:])
                selC = p5.tile([96, 1], F32)
                for e in range(6):
                    nc.sync.dma_start(selC[16 * e:16 * (e + 1), :],
                                      _dap(ohD[:], [[6, 16], [1, 1]], extra_off=e))
                SEL = p5.tile([96, 16], F32)
                nc.vector.tensor_tensor(out=SEL[:], in0=selC[:].to_broadcast([96, 16]),
                                        in1=c96_s[:, 810:826], op=OP.mult)
                psi = psp.tile([16, LV], F32, tag="psi", space="PSUM")
                nc.tensor.matmul(out=psi[:], lhsT=SEL[:], rhs=idsA[:], start=True, stop=True)
                idsI = p5.tile([16, LV], I32)
                nc.vector.tensor_copy(out=idsI[:], in_=psi[:])
                nc.sync.dma_start(ids_o[:].rearrange("(p a) -> p a", p=16), idsI[:])
    nc.compile()
    return nc


def _programs_fused():
    if "fused" not in _cache:
        _cache["fused"] = build_fused()
    return _cache["fused"]


def _vrow_char(p):
    return (p // 1024) * 1024 + (p % 128) * 8 + ((p % 1024) // 128)


def _vrow_word(t):
    return (t // 256) * 256 + (t % 128) * 2 + ((t % 256) // 128)


def _cf_rows_fwd(p):
    kk2 = p // 2048
    pl = p % 2048
    return kk2 * 2048 + (pl % 64) * 32 + (pl // 64)


def _cf_rows_bwd(p):
    pb = (C - 1) - p
    kk2 = pb // 2048
    pl = pb % 2048
    return (4 + kk2) * 2048 + (pl % 64) * 32 + (pl // 64)


def kernel_fused(**inp):
    inp = {k: np.asarray(v) for k, v in inp.items()}
    ncf = _programs_fused()
    perf = {}

    chars = np.asarray(inp["chars"], dtype=np.int32)
    words = np.asarray(inp["words"], dtype=np.int32)
    ix = np.asarray(inp["ix_seq"], dtype=np.int64)
    cemb = np.asarray(inp["char_embed"], dtype=np.float32)
    wemb = np.asarray(inp["word_embed"], dtype=np.float32)

    l1_dir = []
    for d, suf in ((0, "f"), (1, "b")):
        Wih = _reorder(inp[f"c_Wih_{suf}"], CH)
        Whh = _reorder(inp[f"c_Whh_{suf}"], CH)
        bias = _reorder(inp[f"c_bih_{suf}"] + inp[f"c_bhh_{suf}"], CH)
        wih16 = _chunkT(np.ascontiguousarray(Wih.T), 2).astype(np.float16)
        whh16 = np.ascontiguousarray(Whh.T).astype(np.float16)
        l1_dir.append({
            "cwpack": np.concatenate([wih16, whh16], axis=1),   # [128, 1536]
            "bias": np.ascontiguousarray(bias.reshape(4, 128).T.astype(np.float32)),
            "src": chars if d == 0 else chars[::-1],
        })
    CT16 = cemb[chars].astype(np.float16)          # [8192, 256]
    WT16 = wemb[words].astype(np.float16)          # [2048, 1024]
    l2_dir = []
    for d, suf in ((0, "f"), (1, "b")):
        Wih = _reorder(inp[f"w_Wih_{suf}"], WH)
        Whh = _reorder(inp[f"w_Whh_{suf}"], WH)
        bias = _reorder(inp[f"w_bih_{suf}"] + inp[f"w_bhh_{suf}"], WH)
        P = np.concatenate([
            _chunkT(np.ascontiguousarray(Wih[:, 512:].T), 8),
            _chunkT(np.ascontiguousarray(Wih[:, :512].T), 4),
            _chunkT(np.ascontiguousarray(Whh.T), 4),
        ], axis=1).astype(np.float16)
        l2_dir.append({
            "P": P,
            "bias": np.ascontiguousarray(bias.reshape(16, 128).T.astype(np.float32)),
            "wsrc": words if d == 0 else words[::-1],
            "h2t": inp["hid2tag_W"][:, :WH] if d == 0 else inp["hid2tag_W"][:, WH:],
        })
    # viterbi constant packs
    trans = inp["transition"].astype(np.float32)
    c16v = np.zeros((16, 90), np.float32)
    c16v[:, 0:36] = trans.reshape(1, 36)
    c16v[:, 36:72] = (np.arange(36) % 6 - 6).astype(np.float32)[None, :]
    maskV = np.ones((16, 6), np.float32)
    maskV[0] = 0.0
    fillV = np.zeros((16, 6), np.float32)
    fv0 = np.full(6, NEG, np.float32)
    fv0[4] = 0.0
    fillV[0] = fv0
    c16v[:, 72:78] = maskV
    c16v[:, 78:84] = fillV
    c16v[:, 84:90] = trans[:, 5][None, :]
    c96v = np.zeros((96, 826), np.float16)
    c96v[:, 0:36] = (np.arange(36) // 6 - 6).astype(np.float16)[None, :]
    c96v[:, 36:804] = (np.arange(768) % 6).astype(np.float16)[None, :]
    uinit = np.zeros((96, 6), np.float16)
    for e in range(6):
        uinit[16 * e:16 * (e + 1), e] = 1.0
    c96v[:, 804:810] = uinit
    bmask = np.zeros((96, 16), np.float16)
    for e in range(6):
        for c in range(16):
            bmask[16 * e + c, c] = 1.0
    c96v[:, 810:826] = bmask

    in_maps = []
    for core in range(8):
        d, kk = core // 4, core % 4
        d1 = l1_dir[d]
        d2 = l2_dir[d]
        bigA = np.zeros((128, NA), np.float16)
        cstv = np.zeros((128, NC32), np.float32)
        rows = (512 * kk - W2 + np.arange(WIN)).clip(0, T - 1)
        bigA[:, OWSH:OWSH + 8192] = d2["P"][:, kk * 8192:(kk + 1) * 8192]
        bigA[:, OWSH + 8192:OWSH + NSH] = d1["cwpack"][:, kk * 384:(kk + 1) * 384]
        bigA[:, OCT:OCT + 2048] = CT16[1024 * core:1024 * (core + 1)].reshape(
            8, 128, 256).transpose(1, 0, 2).reshape(128, 2048)
        bigA[:, OWT:OWT + 2048] = WT16[256 * core:256 * (core + 1)].reshape(
            2, 128, 1024).transpose(1, 0, 2).reshape(128, 2048)
        bigA[0:96, OC96:OC96 + 826] = c96v
        # cst pack
        cpk1 = np.zeros((128, 100), np.float32)
        cpk1[:, 0:4] = d1["bias"]
        maskH = np.ones((128, LC), np.float32)
        fillH = np.zeros((128, LC), np.float32)
        fillC = np.zeros((128, LC), np.float32)
        if kk == 0:
            maskH[:, 0] = 0.0
            fillH[:, 0] = inp["c_h0"][d]
            fillC[:, 0] = inp["c_c0"][d]
        cpk1[:, 4:4 + LC] = maskH
        cpk1[:, 4 + LC:4 + 2 * LC] = fillH
        cpk1[:, 4 + 2 * LC:4 + 3 * LC] = fillC
        cstv[:, OCPK1:OCPK1 + 100] = cpk1
        cpk2 = np.zeros((128, 430), np.float32)
        cpk2[:, 0:16] = d2["bias"]
        maskH2 = np.ones((128, 4 * LW), np.float32)
        fillH2 = np.zeros((128, 4 * LW), np.float32)
        fillC2 = np.zeros((128, 4 * LW), np.float32)
        if kk == 0:
            for k in range(4):
                maskH2[:, k * LW] = 0.0
                fillH2[:, k * LW] = inp["w_h0"][d][k * 128:(k + 1) * 128]
                fillC2[:, k * LW] = inp["w_c0"][d][k * 128:(k + 1) * 128]
        cpk2[:, 16:144] = maskH2
        cpk2[:, 144:272] = fillH2
        cpk2[:, 272:400] = fillC2
        cpk2[:, 400:424] = _chunkT(np.ascontiguousarray(d2["h2t"].T), 4)
        if d == 0:
            cpk2[:, 424:430] = inp["hid2tag_b"][None, :]
        cstv[:, OCPK2:OCPK2 + 430] = cpk2
        cstv[0:16, OC16:OC16 + 90] = c16v
        rev = np.zeros((128, 128), np.float32)
        if d == 0:
            rev[np.arange(128), np.arange(128)] = 1.0
            cstv[:, OSEL] = 1.0
        else:
            rev[np.arange(128), 127 - np.arange(128)] = 1.0
            cstv[:, OSEL + 1] = 1.0
        cstv[:, OREV:OREV + 128] = rev
        # cf gather indices
        tglob = rows if d == 0 else (T - 1 - rows)
        st = ix[tglob].astype(np.int64)
        en = (ix[tglob + 1] - 1).astype(np.int64)
        idxv = np.zeros((4, 640), np.int32)
        idxv[0, :WIN] = _cf_rows_fwd(st)
        idxv[1, :WIN] = _cf_rows_bwd(st)
        idxv[2, :WIN] = _cf_rows_fwd(en)
        idxv[3, :WIN] = _cf_rows_bwd(en)
        # char/word window table-gather indices
        pdir = (2048 * kk - W1 + np.arange(17 * 128)).clip(0, C - 1)
        pg = pdir if d == 0 else (C - 1 - pdir)
        cidx = _vrow_char(pg).astype(np.int32)
        wdir = (512 * kk - W2 + np.arange(5 * 128)).clip(0, T - 1)
        tg = wdir if d == 0 else (T - 1 - wdir)
        widx = _vrow_word(tg).astype(np.int32)
        in_maps.append({
            "bigA": bigA,
            "cst": cstv,
            "idx": np.concatenate([idxv.reshape(-1), cidx, widx]).reshape(-1, 1),
        })
    t0 = _time.time()
    r = run_bass_kernel_spmd(ncf, in_maps, core_ids=list(range(8)),
                             trace=False, tmpdir=None)
    perf["fused_wall"] = _time.time() - t0
    kernel_fused.last_perf = perf
    return r.results[0]["ids_o"].astype(np.int32)


kernel_fused.last_perf = {}

FUSED = True


def kernel_dispatch(**inp):
    if FUSED:
        out = kernel_fused(**inp)
        kernel.last_perf = kernel_fused.last_perf
        return out
    return _kernel_3launch(**inp)


def kernel(**inp):
    return kernel_dispatch(**inp)


kernel.last_perf = {}
